# revision 18
# baseline (speedup 1.0000x reference)
"""MultiHeadCrossAttention Trainium2 kernel (8 NeuronCores, SPMD).

Problem: B=4, SQ=SK=2048, D=1024, H=16 (HD=64), f32 in/out.

Distribution (3 SPMD launches):
  Phase 1 (row-parallel): QKV projections in fp8 e4m3 with DoubleRow matmuls
    (2 contraction rows per partition -> 0.5 cyc/row). Weights host-prescaled
    by 64; outputs written as scaled fp8 (q*4, k*2, v*1).
  Phase 2 (head-parallel): attention, 2 heads/core. Keys mask-compacted on
    host. Scores S = (4q).(2k) accumulate in PSUM via fp8-DR matmuls over the
    hd=64 contraction split as [32 partitions x 2 slots]; 64*bias (fp8) is
    injected into the same PSUM via a DoubleRow identity matmul, so
    exp(score+bias) needs no elementwise multiply. exp runs split across
    engines: ScalarE true exp (scale=1/64) -> fp8 weights (DR AV matmul), and
    Schraudolph bit-trick exp on DVE/Pool (tensor_scalar -> int16, bitcast to
    bf16 -> plain AV matmul). Mask + normalizer ride as an extra fp8 value
    column; normalization multiplies by 32/norm -> fp8 ctx output.
  Phase 3 (row-parallel): out projection (fp8 DR, ctxT*32 @ woT*64, /2048
    folded into the residual add), one-pass mean/var LayerNorm.
"""

import sys

sys.path.insert(0, "/opt/trn_rl_repo")

import numpy as np
import ml_dtypes

import concourse.bass as bass
import concourse.tile as tile
from concourse import bacc, mybir
from concourse import bass_utils

BF16 = ml_dtypes.bfloat16

B, SQ, SK, D, H = 4, 2048, 2048, 1024, 16
HD = D // H  # 64
NCORES = 8
HPC = H // NCORES          # heads per core = 2
RPC = B * SQ // NCORES     # rows per core (phases 1/3) = 1024
LN_EPS = 1e-5

dt = mybir.dt
AF = mybir.ActivationFunctionType
ALU = mybir.AluOpType
MPM = mybir.MatmulPerfMode

F8 = np.dtype(mybir.dt.np(dt.float8e4))
F32 = np.float32

# Schraudolph fast-exp in bf16 bit space: bf16_bits(exp(x)) ~= x*128*log2e + B
SCH_A = 128.0 * 1.4426950408889634
SCH_B = 127.0 * 128.0 - 0.0436 * 128.0

_programs = {}


# --------------------------------------------------------------------------
# Phase 1: QKV projection (row-parallel, fp8 DoubleRow, no bias on device —
# host folds biases into the outputs if nonzero).
#   inputs (per core): xqT/xkT/xvT [D, RPC] fp8 (input^T), wqT/wkT/wvT
#                      [D, D] fp8 (W^T * 64)
#   outputs: qT_o/kT_o [D, RPC] fp8 (4*q^T, 2*k^T), v_o [RPC, D] fp8 (v)
# --------------------------------------------------------------------------
def build_phase1(reps=1):
    nc = bacc.Bacc("TRN2", debug=False, num_devices=NCORES)
    KC = D // 128  # 8 chunks of 128 = 4 double-chunks

    ins = {}
    for nm in ("xqT", "xkT", "xvT"):
        ins[nm] = nc.dram_tensor(nm, [D, RPC], dt.float8e4, kind="ExternalInput").ap()
    for nm in ("wqT", "wkT", "wvT"):
        ins[nm] = nc.dram_tensor(nm, [D, D], dt.float8e4, kind="ExternalInput").ap()
    qT_o = nc.dram_tensor("qT_o", [D, RPC], dt.float8e4, kind="ExternalOutput").ap()
    kT_o = nc.dram_tensor("kT_o", [D, RPC], dt.float8e4, kind="ExternalOutput").ap()
    v_o = nc.dram_tensor("v_o", [RPC, D], dt.float8e4, kind="ExternalOutput").ap()

    # greedy engine assignment for the 48 PSUM->SBUF scaled copies
    # (GPSIMD/Pool cannot touch PSUM on TRN2, so only Act/DVE)
    eng_cost = {"act": 570.0, "dve": 660.0}
    eng_load = {"act": 0.0, "dve": 0.0}
    copy_plan = []
    for _ in range(3 * (D // 128) * 2):
        e = min(eng_load, key=lambda k: eng_load[k] + eng_cost[k])
        copy_plan.append(e)
        eng_load[e] += eng_cost[e]
    copy_i = [0]

    with tile.TileContext(nc) as tc:
        with (
            tc.tile_pool(name="big", bufs=1) as bigp,
            tc.tile_pool(name="outp", bufs=3) as outp,
            tc.tile_pool(name="ps", bufs=2, space="PSUM") as psp,
        ):
            warm = bigp.tile([1, 1], dt.float32)
            nc.vector.memset(warm[:], 1.0)
            warm2 = bigp.tile([1, 1], dt.float32)
            nc.scalar.activation(warm2[:], warm[:], AF.Copy)
            sb = {}
            for nm in ("xqT", "xkT", "xvT", "wqT", "wkT", "wvT"):
                ncols = ins[nm].shape[1]
                sb[nm] = bigp.tile([128, KC, ncols], dt.float8e4, name=f"{nm}_sb")
            for pair in (("wqT", "xqT"), ("wkT", "xkT"), ("wvT", "xvT")):
                for nm in pair:
                    nc.sync.dma_start(
                        sb[nm][:],
                        ins[nm][:, :].rearrange("(k p) c -> p k c", p=128),
                    )

            def copy_out(dst, src, scale):
                e = copy_plan[copy_i[0] % len(copy_plan)]
                copy_i[0] += 1
                if e == "act":
                    nc.scalar.activation(dst, src, AF.Copy, scale=scale)
                elif e == "dve":
                    nc.vector.tensor_scalar(
                        out=dst, in0=src, scalar1=scale, scalar2=None, op0=ALU.mult
                    )
                else:
                    nc.gpsimd.tensor_scalar(
                        out=dst, in0=src, scalar1=scale, scalar2=None, op0=ALU.mult
                    )

            def proj(x_nm, w_nm, out_dram, transposed_out, scale):
                xt = sb[x_nm]
                wt = sb[w_nm]
                if transposed_out:
                    lt, rt = wt, xt   # out[d_out, rows]
                else:
                    lt, rt = xt, wt   # out[rows, d_out]
                n_m = lt.shape[2] // 128
                n_n = rt.shape[2] // 512
                MG = 2
                for mg in range(0, n_m, MG):
                    ms = range(mg, min(mg + MG, n_m))
                    pss = {}
                    for m in ms:
                        for n in range(n_n):
                            pss[m, n] = psp.tile(
                                [128, 512], dt.float32, name="ps", tag=f"ps{m % MG}_{n}"
                            )
                    for k2 in range(KC // 2):
                        for m in ms:
                            for n in range(n_n):
                                nc.tensor.matmul(
                                    pss[m, n][:],
                                    lhsT=lt[:, 2 * k2 : 2 * k2 + 2, m * 128 : (m + 1) * 128],
                                    rhs=rt[:, 2 * k2 : 2 * k2 + 2, n * 512 : (n + 1) * 512],
                                    start=(k2 == 0),
                                    stop=(k2 == KC // 2 - 1),
                                    perf_mode=MPM.DoubleRow,
                                )
                    osb = outp.tile(
                        [128, MG, rt.shape[2]], dt.float8e4, name=f"{x_nm}_osb", tag="osb"
                    )
                    for m in ms:
                        for n in range(n_n):
                            copy_out(osb[:, m - mg, n * 512 : (n + 1) * 512], pss[m, n][:], scale)
                    nc.scalar.dma_start(
                        out_dram[mg * 128 : (mg + MG) * 128, :].rearrange(
                            "(g p) c -> p g c", p=128
                        ),
                        osb[:],
                    )

            for _ in range(reps):
                proj("xqT", "wqT", qT_o, True, 4.0 / 64.0)
                proj("xkT", "wkT", kT_o, True, 2.0 / 64.0)
                proj("xvT", "wvT", v_o, False, 1.0 / 64.0)

    nc.compile()
    return nc


# --------------------------------------------------------------------------
# Phase 2: attention (head-parallel, 2 heads/core).
#   inputs (per core):
#     qT  [128, B*SQ] fp8  (rows = 2 heads x 64 dims, = 4*q^T)
#     kT  [128, TNV] fp8   (compacted, = 2*k^T)
#     va  [TNV, HPC*(HD+1)] fp8 (v*mask | mask column per head)
#     va16 same as va in bf16 (for the Schraudolph bf16 AV matmuls)
#     eb  [HPC, TNV, SQ] fp8 (64 * bias^T per head, compacted rows)
#     i2  [128, 256] fp8   (DoubleRow identity: [:, :128]=I, [:, 128:]=0)
#   outputs: ctx_o [128, B*SQ] fp8 = 32*ctx/norm in [p, t, d] layout
# --------------------------------------------------------------------------
def build_phase2(nvts=(8, 8, 8, 8), reps=1, sp_bufs=6, cp_bufs=2, wm_bufs=3):
    nc = bacc.Bacc("TRN2", debug=False, num_devices=NCORES)
    QC = 512
    NQC = SQ // QC
    snvt = [0]
    for t in nvts:
        snvt.append(snvt[-1] + t)
    TNT = snvt[-1]
    TNV = TNT * 128
    NTMAX = max(nvts)

    qT = nc.dram_tensor("qT", [128, B * SQ], dt.float8e4, kind="ExternalInput").ap()
    kT = nc.dram_tensor("kT", [128, TNV], dt.float8e4, kind="ExternalInput").ap()
    va = nc.dram_tensor("va", [TNV, HPC * (HD + 1)], dt.float8e4, kind="ExternalInput").ap()
    va16 = nc.dram_tensor("va16", [TNV, HPC * (HD + 1)], dt.bfloat16, kind="ExternalInput").ap()
    eb = nc.dram_tensor("eb", [HPC, TNV, SQ], dt.float8e4, kind="ExternalInput").ap()
    i2 = nc.dram_tensor("i2", [128, 256], dt.float8e4, kind="ExternalInput").ap()
    ctx_o = nc.dram_tensor("ctx_o", [128, B * SQ], dt.float8e4, kind="ExternalOutput").ap()

    with tile.TileContext(nc) as tc:
        with (
            tc.tile_pool(name="big", bufs=1) as bigp,
            tc.tile_pool(name="ebp", bufs=5) as ebp,
            tc.tile_pool(name="wp", bufs=wm_bufs) as wp,
            tc.tile_pool(name="ip", bufs=wm_bufs) as ip,
            tc.tile_pool(name="np_", bufs=6) as normp,
            tc.tile_pool(name="Sp", bufs=sp_bufs, space="PSUM") as Sp,
            tc.tile_pool(name="cp", bufs=cp_bufs, space="PSUM") as cp,
        ):
            # hd-split layouts for DoubleRow: [32 partitions, 2 slots, head, cols]
            qT_sb = bigp.tile([32, 2, HPC, B * SQ], dt.float8e4)
            kT_sb = bigp.tile([32, 2, HPC, TNV], dt.float8e4)
            va_sb = bigp.tile([128, TNT, HPC * (HD + 1)], dt.float8e4)
            va16_sb = bigp.tile([128, TNT, HPC * (HD + 1)], dt.bfloat16)
            i2_sb = bigp.tile([128, 2, 128], dt.float8e4)
            warm = bigp.tile([1, 1], dt.float32)
            nc.vector.memset(warm[:], 0.0)
            warm2 = bigp.tile([1, 1], dt.float32)
            nc.scalar.activation(warm2[:], warm[:], AF.Exp)

            def load_qk(b):
                for h in range(HPC):
                    nc.sync.dma_start(
                        qT_sb[:, :, h, b * SQ : (b + 1) * SQ],
                        qT[64 * h : 64 * h + 64, b * SQ : (b + 1) * SQ].rearrange(
                            "(s p) c -> p s c", p=32
                        ),
                    )
                    cs, ce = snvt[b] * 128, snvt[b + 1] * 128
                    nc.sync.dma_start(
                        kT_sb[:, :, h, cs:ce],
                        kT[64 * h : 64 * h + 64, cs:ce].rearrange("(s p) c -> p s c", p=32),
                    )

            def load_va(b):
                cs, ce = snvt[b] * 128, snvt[b + 1] * 128
                nc.sync.dma_start(
                    va_sb[:, snvt[b] : snvt[b + 1], :],
                    va[cs:ce, :].rearrange("(t p) d -> p t d", p=128),
                )
                nc.sync.dma_start(
                    va16_sb[:, snvt[b] : snvt[b + 1], :],
                    va16[cs:ce, :].rearrange("(t p) d -> p t d", p=128),
                )

            def load_b(b):
                load_qk(b)
                load_va(b)

            load_qk(0)
            load_va(0)
            nc.sync.dma_start(i2_sb[:], i2[:])

            iters = [(qc, b) for qc in range(NQC) for b in range(B)] * reps

            def load_slab(qc, b, split=False):
                NT = nvts[b]
                eb_sb = ebp.tile(
                    [128, NTMAX + 1, HPC, QC], dt.float8e4, name="eb_sb", tag="eb"
                )
                src_r = eb[:, snvt[b] * 128 : snvt[b + 1] * 128, :].rearrange(
                    "h (t p) q -> h p t q", p=128
                )[:, :, :, qc * QC : (qc + 1) * QC]
                if split:
                    for kj in range(NT):
                        for h in range(HPC):
                            nc.sync.dma_start(eb_sb[:, kj, h, :], src_r[h, :, kj, :])
                else:
                    for h in range(HPC):
                        nc.sync.dma_start(eb_sb[:, 0:NT, h, :], src_r[h])
                # pad tile (read by the DR inject's zero slot on the last key
                # tile) must be initialized for the race detector
                nc.gpsimd.memset(eb_sb[:, NT, :, :], 0.0)
                return eb_sb

            slabs = {}
            slabs[0] = load_slab(*iters[0], split=True)
            for b in range(1, B):
                load_b(b)
                slabs[b] = load_slab(*iters[b], split=(b == 1))

            def emit_norm_piece(state):
                # mask column is 1/32, so 1/normcol = 32/sum(w): the x32 ctx
                # scaling is free. Pieces alternate DVE / Act to balance load.
                ctx, col0, holder = state
                if holder[0] is None:
                    holder[0] = normp.tile(
                        [128, QC // 128, HPC * HD], dt.float8e4, name="ctxn", tag="ctxn"
                    )
                ctxn = holder[0]
                t = holder[1]
                holder[1] += 1
                ti, tt = t // 2, t % 2
                for h in range(HPC):
                    rec = normp.tile([128, 1], dt.float32, name="rec", tag="rec")
                    nc.vector.reciprocal(
                        rec[:], ctx[ti][:, tt, h * (HD + 1) + HD : h * (HD + 1) + HD + 1]
                    )
                    if t % 2 == 0:
                        nc.vector.tensor_scalar(
                            out=ctxn[:, t, h * HD : (h + 1) * HD],
                            in0=ctx[ti][:, tt, h * (HD + 1) : h * (HD + 1) + HD],
                            scalar1=rec[:],
                            scalar2=None,
                            op0=ALU.mult,
                        )
                    else:
                        nc.scalar.activation(
                            ctxn[:, t, h * HD : (h + 1) * HD],
                            ctx[ti][:, tt, h * (HD + 1) : h * (HD + 1) + HD],
                            AF.Copy,
                            scale=rec[:],
                        )
                if t == QC // 128 - 1:
                    nc.scalar.dma_start(ctx_o[:, col0 : col0 + QC], ctxn[:])

            def emit_norm(state):
                while state[2][1] < QC // 128:
                    emit_norm_piece(state)

            def emit_av_pair(ctx, tbase, pj, wm2, start, stop):
                # DoubleRow fp8 AV over a kj pair
                for ti in range(QC // 256):
                    for tt in range(2):
                        for h in range(HPC):
                            t = ti * 2 + tt
                            nc.tensor.matmul(
                                ctx[ti][:, tt, h * (HD + 1) : (h + 1) * (HD + 1)],
                                lhsT=wm2[:, :, h, t * 128 : (t + 1) * 128],
                                rhs=va_sb[:, tbase + 2 * pj : tbase + 2 * pj + 2,
                                          h * (HD + 1) : (h + 1) * (HD + 1)],
                                start=start and (tt == 0) and (h == 0),
                                stop=stop and (ti == QC // 256 - 1) and (tt == 1) and (h == HPC - 1),
                                perf_mode=MPM.DoubleRow,
                                skip_group_check=True,
                            )

            def emit_av_sch(ctx, tbase, kj, i16, sl, start, stop):
                # plain bf16 AV for one Schraudolph kj tile (bitcast int16 weights)
                for ti in range(QC // 256):
                    for tt in range(2):
                        for h in range(HPC):
                            t = ti * 2 + tt
                            nc.tensor.matmul(
                                ctx[ti][:, tt, h * (HD + 1) : (h + 1) * (HD + 1)],
                                lhsT=i16[:, sl, h, t * 128 : (t + 1) * 128].bitcast(dt.bfloat16),
                                rhs=va16_sb[:, tbase + kj, h * (HD + 1) : (h + 1) * (HD + 1)],
                                start=start and (tt == 0) and (h == 0),
                                stop=stop and (ti == QC // 256 - 1) and (tt == 1) and (h == HPC - 1),
                                skip_group_check=True,
                            )

            def emit_av_one(ctx, tbase, kj, wm1, start, stop):
                # plain fp8 AV for the odd tail tile
                for ti in range(QC // 256):
                    for tt in range(2):
                        for h in range(HPC):
                            t = ti * 2 + tt
                            nc.tensor.matmul(
                                ctx[ti][:, tt, h * (HD + 1) : (h + 1) * (HD + 1)],
                                lhsT=wm1[:, 0, h, t * 128 : (t + 1) * 128],
                                rhs=va_sb[:, tbase + kj, h * (HD + 1) : (h + 1) * (HD + 1)],
                                start=start and (tt == 0) and (h == 0),
                                stop=stop and (ti == QC // 256 - 1) and (tt == 1) and (h == HPC - 1),
                                skip_group_check=True,
                            )

            tail_av = []     # AV thunks deferred from the previous kj
            tail_norm = None

            for it_i, (qc, b) in enumerate(iters):
                NT = nvts[b]
                NP = NT // 2
                eb_sb = slabs.pop(it_i)
                if it_i + 4 < len(iters):
                    slabs[it_i + 4] = load_slab(*iters[it_i + 4])
                ctx = [
                    cp.tile([128, 2, HPC * (HD + 1)], dt.float32, name=f"ctx{t}", tag="ctx")
                    for t in range(QC // 256)
                ]
                col0 = b * SQ + qc * QC
                tbase = snvt[b]

                def make_S(kj):
                    # per-head 1-bank S tiles -> deeper PSUM pipeline
                    Ss = []
                    kcol = tbase * 128 + kj * 128
                    for h in range(HPC):
                        S = Sp.tile([128, QC], dt.float32, name="S", tag="S")
                        nc.tensor.matmul(
                            S[:],
                            lhsT=kT_sb[:, :, h, kcol : kcol + 128],
                            rhs=qT_sb[:, :, h, col0 : col0 + QC],
                            start=True,
                            stop=False,
                            perf_mode=MPM.DoubleRow,
                            skip_group_check=True,
                        )
                        nc.tensor.matmul(
                            S[:],
                            lhsT=i2_sb[:],
                            rhs=eb_sb[:, kj : kj + 2, h, :],
                            start=False,
                            stop=True,
                            perf_mode=MPM.DoubleRow,
                            skip_group_check=True,
                        )
                        Ss.append(S)
                    return Ss

                first_av = [True]
                wm2_cur = [None]
                i16_cur = [None]
                for kj in range(NT):
                    S = make_S(kj)
                    # drain the deferred AVs / previous iteration's norm
                    if tail_av:
                        fin = (kj == 0)
                        for j, (fn, args) in enumerate(tail_av):
                            fn(*args, stop=(fin and j == len(tail_av) - 1) if fin else False)
                        tail_av = []
                    if tail_norm is not None and kj >= 1:
                        emit_norm_piece(tail_norm)
                        if tail_norm[2][1] >= QC // 128:
                            tail_norm = None
                    pj = kj // 2
                    is_odd_tail = (kj == NT - 1) and (NT % 2 == 1)
                    path = "act" if (is_odd_tail or pj % 2 == 0) else "sch"
                    if path == "act":
                        if is_odd_tail:
                            wm1 = wp.tile([128, 1, HPC, QC], dt.float8e4, name="wm1", tag="wm1")
                            for h in range(HPC):
                                nc.scalar.activation(
                                    wm1[:, 0, h], S[h][:], AF.Exp, scale=1.0 / 64.0
                                )
                            tail_av.append((emit_av_one, [ctx, tbase, kj, wm1, first_av[0]]))
                            first_av[0] = False
                        else:
                            if kj % 2 == 0:
                                wm2_cur[0] = wp.tile(
                                    [128, 2, HPC, QC], dt.float8e4, name="wm2", tag="wm2"
                                )
                            for h in range(HPC):
                                nc.scalar.activation(
                                    wm2_cur[0][:, kj % 2, h], S[h][:], AF.Exp, scale=1.0 / 64.0
                                )
                            if kj % 2 == 1:
                                tail_av.append(
                                    (emit_av_pair, [ctx, tbase, pj, wm2_cur[0], first_av[0]])
                                )
                                first_av[0] = False
                    else:
                        if kj % 2 == 0 or is_odd_tail:
                            i16_cur[0] = ip.tile(
                                [128, 2, HPC, QC], dt.int16, name="i16", tag="i16"
                            )
                        sl_ = 0 if is_odd_tail else kj % 2
                        for h in range(HPC):
                            nc.vector.tensor_scalar(
                                out=i16_cur[0][:, sl_, h], in0=S[h][:], scalar1=SCH_A / 64.0,
                                scalar2=SCH_B, op0=ALU.mult, op1=ALU.add,
                            )
                        tail_av.append(
                            (emit_av_sch, [ctx, tbase, kj, i16_cur[0], sl_, first_av[0]])
                        )
                        first_av[0] = False

                if tail_norm is not None:
                    emit_norm(tail_norm)   # short iterations: flush leftovers
                tail_norm = (ctx, col0, [None, 0])

            for j, (fn, args) in enumerate(tail_av):
                fn(*args, stop=(j == len(tail_av) - 1))
            emit_norm(tail_norm)

    nc.compile()
    return nc


# --------------------------------------------------------------------------
# Phase 3: out projection + residual + LayerNorm (row-parallel, fp8 DR GEMM).
#   inputs (per core): ctxT [D, RPC] fp8 (=32*ctx^T), woT [D, D] fp8 (=64*Wo^T),
#     resid [RPC, D] bf16 (query rows + bo), [gammab/betab [128, D] f32 if
#     not trivial_ln]
#   outputs: out_o [RPC, D] f32
# --------------------------------------------------------------------------
def build_phase3(trivial_ln=True, reps=1):
    nc = bacc.Bacc("TRN2", debug=False, num_devices=NCORES)
    KC = D // 128

    ctxT = nc.dram_tensor("ctxT", [D, RPC], dt.float8e4, kind="ExternalInput").ap()
    woT = nc.dram_tensor("woT", [D, D], dt.float8e4, kind="ExternalInput").ap()
    resid = nc.dram_tensor("resid", [RPC, D], dt.bfloat16, kind="ExternalInput").ap()
    if not trivial_ln:
        gammab = nc.dram_tensor("gammab", [128, D], dt.float32, kind="ExternalInput").ap()
        betab = nc.dram_tensor("betab", [128, D], dt.float32, kind="ExternalInput").ap()
    out_o = nc.dram_tensor("out_o", [RPC, D], dt.float32, kind="ExternalOutput").ap()
    PS_SCALE = 1.0 / (32.0 * 64.0)

    with tile.TileContext(nc) as tc:
        with (
            tc.tile_pool(name="big", bufs=1) as bigp,
            tc.tile_pool(name="rp", bufs=4) as rp,
            tc.tile_pool(name="wk", bufs=3) as wk,
            tc.tile_pool(name="ps", bufs=6, space="PSUM") as psp,
        ):
            ctx_sb = bigp.tile([128, KC, RPC], dt.float8e4)
            wo_sb = bigp.tile([128, KC, D], dt.float8e4)
            nc.sync.dma_start(
                ctx_sb[:], ctxT[:, :].rearrange("(k p) c -> p k c", p=128)
            )
            nc.sync.dma_start(
                wo_sb[:], woT[:, :].rearrange("(k p) c -> p k c", p=128)
            )
            eps_sb = bigp.tile([128, 1], dt.float32)
            nc.vector.memset(eps_sb[:], LN_EPS)
            warm = bigp.tile([1, 1], dt.float32)
            nc.vector.memset(warm[:], 1.0)
            warm2 = bigp.tile([1, 1], dt.float32)
            nc.scalar.activation(warm2[:], warm[:], AF.Sqrt)
            warm3 = bigp.tile([1, 1], dt.float32)
            nc.scalar.activation(warm3[:], warm[:], AF.Square)
            if not trivial_ln:
                gam_sb = bigp.tile([128, D], dt.float32)
                nc.sync.dma_start(gam_sb[:], gammab[:])
                bet_sb = bigp.tile([128, D], dt.float32)
                nc.sync.dma_start(bet_sb[:], betab[:])

            for m in [m for _ in range(reps) for m in range(RPC // 128)]:
                res_sb = rp.tile([128, D], dt.bfloat16, name="res_sb", tag="res")
                nc.sync.dma_start(res_sb[:], resid[m * 128 : (m + 1) * 128, :])
                ps = [psp.tile([128, 512], dt.float32, name=f"ps{n}", tag="ps") for n in range(2)]
                for n in range(2):
                    for k2 in range(KC // 2):
                        nc.tensor.matmul(
                            ps[n][:],
                            lhsT=ctx_sb[:, 2 * k2 : 2 * k2 + 2, m * 128 : (m + 1) * 128],
                            rhs=wo_sb[:, 2 * k2 : 2 * k2 + 2, n * 512 : (n + 1) * 512],
                            start=(k2 == 0),
                            stop=(k2 == KC // 2 - 1),
                            perf_mode=MPM.DoubleRow,
                        )
                x_sb = wk.tile([128, D], dt.float32, name="x_sb", tag="x")
                acc = [wk.tile([128, 1], dt.float32, name=f"acc{n}", tag=f"acc{n}") for n in range(2)]
                for n in range(2):
                    nc.vector.scalar_tensor_tensor(
                        out=x_sb[:, n * 512 : (n + 1) * 512],
                        in0=ps[n][:],
                        scalar=PS_SCALE,
                        in1=res_sb[:, n * 512 : (n + 1) * 512],
                        op0=ALU.mult,
                        op1=ALU.add,
                        accum_out=acc[n][:],
                    )
                mu = wk.tile([128, 1], dt.float32, name="mu", tag="mu")
                nc.vector.tensor_scalar(
                    out=mu[:], in0=acc[0][:], scalar1=acc[1][:], scalar2=1.0 / D,
                    op0=ALU.add, op1=ALU.mult,
                )
                sq = wk.tile([128, D], dt.bfloat16, name="sq", tag="sq")
                s2 = wk.tile([128, 1], dt.float32, name="s2", tag="s2")
                nc.scalar.activation(sq[:], x_sb[:], AF.Square, accum_out=s2[:])
                var = wk.tile([128, 1], dt.float32, name="var", tag="var")
                # var = s2/D - mu^2  (one fused op: (s2*(1/D)) - mu2)
                mu2 = wk.tile([128, 1], dt.float32, name="mu2", tag="mu2")
                nc.vector.tensor_tensor(mu2[:], mu[:], mu[:], op=ALU.mult)
                nc.vector.tensor_scalar(
                    out=var[:], in0=s2[:], scalar1=1.0 / D, scalar2=mu2[:],
                    op0=ALU.mult, op1=ALU.subtract,
                )
                std = wk.tile([128, 1], dt.float32, name="std", tag="std")
                nc.scalar.activation(std[:], var[:], AF.Sqrt, bias=eps_sb[:])
                rstd = wk.tile([128, 1], dt.float32, name="rstd", tag="rstd")
                nc.vector.reciprocal(rstd[:], std[:])
                mrs = wk.tile([128, 1], dt.float32, name="mrs", tag="mrs")
                nc.vector.tensor_tensor(mrs[:], mu[:], rstd[:], op=ALU.mult)
                out_sb = wk.tile([128, D], dt.float32, name="out_sb", tag="out_sb")
                if trivial_ln:
                    nc.vector.tensor_scalar(
                        out=out_sb[:], in0=x_sb[:], scalar1=rstd[:], scalar2=mrs[:],
                        op0=ALU.mult, op1=ALU.subtract,
                    )
                else:
                    tmp = wk.tile([128, D], dt.float32, name="tmp", tag="tmp")
                    nc.vector.tensor_scalar(
                        out=tmp[:], in0=x_sb[:], scalar1=rstd[:], scalar2=mrs[:],
                        op0=ALU.mult, op1=ALU.subtract,
                    )
                    y = wk.tile([128, D], dt.float32, name="y", tag="y")
                    nc.vector.scalar_tensor_tensor(
                        out=y[:], in0=tmp[:], scalar=0.0, in1=gam_sb[:],
                        op0=ALU.add, op1=ALU.mult,
                    )
                    nc.gpsimd.tensor_add(out_sb[:], y[:], bet_sb[:])
                nc.sync.dma_start(out_o[m * 128 : (m + 1) * 128, :], out_sb[:])

    nc.compile()
    return nc


def _get_program(key, builder, *args, **kwargs):
    if key not in _programs:
        _programs[key] = builder(*args, **kwargs)
    return _programs[key]


def _run(nc, in_maps):
    return bass_utils.run_bass_kernel_spmd(nc, in_maps, core_ids=list(range(NCORES)))


def kernel(query, key, value, attention_mask, relative_position_bias,
           Wq, bq, Wk, bk, Wv, bv, Wo, bo, ln_gamma, ln_beta,
           _collect_results=None):
    query = np.asarray(query, dtype=np.float32)
    key = np.asarray(key, dtype=np.float32)
    value = np.asarray(value, dtype=np.float32)
    attention_mask = np.asarray(attention_mask)
    relative_position_bias = np.asarray(relative_position_bias, dtype=np.float32)

    def xT8(x):
        return np.ascontiguousarray(x.reshape(-1, D).T).astype(F8)

    def wT8(W, scale):
        return (np.ascontiguousarray(np.asarray(W, np.float32).T) * scale).astype(F8)

    xqT = xT8(query)
    xkT = xT8(key)
    xvT = xT8(value)
    wqT = wT8(Wq, 64.0)
    wkT = wT8(Wk, 64.0)
    wvT = wT8(Wv, 64.0)

    # ---------------- phase 1 ----------------
    in1 = []
    for c in range(NCORES):
        sl = slice(c * RPC, (c + 1) * RPC)
        in1.append({
            "xqT": np.ascontiguousarray(xqT[:, sl]),
            "xkT": np.ascontiguousarray(xkT[:, sl]),
            "xvT": np.ascontiguousarray(xvT[:, sl]),
            "wqT": wqT, "wkT": wkT, "wvT": wvT,
        })
    r1 = _run(_get_program("p1", build_phase1), in1)

    qT_full = np.empty((D, B * SQ), dtype=F8)
    kT_full = np.empty((D, B * SK), dtype=F8)
    v_full = np.empty((B * SK, D), dtype=F8)
    for c in range(NCORES):
        sl = slice(c * RPC, (c + 1) * RPC)
        qT_full[:, sl] = r1.results[c]["qT_o"]
        kT_full[:, sl] = r1.results[c]["kT_o"]
        v_full[sl, :] = r1.results[c]["v_o"]

    # fold any nonzero projection biases in on the host (zero in practice)
    if np.any(np.asarray(bq)):
        qT_full = (qT_full.astype(np.float32)
                   + 4.0 * np.asarray(bq, np.float32)[:, None]).astype(F8)
    if np.any(np.asarray(bk)):
        kT_full = (kT_full.astype(np.float32)
                   + 2.0 * np.asarray(bk, np.float32)[:, None]).astype(F8)
    if np.any(np.asarray(bv)):
        v_full = (v_full.astype(np.float32)
                  + np.asarray(bv, np.float32)[None, :]).astype(F8)

    # ---------------- phase 2 ----------------
    mask2 = (attention_mask.reshape(B, SK) != 0)
    valid = [np.nonzero(mask2[b])[0] for b in range(B)]
    nvts = tuple(max(1, -(-len(ix) // 128)) for ix in valid)
    snvt = np.concatenate([[0], np.cumsum(nvts)]).astype(int)
    TNT = int(snvt[-1])
    idx_pad = np.zeros(TNT * 128, dtype=np.int64)
    maskc = np.zeros((TNT * 128,), dtype=bool)
    for b in range(B):
        ix = valid[b]
        o = snvt[b] * 128
        idx_pad[o : o + len(ix)] = ix
        maskc[o : o + len(ix)] = True

    col_idx = (np.repeat(np.arange(B) * SK, np.array(nvts) * 128) + idx_pad)
    kT_c = np.ascontiguousarray(kT_full[:, col_idx])
    v_rows = v_full[col_idx, :]
    va_all = np.zeros((TNT * 128, H * (HD + 1)), dtype=F8)
    inv32 = np.asarray(1.0 / 32.0, dtype=F8)[()]
    for h in range(H):
        blk = np.where(maskc[:, None], v_rows[:, h * HD : (h + 1) * HD], np.zeros((), F8))
        va_all[:, h * (HD + 1) : h * (HD + 1) + HD] = blk
        va_all[:, h * (HD + 1) + HD] = np.where(maskc, inv32, np.zeros((), F8))

    ebT8 = (np.ascontiguousarray(
        relative_position_bias[0].transpose(0, 2, 1)) * 64.0).astype(F8)
    eb_c = ebT8[:, idx_pad, :]  # [H, TNV, SQ] fp8

    i2_host = np.zeros((128, 256), dtype=F8)
    i2_host[:, 0:128] = np.eye(128, dtype=np.float32).astype(F8)

    in2 = []
    for c in range(NCORES):
        rs = slice(c * 128, (c + 1) * 128)
        in2.append({
            "qT": np.ascontiguousarray(qT_full[rs, :]),
            "kT": np.ascontiguousarray(kT_c[rs, :]),
            "va": np.ascontiguousarray(
                va_all[:, c * HPC * (HD + 1) : (c + 1) * HPC * (HD + 1)]
            ),
            "va16": np.ascontiguousarray(
                va_all[:, c * HPC * (HD + 1) : (c + 1) * HPC * (HD + 1)]
            ).astype(BF16),
            "eb": np.ascontiguousarray(eb_c[c * HPC : (c + 1) * HPC]),
            "i2": i2_host,
        })
    r2 = _run(_get_program(("p2",) + nvts, build_phase2, nvts), in2)

    # ctx_o[c] is [128 q-part, t, 128 d] for d-block c -> assemble ctxT [D, B*SQ]
    ctxT_full = np.empty((D, B * SQ), dtype=F8)
    for c in range(NCORES):
        blk = r2.results[c]["ctx_o"].reshape(128, B * SQ // 128, 128)
        ctxT_full[c * 128 : (c + 1) * 128, :] = (
            blk.transpose(2, 1, 0).reshape(128, B * SQ)
        )

    # ---------------- phase 3 ----------------
    woT8 = wT8(Wo, 64.0)
    q2d = query.reshape(-1, D)
    resid_h = (q2d + np.asarray(bo, np.float32)[None, :]).astype(BF16)
    trivial = (not np.any(np.asarray(ln_beta))) and np.all(
        np.asarray(ln_gamma, np.float32) == 1.0
    )
    in3 = []
    for c in range(NCORES):
        sl = slice(c * RPC, (c + 1) * RPC)
        d = {
            "ctxT": np.ascontiguousarray(ctxT_full[:, sl]),
            "woT": woT8,
            "resid": np.ascontiguousarray(resid_h[sl, :]),
        }
        if not trivial:
            d["gammab"] = np.ascontiguousarray(
                np.broadcast_to(np.asarray(ln_gamma, np.float32)[None, :], (128, D))
            )
            d["betab"] = np.ascontiguousarray(
                np.broadcast_to(np.asarray(ln_beta, np.float32)[None, :], (128, D))
            )
        in3.append(d)
    r3 = _run(_get_program(("p3", trivial), build_phase3, trivial), in3)

    out = np.empty((B * SQ, D), dtype=np.float32)
    for c in range(NCORES):
        out[c * RPC : (c + 1) * RPC, :] = r3.results[c]["out_o"]

    if _collect_results is not None:
        _collect_results.extend([r1, r2, r3])
    return out.reshape(B, SQ, D)


# revision 19
# speedup vs baseline: 1.0048x; 1.0048x over previous
"""MultiHeadCrossAttention Trainium2 kernel (8 NeuronCores, SPMD).

Problem: B=4, SQ=SK=2048, D=1024, H=16 (HD=64), f32 in/out.

Distribution (3 SPMD launches):
  Phase 1 (row-parallel): QKV projections in fp8 e4m3 with DoubleRow matmuls
    (2 contraction rows per partition -> 0.5 cyc/row). Weights host-prescaled
    by 64; outputs written as scaled fp8 (q*4, k*2, v*1).
  Phase 2 (head-parallel): attention, 2 heads/core. Keys mask-compacted on
    host. Scores S = (4q).(2k) accumulate in PSUM via fp8-DR matmuls over the
    hd=64 contraction split as [32 partitions x 2 slots]; 64*bias (fp8) is
    injected into the same PSUM via a DoubleRow identity matmul, so
    exp(score+bias) needs no elementwise multiply. exp runs split across
    engines: ScalarE true exp (scale=1/64) -> fp8 weights (DR AV matmul), and
    Schraudolph bit-trick exp on DVE/Pool (tensor_scalar -> int16, bitcast to
    bf16 -> plain AV matmul). Mask + normalizer ride as an extra fp8 value
    column; normalization multiplies by 32/norm -> fp8 ctx output.
  Phase 3 (row-parallel): out projection (fp8 DR, ctxT*32 @ woT*64, /2048
    folded into the residual add), one-pass mean/var LayerNorm.
"""

import sys

sys.path.insert(0, "/opt/trn_rl_repo")

import numpy as np
import ml_dtypes

import concourse.bass as bass
import concourse.tile as tile
from concourse import bacc, mybir
from concourse import bass_utils

BF16 = ml_dtypes.bfloat16

B, SQ, SK, D, H = 4, 2048, 2048, 1024, 16
HD = D // H  # 64
NCORES = 8
HPC = H // NCORES          # heads per core = 2
RPC = B * SQ // NCORES     # rows per core (phases 1/3) = 1024
LN_EPS = 1e-5

dt = mybir.dt
AF = mybir.ActivationFunctionType
ALU = mybir.AluOpType
MPM = mybir.MatmulPerfMode

F8 = np.dtype(mybir.dt.np(dt.float8e4))
F32 = np.float32

# Schraudolph fast-exp in bf16 bit space: bf16_bits(exp(x)) ~= x*128*log2e + B
SCH_A = 128.0 * 1.4426950408889634
SCH_B = 127.0 * 128.0 - 0.0436 * 128.0

_programs = {}


# --------------------------------------------------------------------------
# Phase 1: QKV projection (row-parallel, fp8 DoubleRow, no bias on device —
# host folds biases into the outputs if nonzero).
#   inputs (per core): xqT/xkT/xvT [D, RPC] fp8 (input^T), wqT/wkT/wvT
#                      [D, D] fp8 (W^T * 64)
#   outputs: qT_o/kT_o [D, RPC] fp8 (4*q^T, 2*k^T), v_o [RPC, D] fp8 (v)
# --------------------------------------------------------------------------
def build_phase1(reps=1):
    nc = bacc.Bacc("TRN2", debug=False, num_devices=NCORES)
    KC = D // 128  # 8 chunks of 128 = 4 double-chunks

    ins = {}
    for nm in ("xqT", "xkT", "xvT"):
        ins[nm] = nc.dram_tensor(nm, [D, RPC], dt.float8e4, kind="ExternalInput").ap()
    for nm in ("wqT", "wkT", "wvT"):
        ins[nm] = nc.dram_tensor(nm, [D, D], dt.float8e4, kind="ExternalInput").ap()
    qT_o = nc.dram_tensor("qT_o", [D, RPC], dt.float8e4, kind="ExternalOutput").ap()
    kT_o = nc.dram_tensor("kT_o", [D, RPC], dt.float8e4, kind="ExternalOutput").ap()
    v_o = nc.dram_tensor("v_o", [RPC, D], dt.float8e4, kind="ExternalOutput").ap()

    # greedy engine assignment for the 48 PSUM->SBUF scaled copies
    # (GPSIMD/Pool cannot touch PSUM on TRN2, so only Act/DVE)
    eng_cost = {"act": 570.0, "dve": 660.0}
    eng_load = {"act": 0.0, "dve": 0.0}
    copy_plan = []
    for _ in range(3 * (D // 128) * 2):
        e = min(eng_load, key=lambda k: eng_load[k] + eng_cost[k])
        copy_plan.append(e)
        eng_load[e] += eng_cost[e]
    copy_i = [0]

    with tile.TileContext(nc) as tc:
        with (
            tc.tile_pool(name="big", bufs=1) as bigp,
            tc.tile_pool(name="outp", bufs=3) as outp,
            tc.tile_pool(name="ps", bufs=2, space="PSUM") as psp,
        ):
            warm = bigp.tile([1, 1], dt.float32)
            nc.vector.memset(warm[:], 1.0)
            warm2 = bigp.tile([1, 1], dt.float32)
            nc.scalar.activation(warm2[:], warm[:], AF.Copy)
            sb = {}
            for nm in ("xqT", "xkT", "xvT", "wqT", "wkT", "wvT"):
                ncols = ins[nm].shape[1]
                sb[nm] = bigp.tile([128, KC, ncols], dt.float8e4, name=f"{nm}_sb")
            for pair in (("wqT", "xqT"), ("wkT", "xkT"), ("wvT", "xvT")):
                for nm in pair:
                    nc.sync.dma_start(
                        sb[nm][:],
                        ins[nm][:, :].rearrange("(k p) c -> p k c", p=128),
                    )

            def copy_out(dst, src, scale):
                e = copy_plan[copy_i[0] % len(copy_plan)]
                copy_i[0] += 1
                if e == "act":
                    nc.scalar.activation(dst, src, AF.Copy, scale=scale)
                elif e == "dve":
                    nc.vector.tensor_scalar(
                        out=dst, in0=src, scalar1=scale, scalar2=None, op0=ALU.mult
                    )
                else:
                    nc.gpsimd.tensor_scalar(
                        out=dst, in0=src, scalar1=scale, scalar2=None, op0=ALU.mult
                    )

            def proj(x_nm, w_nm, out_dram, transposed_out, scale):
                xt = sb[x_nm]
                wt = sb[w_nm]
                if transposed_out:
                    lt, rt = wt, xt   # out[d_out, rows]
                else:
                    lt, rt = xt, wt   # out[rows, d_out]
                n_m = lt.shape[2] // 128
                n_n = rt.shape[2] // 512
                MG = 2
                for mg in range(0, n_m, MG):
                    ms = range(mg, min(mg + MG, n_m))
                    pss = {}
                    for m in ms:
                        for n in range(n_n):
                            pss[m, n] = psp.tile(
                                [128, 512], dt.float32, name="ps", tag=f"ps{m % MG}_{n}"
                            )
                    for k2 in range(KC // 2):
                        for m in ms:
                            for n in range(n_n):
                                nc.tensor.matmul(
                                    pss[m, n][:],
                                    lhsT=lt[:, 2 * k2 : 2 * k2 + 2, m * 128 : (m + 1) * 128],
                                    rhs=rt[:, 2 * k2 : 2 * k2 + 2, n * 512 : (n + 1) * 512],
                                    start=(k2 == 0),
                                    stop=(k2 == KC // 2 - 1),
                                    perf_mode=MPM.DoubleRow,
                                )
                    osb = outp.tile(
                        [128, MG, rt.shape[2]], dt.float8e4, name=f"{x_nm}_osb", tag="osb"
                    )
                    for m in ms:
                        for n in range(n_n):
                            copy_out(osb[:, m - mg, n * 512 : (n + 1) * 512], pss[m, n][:], scale)
                    nc.scalar.dma_start(
                        out_dram[mg * 128 : (mg + MG) * 128, :].rearrange(
                            "(g p) c -> p g c", p=128
                        ),
                        osb[:],
                    )

            for _ in range(reps):
                proj("xqT", "wqT", qT_o, True, 4.0 / 64.0)
                proj("xkT", "wkT", kT_o, True, 2.0 / 64.0)
                proj("xvT", "wvT", v_o, False, 1.0 / 64.0)

    nc.compile()
    return nc


# --------------------------------------------------------------------------
# Phase 2: attention (head-parallel, 2 heads/core).
#   inputs (per core):
#     qT  [128, B*SQ] fp8  (rows = 2 heads x 64 dims, = 4*q^T)
#     kT  [128, TNV] fp8   (compacted, = 2*k^T)
#     va  [TNV, HPC*(HD+1)] fp8 (v*mask | mask column per head)
#     va16 same as va in bf16 (for the Schraudolph bf16 AV matmuls)
#     eb  [HPC, TNV, SQ] fp8 (64 * bias^T per head, compacted rows)
#     i2  [128, 256] fp8   (DoubleRow identity: [:, :128]=I, [:, 128:]=0)
#   outputs: ctx_o [128, B*SQ] fp8 = 32*ctx/norm in [p, t, d] layout
# --------------------------------------------------------------------------
def build_phase2(nvts=(8, 8, 8, 8), reps=1, sp_bufs=6, cp_bufs=2, wm_bufs=3):
    nc = bacc.Bacc("TRN2", debug=False, num_devices=NCORES)
    QC = 512
    NQC = SQ // QC
    snvt = [0]
    for t in nvts:
        snvt.append(snvt[-1] + t)
    TNT = snvt[-1]
    TNV = TNT * 128
    NTMAX = max(nvts)

    qT = nc.dram_tensor("qT", [128, B * SQ], dt.float8e4, kind="ExternalInput").ap()
    kT = nc.dram_tensor("kT", [128, TNV], dt.float8e4, kind="ExternalInput").ap()
    va = nc.dram_tensor("va", [TNV, HPC * (HD + 1)], dt.float8e4, kind="ExternalInput").ap()
    va16 = nc.dram_tensor("va16", [TNV, HPC * (HD + 1)], dt.bfloat16, kind="ExternalInput").ap()
    eb = nc.dram_tensor("eb", [HPC, TNV, SQ], dt.float8e4, kind="ExternalInput").ap()
    i2 = nc.dram_tensor("i2", [128, 256], dt.float8e4, kind="ExternalInput").ap()
    ctx_o = nc.dram_tensor("ctx_o", [128, B * SQ], dt.float8e4, kind="ExternalOutput").ap()

    with tile.TileContext(nc) as tc:
        with (
            tc.tile_pool(name="big", bufs=1) as bigp,
            tc.tile_pool(name="ebp", bufs=5) as ebp,
            tc.tile_pool(name="wp", bufs=wm_bufs) as wp,
            tc.tile_pool(name="ip", bufs=wm_bufs) as ip,
            tc.tile_pool(name="np_", bufs=6) as normp,
            tc.tile_pool(name="Sp", bufs=sp_bufs, space="PSUM") as Sp,
            tc.tile_pool(name="cp", bufs=cp_bufs, space="PSUM") as cp,
        ):
            # hd-split layouts for DoubleRow: [32 partitions, 2 slots, head, cols]
            qT_sb = bigp.tile([32, 2, HPC, B * SQ], dt.float8e4)
            kT_sb = bigp.tile([32, 2, HPC, TNV], dt.float8e4)
            va_sb = bigp.tile([128, TNT, HPC * (HD + 1)], dt.float8e4)
            va16_sb = bigp.tile([128, TNT, HPC * (HD + 1)], dt.bfloat16)
            i2_sb = bigp.tile([128, 2, 128], dt.float8e4)
            warm = bigp.tile([1, 1], dt.float32)
            nc.vector.memset(warm[:], 0.0)
            warm2 = bigp.tile([1, 1], dt.float32)
            nc.scalar.activation(warm2[:], warm[:], AF.Exp)

            def load_qk(b):
                for h in range(HPC):
                    nc.sync.dma_start(
                        qT_sb[:, :, h, b * SQ : (b + 1) * SQ],
                        qT[64 * h : 64 * h + 64, b * SQ : (b + 1) * SQ].rearrange(
                            "(s p) c -> p s c", p=32
                        ),
                    )
                    cs, ce = snvt[b] * 128, snvt[b + 1] * 128
                    nc.sync.dma_start(
                        kT_sb[:, :, h, cs:ce],
                        kT[64 * h : 64 * h + 64, cs:ce].rearrange("(s p) c -> p s c", p=32),
                    )

            def load_va(b):
                cs, ce = snvt[b] * 128, snvt[b + 1] * 128
                nc.sync.dma_start(
                    va_sb[:, snvt[b] : snvt[b + 1], :],
                    va[cs:ce, :].rearrange("(t p) d -> p t d", p=128),
                )
                nc.sync.dma_start(
                    va16_sb[:, snvt[b] : snvt[b + 1], :],
                    va16[cs:ce, :].rearrange("(t p) d -> p t d", p=128),
                )

            def load_b(b):
                load_qk(b)
                load_va(b)

            load_qk(0)
            nc.sync.dma_start(i2_sb[:], i2[:])

            iters = [(qc, b) for qc in range(NQC) for b in range(B)] * reps

            def load_slab(qc, b, split=False, kj_range=None):
                NT = nvts[b]
                eb_sb = ebp.tile(
                    [128, NTMAX + 1, HPC, QC], dt.float8e4, name="eb_sb", tag="eb"
                )
                src_r = eb[:, snvt[b] * 128 : snvt[b + 1] * 128, :].rearrange(
                    "h (t p) q -> h p t q", p=128
                )[:, :, :, qc * QC : (qc + 1) * QC]

                def emit(kjs):
                    for kj in kjs:
                        for h in range(HPC):
                            nc.sync.dma_start(eb_sb[:, kj, h, :], src_r[h, :, kj, :])

                if split:
                    emit(range(NT) if kj_range is None else kj_range)
                else:
                    for h in range(HPC):
                        nc.sync.dma_start(eb_sb[:, 0:NT, h, :], src_r[h])
                # pad tile (read by the DR inject's zero slot on the last key
                # tile) must be initialized for the race detector
                if kj_range is None or list(kj_range)[-1] == NT - 1:
                    nc.gpsimd.memset(eb_sb[:, NT, :, :], 0.0)
                return eb_sb, emit

            slabs = {}
            # first two key tiles of iteration 0 land before the va bulk loads
            eb0, emit0 = load_slab(*iters[0], split=True, kj_range=range(2))
            load_va(0)
            emit0(range(2, nvts[iters[0][1]]))
            nc.gpsimd.memset(eb0[:, nvts[iters[0][1]], :, :], 0.0)
            slabs[0] = eb0
            for b in range(1, B):
                load_b(b)
                slabs[b], _ = load_slab(*iters[b], split=(b == 1))

            def emit_norm_piece(state):
                # mask column is 1/32, so 1/normcol = 32/sum(w): the x32 ctx
                # scaling is free. Pieces alternate DVE / Act to balance load.
                ctx, col0, holder = state
                if holder[0] is None:
                    holder[0] = normp.tile(
                        [128, QC // 128, HPC * HD], dt.float8e4, name="ctxn", tag="ctxn"
                    )
                ctxn = holder[0]
                t = holder[1]
                holder[1] += 1
                ti, tt = t // 2, t % 2
                for h in range(HPC):
                    rec = normp.tile([128, 1], dt.float32, name="rec", tag="rec")
                    nc.vector.reciprocal(
                        rec[:], ctx[ti][:, tt, h * (HD + 1) + HD : h * (HD + 1) + HD + 1]
                    )
                    if t != 3:
                        nc.vector.tensor_scalar(
                            out=ctxn[:, t, h * HD : (h + 1) * HD],
                            in0=ctx[ti][:, tt, h * (HD + 1) : h * (HD + 1) + HD],
                            scalar1=rec[:],
                            scalar2=None,
                            op0=ALU.mult,
                        )
                    else:
                        nc.scalar.activation(
                            ctxn[:, t, h * HD : (h + 1) * HD],
                            ctx[ti][:, tt, h * (HD + 1) : h * (HD + 1) + HD],
                            AF.Copy,
                            scale=rec[:],
                        )
                if t == QC // 128 - 1:
                    nc.scalar.dma_start(ctx_o[:, col0 : col0 + QC], ctxn[:])

            def emit_norm(state):
                while state[2][1] < QC // 128:
                    emit_norm_piece(state)

            def emit_av_pair(ctx, tbase, pj, wm2, start, stop):
                # DoubleRow fp8 AV over a kj pair
                for ti in range(QC // 256):
                    for tt in range(2):
                        for h in range(HPC):
                            t = ti * 2 + tt
                            nc.tensor.matmul(
                                ctx[ti][:, tt, h * (HD + 1) : (h + 1) * (HD + 1)],
                                lhsT=wm2[:, :, h, t * 128 : (t + 1) * 128],
                                rhs=va_sb[:, tbase + 2 * pj : tbase + 2 * pj + 2,
                                          h * (HD + 1) : (h + 1) * (HD + 1)],
                                start=start and (tt == 0) and (h == 0),
                                stop=stop and (ti == QC // 256 - 1) and (tt == 1) and (h == HPC - 1),
                                perf_mode=MPM.DoubleRow,
                                skip_group_check=True,
                            )

            def emit_av_sch(ctx, tbase, kj, i16, sl, start, stop):
                # plain bf16 AV for one Schraudolph kj tile (bitcast int16 weights)
                for ti in range(QC // 256):
                    for tt in range(2):
                        for h in range(HPC):
                            t = ti * 2 + tt
                            nc.tensor.matmul(
                                ctx[ti][:, tt, h * (HD + 1) : (h + 1) * (HD + 1)],
                                lhsT=i16[:, sl, h, t * 128 : (t + 1) * 128].bitcast(dt.bfloat16),
                                rhs=va16_sb[:, tbase + kj, h * (HD + 1) : (h + 1) * (HD + 1)],
                                start=start and (tt == 0) and (h == 0),
                                stop=stop and (ti == QC // 256 - 1) and (tt == 1) and (h == HPC - 1),
                                skip_group_check=True,
                            )

            def emit_av_one(ctx, tbase, kj, wm1, start, stop):
                # plain fp8 AV for the odd tail tile
                for ti in range(QC // 256):
                    for tt in range(2):
                        for h in range(HPC):
                            t = ti * 2 + tt
                            nc.tensor.matmul(
                                ctx[ti][:, tt, h * (HD + 1) : (h + 1) * (HD + 1)],
                                lhsT=wm1[:, 0, h, t * 128 : (t + 1) * 128],
                                rhs=va_sb[:, tbase + kj, h * (HD + 1) : (h + 1) * (HD + 1)],
                                start=start and (tt == 0) and (h == 0),
                                stop=stop and (ti == QC // 256 - 1) and (tt == 1) and (h == HPC - 1),
                                skip_group_check=True,
                            )

            tail_av = []     # AV thunks deferred from the previous kj
            tail_norm = None

            for it_i, (qc, b) in enumerate(iters):
                NT = nvts[b]
                NP = NT // 2
                eb_sb = slabs.pop(it_i)
                if it_i + 4 < len(iters):
                    slabs[it_i + 4], _ = load_slab(*iters[it_i + 4])
                ctx = [
                    cp.tile([128, 2, HPC * (HD + 1)], dt.float32, name=f"ctx{t}", tag="ctx")
                    for t in range(QC // 256)
                ]
                col0 = b * SQ + qc * QC
                tbase = snvt[b]

                def make_S(kj):
                    # per-head 1-bank S tiles -> deeper PSUM pipeline
                    Ss = []
                    kcol = tbase * 128 + kj * 128
                    for h in range(HPC):
                        S = Sp.tile([128, QC], dt.float32, name="S", tag="S")
                        nc.tensor.matmul(
                            S[:],
                            lhsT=kT_sb[:, :, h, kcol : kcol + 128],
                            rhs=qT_sb[:, :, h, col0 : col0 + QC],
                            start=True,
                            stop=False,
                            perf_mode=MPM.DoubleRow,
                            skip_group_check=True,
                        )
                        nc.tensor.matmul(
                            S[:],
                            lhsT=i2_sb[:],
                            rhs=eb_sb[:, kj : kj + 2, h, :],
                            start=False,
                            stop=True,
                            perf_mode=MPM.DoubleRow,
                            skip_group_check=True,
                        )
                        Ss.append(S)
                    return Ss

                first_av = [True]
                wm2_cur = [None]
                i16_cur = [None]
                for kj in range(NT):
                    S = make_S(kj)
                    # drain the deferred AVs / previous iteration's norm
                    if tail_av:
                        fin = (kj == 0)
                        for j, (fn, args) in enumerate(tail_av):
                            fn(*args, stop=(fin and j == len(tail_av) - 1) if fin else False)
                        tail_av = []
                    if tail_norm is not None and kj >= 1:
                        emit_norm_piece(tail_norm)
                        if tail_norm[2][1] >= QC // 128:
                            tail_norm = None
                    pj = kj // 2
                    is_odd_tail = (kj == NT - 1) and (NT % 2 == 1)
                    path = "act" if (is_odd_tail or pj % 2 == 0) else "sch"
                    if path == "act":
                        if is_odd_tail:
                            wm1 = wp.tile([128, 1, HPC, QC], dt.float8e4, name="wm1", tag="wm1")
                            for h in range(HPC):
                                nc.scalar.activation(
                                    wm1[:, 0, h], S[h][:], AF.Exp, scale=1.0 / 64.0
                                )
                            tail_av.append((emit_av_one, [ctx, tbase, kj, wm1, first_av[0]]))
                            first_av[0] = False
                        else:
                            if kj % 2 == 0:
                                wm2_cur[0] = wp.tile(
                                    [128, 2, HPC, QC], dt.float8e4, name="wm2", tag="wm2"
                                )
                            for h in range(HPC):
                                nc.scalar.activation(
                                    wm2_cur[0][:, kj % 2, h], S[h][:], AF.Exp, scale=1.0 / 64.0
                                )
                            if kj % 2 == 1:
                                tail_av.append(
                                    (emit_av_pair, [ctx, tbase, pj, wm2_cur[0], first_av[0]])
                                )
                                first_av[0] = False
                    else:
                        if kj % 2 == 0 or is_odd_tail:
                            i16_cur[0] = ip.tile(
                                [128, 2, HPC, QC], dt.int16, name="i16", tag="i16"
                            )
                        sl_ = 0 if is_odd_tail else kj % 2
                        for h in range(HPC):
                            nc.vector.tensor_scalar(
                                out=i16_cur[0][:, sl_, h], in0=S[h][:], scalar1=SCH_A / 64.0,
                                scalar2=SCH_B, op0=ALU.mult, op1=ALU.add,
                            )
                        tail_av.append(
                            (emit_av_sch, [ctx, tbase, kj, i16_cur[0], sl_, first_av[0]])
                        )
                        first_av[0] = False

                if tail_norm is not None:
                    emit_norm(tail_norm)   # short iterations: flush leftovers
                tail_norm = (ctx, col0, [None, 0])

            for j, (fn, args) in enumerate(tail_av):
                fn(*args, stop=(j == len(tail_av) - 1))
            emit_norm(tail_norm)

    nc.compile()
    return nc


# --------------------------------------------------------------------------
# Phase 3: out projection + residual + LayerNorm (row-parallel, fp8 DR GEMM).
#   inputs (per core): ctxT [D, RPC] fp8 (=32*ctx^T), woT [D, D] fp8 (=64*Wo^T),
#     resid [RPC, D] bf16 (query rows + bo), [gammab/betab [128, D] f32 if
#     not trivial_ln]
#   outputs: out_o [RPC, D] f32
# --------------------------------------------------------------------------
def build_phase3(trivial_ln=True, reps=1):
    nc = bacc.Bacc("TRN2", debug=False, num_devices=NCORES)
    KC = D // 128

    ctxT = nc.dram_tensor("ctxT", [D, RPC], dt.float8e4, kind="ExternalInput").ap()
    woT = nc.dram_tensor("woT", [D, D], dt.float8e4, kind="ExternalInput").ap()
    resid = nc.dram_tensor("resid", [RPC, D], dt.bfloat16, kind="ExternalInput").ap()
    if not trivial_ln:
        gammab = nc.dram_tensor("gammab", [128, D], dt.float32, kind="ExternalInput").ap()
        betab = nc.dram_tensor("betab", [128, D], dt.float32, kind="ExternalInput").ap()
    out_o = nc.dram_tensor("out_o", [RPC, D], dt.float32, kind="ExternalOutput").ap()
    PS_SCALE = 1.0 / (32.0 * 64.0)

    with tile.TileContext(nc) as tc:
        with (
            tc.tile_pool(name="big", bufs=1) as bigp,
            tc.tile_pool(name="rp", bufs=4) as rp,
            tc.tile_pool(name="wk", bufs=3) as wk,
            tc.tile_pool(name="ps", bufs=6, space="PSUM") as psp,
        ):
            ctx_sb = bigp.tile([128, KC, RPC], dt.float8e4)
            wo_sb = bigp.tile([128, KC, D], dt.float8e4)
            nc.sync.dma_start(
                ctx_sb[:], ctxT[:, :].rearrange("(k p) c -> p k c", p=128)
            )
            nc.sync.dma_start(
                wo_sb[:], woT[:, :].rearrange("(k p) c -> p k c", p=128)
            )
            eps_sb = bigp.tile([128, 1], dt.float32)
            nc.vector.memset(eps_sb[:], LN_EPS)
            warm = bigp.tile([1, 1], dt.float32)
            nc.vector.memset(warm[:], 1.0)
            warm2 = bigp.tile([1, 1], dt.float32)
            nc.scalar.activation(warm2[:], warm[:], AF.Sqrt)
            warm3 = bigp.tile([1, 1], dt.float32)
            nc.scalar.activation(warm3[:], warm[:], AF.Square)
            if not trivial_ln:
                gam_sb = bigp.tile([128, D], dt.float32)
                nc.sync.dma_start(gam_sb[:], gammab[:])
                bet_sb = bigp.tile([128, D], dt.float32)
                nc.sync.dma_start(bet_sb[:], betab[:])

            for m in [m for _ in range(reps) for m in range(RPC // 128)]:
                res_sb = rp.tile([128, D], dt.bfloat16, name="res_sb", tag="res")
                nc.sync.dma_start(res_sb[:], resid[m * 128 : (m + 1) * 128, :])
                ps = [psp.tile([128, 512], dt.float32, name=f"ps{n}", tag="ps") for n in range(2)]
                for n in range(2):
                    for k2 in range(KC // 2):
                        nc.tensor.matmul(
                            ps[n][:],
                            lhsT=ctx_sb[:, 2 * k2 : 2 * k2 + 2, m * 128 : (m + 1) * 128],
                            rhs=wo_sb[:, 2 * k2 : 2 * k2 + 2, n * 512 : (n + 1) * 512],
                            start=(k2 == 0),
                            stop=(k2 == KC // 2 - 1),
                            perf_mode=MPM.DoubleRow,
                        )
                x_sb = wk.tile([128, D], dt.float32, name="x_sb", tag="x")
                acc = [wk.tile([128, 1], dt.float32, name=f"acc{n}", tag=f"acc{n}") for n in range(2)]
                for n in range(2):
                    nc.vector.scalar_tensor_tensor(
                        out=x_sb[:, n * 512 : (n + 1) * 512],
                        in0=ps[n][:],
                        scalar=PS_SCALE,
                        in1=res_sb[:, n * 512 : (n + 1) * 512],
                        op0=ALU.mult,
                        op1=ALU.add,
                        accum_out=acc[n][:],
                    )
                mu = wk.tile([128, 1], dt.float32, name="mu", tag="mu")
                nc.vector.tensor_scalar(
                    out=mu[:], in0=acc[0][:], scalar1=acc[1][:], scalar2=1.0 / D,
                    op0=ALU.add, op1=ALU.mult,
                )
                sq = wk.tile([128, D], dt.bfloat16, name="sq", tag="sq")
                s2 = wk.tile([128, 1], dt.float32, name="s2", tag="s2")
                nc.scalar.activation(sq[:], x_sb[:], AF.Square, accum_out=s2[:])
                var = wk.tile([128, 1], dt.float32, name="var", tag="var")
                # var = s2/D - mu^2  (one fused op: (s2*(1/D)) - mu2)
                mu2 = wk.tile([128, 1], dt.float32, name="mu2", tag="mu2")
                nc.vector.tensor_tensor(mu2[:], mu[:], mu[:], op=ALU.mult)
                nc.vector.tensor_scalar(
                    out=var[:], in0=s2[:], scalar1=1.0 / D, scalar2=mu2[:],
                    op0=ALU.mult, op1=ALU.subtract,
                )
                std = wk.tile([128, 1], dt.float32, name="std", tag="std")
                nc.scalar.activation(std[:], var[:], AF.Sqrt, bias=eps_sb[:])
                rstd = wk.tile([128, 1], dt.float32, name="rstd", tag="rstd")
                nc.vector.reciprocal(rstd[:], std[:])
                mrs = wk.tile([128, 1], dt.float32, name="mrs", tag="mrs")
                nc.vector.tensor_tensor(mrs[:], mu[:], rstd[:], op=ALU.mult)
                out_sb = wk.tile([128, D], dt.float32, name="out_sb", tag="out_sb")
                if trivial_ln:
                    nc.vector.tensor_scalar(
                        out=out_sb[:], in0=x_sb[:], scalar1=rstd[:], scalar2=mrs[:],
                        op0=ALU.mult, op1=ALU.subtract,
                    )
                else:
                    tmp = wk.tile([128, D], dt.float32, name="tmp", tag="tmp")
                    nc.vector.tensor_scalar(
                        out=tmp[:], in0=x_sb[:], scalar1=rstd[:], scalar2=mrs[:],
                        op0=ALU.mult, op1=ALU.subtract,
                    )
                    y = wk.tile([128, D], dt.float32, name="y", tag="y")
                    nc.vector.scalar_tensor_tensor(
                        out=y[:], in0=tmp[:], scalar=0.0, in1=gam_sb[:],
                        op0=ALU.add, op1=ALU.mult,
                    )
                    nc.gpsimd.tensor_add(out_sb[:], y[:], bet_sb[:])
                nc.sync.dma_start(out_o[m * 128 : (m + 1) * 128, :], out_sb[:])

    nc.compile()
    return nc


def _get_program(key, builder, *args, **kwargs):
    if key not in _programs:
        _programs[key] = builder(*args, **kwargs)
    return _programs[key]


def _run(nc, in_maps):
    return bass_utils.run_bass_kernel_spmd(nc, in_maps, core_ids=list(range(NCORES)))


def kernel(query, key, value, attention_mask, relative_position_bias,
           Wq, bq, Wk, bk, Wv, bv, Wo, bo, ln_gamma, ln_beta,
           _collect_results=None):
    query = np.asarray(query, dtype=np.float32)
    key = np.asarray(key, dtype=np.float32)
    value = np.asarray(value, dtype=np.float32)
    attention_mask = np.asarray(attention_mask)
    relative_position_bias = np.asarray(relative_position_bias, dtype=np.float32)

    def xT8(x):
        return np.ascontiguousarray(x.reshape(-1, D).T).astype(F8)

    def wT8(W, scale):
        return (np.ascontiguousarray(np.asarray(W, np.float32).T) * scale).astype(F8)

    xqT = xT8(query)
    xkT = xT8(key)
    xvT = xT8(value)
    wqT = wT8(Wq, 64.0)
    wkT = wT8(Wk, 64.0)
    wvT = wT8(Wv, 64.0)

    # ---------------- phase 1 ----------------
    in1 = []
    for c in range(NCORES):
        sl = slice(c * RPC, (c + 1) * RPC)
        in1.append({
            "xqT": np.ascontiguousarray(xqT[:, sl]),
            "xkT": np.ascontiguousarray(xkT[:, sl]),
            "xvT": np.ascontiguousarray(xvT[:, sl]),
            "wqT": wqT, "wkT": wkT, "wvT": wvT,
        })
    r1 = _run(_get_program("p1", build_phase1), in1)

    qT_full = np.empty((D, B * SQ), dtype=F8)
    kT_full = np.empty((D, B * SK), dtype=F8)
    v_full = np.empty((B * SK, D), dtype=F8)
    for c in range(NCORES):
        sl = slice(c * RPC, (c + 1) * RPC)
        qT_full[:, sl] = r1.results[c]["qT_o"]
        kT_full[:, sl] = r1.results[c]["kT_o"]
        v_full[sl, :] = r1.results[c]["v_o"]

    # fold any nonzero projection biases in on the host (zero in practice)
    if np.any(np.asarray(bq)):
        qT_full = (qT_full.astype(np.float32)
                   + 4.0 * np.asarray(bq, np.float32)[:, None]).astype(F8)
    if np.any(np.asarray(bk)):
        kT_full = (kT_full.astype(np.float32)
                   + 2.0 * np.asarray(bk, np.float32)[:, None]).astype(F8)
    if np.any(np.asarray(bv)):
        v_full = (v_full.astype(np.float32)
                  + np.asarray(bv, np.float32)[None, :]).astype(F8)

    # ---------------- phase 2 ----------------
    mask2 = (attention_mask.reshape(B, SK) != 0)
    valid = [np.nonzero(mask2[b])[0] for b in range(B)]
    nvts = tuple(max(1, -(-len(ix) // 128)) for ix in valid)
    snvt = np.concatenate([[0], np.cumsum(nvts)]).astype(int)
    TNT = int(snvt[-1])
    idx_pad = np.zeros(TNT * 128, dtype=np.int64)
    maskc = np.zeros((TNT * 128,), dtype=bool)
    for b in range(B):
        ix = valid[b]
        o = snvt[b] * 128
        idx_pad[o : o + len(ix)] = ix
        maskc[o : o + len(ix)] = True

    col_idx = (np.repeat(np.arange(B) * SK, np.array(nvts) * 128) + idx_pad)
    kT_c = np.ascontiguousarray(kT_full[:, col_idx])
    v_rows = v_full[col_idx, :]
    va_all = np.zeros((TNT * 128, H * (HD + 1)), dtype=F8)
    inv32 = np.asarray(1.0 / 32.0, dtype=F8)[()]
    for h in range(H):
        blk = np.where(maskc[:, None], v_rows[:, h * HD : (h + 1) * HD], np.zeros((), F8))
        va_all[:, h * (HD + 1) : h * (HD + 1) + HD] = blk
        va_all[:, h * (HD + 1) + HD] = np.where(maskc, inv32, np.zeros((), F8))

    ebT8 = (np.ascontiguousarray(
        relative_position_bias[0].transpose(0, 2, 1)) * 64.0).astype(F8)
    eb_c = ebT8[:, idx_pad, :]  # [H, TNV, SQ] fp8

    i2_host = np.zeros((128, 256), dtype=F8)
    i2_host[:, 0:128] = np.eye(128, dtype=np.float32).astype(F8)

    in2 = []
    for c in range(NCORES):
        rs = slice(c * 128, (c + 1) * 128)
        in2.append({
            "qT": np.ascontiguousarray(qT_full[rs, :]),
            "kT": np.ascontiguousarray(kT_c[rs, :]),
            "va": np.ascontiguousarray(
                va_all[:, c * HPC * (HD + 1) : (c + 1) * HPC * (HD + 1)]
            ),
            "va16": np.ascontiguousarray(
                va_all[:, c * HPC * (HD + 1) : (c + 1) * HPC * (HD + 1)]
            ).astype(BF16),
            "eb": np.ascontiguousarray(eb_c[c * HPC : (c + 1) * HPC]),
            "i2": i2_host,
        })
    r2 = _run(_get_program(("p2",) + nvts, build_phase2, nvts), in2)

    # ctx_o[c] is [128 q-part, t, 128 d] for d-block c -> assemble ctxT [D, B*SQ]
    ctxT_full = np.empty((D, B * SQ), dtype=F8)
    for c in range(NCORES):
        blk = r2.results[c]["ctx_o"].reshape(128, B * SQ // 128, 128)
        ctxT_full[c * 128 : (c + 1) * 128, :] = (
            blk.transpose(2, 1, 0).reshape(128, B * SQ)
        )

    # ---------------- phase 3 ----------------
    woT8 = wT8(Wo, 64.0)
    q2d = query.reshape(-1, D)
    resid_h = (q2d + np.asarray(bo, np.float32)[None, :]).astype(BF16)
    trivial = (not np.any(np.asarray(ln_beta))) and np.all(
        np.asarray(ln_gamma, np.float32) == 1.0
    )
    in3 = []
    for c in range(NCORES):
        sl = slice(c * RPC, (c + 1) * RPC)
        d = {
            "ctxT": np.ascontiguousarray(ctxT_full[:, sl]),
            "woT": woT8,
            "resid": np.ascontiguousarray(resid_h[sl, :]),
        }
        if not trivial:
            d["gammab"] = np.ascontiguousarray(
                np.broadcast_to(np.asarray(ln_gamma, np.float32)[None, :], (128, D))
            )
            d["betab"] = np.ascontiguousarray(
                np.broadcast_to(np.asarray(ln_beta, np.float32)[None, :], (128, D))
            )
        in3.append(d)
    r3 = _run(_get_program(("p3", trivial), build_phase3, trivial), in3)

    out = np.empty((B * SQ, D), dtype=np.float32)
    for c in range(NCORES):
        out[c * RPC : (c + 1) * RPC, :] = r3.results[c]["out_o"]

    if _collect_results is not None:
        _collect_results.extend([r1, r2, r3])
    return out.reshape(B, SQ, D)


# revision 22
# speedup vs baseline: 1.0124x; 1.0075x over previous
"""MultiHeadCrossAttention Trainium2 kernel (8 NeuronCores, SPMD).

Problem: B=4, SQ=SK=2048, D=1024, H=16 (HD=64), f32 in/out.

Distribution (3 SPMD launches):
  Phase 1 (row-parallel): QKV projections in fp8 e4m3 with DoubleRow matmuls
    (2 contraction rows per partition -> 0.5 cyc/row). Weights host-prescaled
    by 64; outputs written as scaled fp8 (q*4, k*2, v*1).
  Phase 2 (head-parallel): attention, 2 heads/core. Keys mask-compacted on
    host. Scores S = (4q).(2k) accumulate in PSUM via fp8-DR matmuls over the
    hd=64 contraction split as [32 partitions x 2 slots]; 64*bias (fp8) is
    injected into the same PSUM via a DoubleRow identity matmul, so
    exp(score+bias) needs no elementwise multiply. exp runs split across
    engines: ScalarE true exp (scale=1/64) -> fp8 weights (DR AV matmul), and
    Schraudolph bit-trick exp on DVE/Pool (tensor_scalar -> int16, bitcast to
    bf16 -> plain AV matmul). Mask + normalizer ride as an extra fp8 value
    column; normalization multiplies by 32/norm -> fp8 ctx output.
  Phase 3 (row-parallel): out projection (fp8 DR, ctxT*32 @ woT*64, /2048
    folded into the residual add), one-pass mean/var LayerNorm.
"""

import sys

sys.path.insert(0, "/opt/trn_rl_repo")

import numpy as np
import ml_dtypes

import concourse.bass as bass
import concourse.tile as tile
from concourse import bacc, mybir
from concourse import bass_utils

BF16 = ml_dtypes.bfloat16

B, SQ, SK, D, H = 4, 2048, 2048, 1024, 16
HD = D // H  # 64
NCORES = 8
HPC = H // NCORES          # heads per core = 2
RPC = B * SQ // NCORES     # rows per core (phases 1/3) = 1024
LN_EPS = 1e-5

dt = mybir.dt
AF = mybir.ActivationFunctionType
ALU = mybir.AluOpType
MPM = mybir.MatmulPerfMode

F8 = np.dtype(mybir.dt.np(dt.float8e4))
F32 = np.float32

# Schraudolph fast-exp in bf16 bit space: bf16_bits(exp(x)) ~= x*128*log2e + B
SCH_A = 128.0 * 1.4426950408889634
SCH_B = 127.0 * 128.0 - 0.0436 * 128.0

_programs = {}


# --------------------------------------------------------------------------
# Phase 1: QKV projection (row-parallel, fp8 DoubleRow, no bias on device —
# host folds biases into the outputs if nonzero).
#   inputs (per core): xqT/xkT/xvT [D, RPC] fp8 (input^T), wqT/wkT/wvT
#                      [D, D] fp8 (W^T * 64)
#   outputs: qT_o/kT_o [D, RPC] fp8 (4*q^T, 2*k^T), v_o [RPC, D] fp8 (v)
# --------------------------------------------------------------------------
def build_phase1(reps=1):
    nc = bacc.Bacc("TRN2", debug=False, num_devices=NCORES)
    KC = D // 128  # 8 chunks of 128 = 4 double-chunks

    ins = {}
    for nm in ("xqT", "xkT", "xvT"):
        ins[nm] = nc.dram_tensor(nm, [D, RPC], dt.float8e4, kind="ExternalInput").ap()
    for nm in ("wqT", "wkT", "wvT"):
        ins[nm] = nc.dram_tensor(nm, [D, D], dt.float8e4, kind="ExternalInput").ap()
    qT_o = nc.dram_tensor("qT_o", [D, RPC], dt.float8e4, kind="ExternalOutput").ap()
    kT_o = nc.dram_tensor("kT_o", [D, RPC], dt.float8e4, kind="ExternalOutput").ap()
    v_o = nc.dram_tensor("v_o", [RPC, D], dt.float8e4, kind="ExternalOutput").ap()

    # greedy engine assignment for the 48 PSUM->SBUF scaled copies
    # (GPSIMD/Pool cannot touch PSUM on TRN2, so only Act/DVE)
    eng_cost = {"act": 570.0, "dve": 660.0}
    eng_load = {"act": 0.0, "dve": 0.0}
    copy_plan = []
    for _ in range(3 * (D // 128) * 2):
        e = min(eng_load, key=lambda k: eng_load[k] + eng_cost[k])
        copy_plan.append(e)
        eng_load[e] += eng_cost[e]
    copy_i = [0]

    with tile.TileContext(nc) as tc:
        with (
            tc.tile_pool(name="big", bufs=1) as bigp,
            tc.tile_pool(name="outp", bufs=3) as outp,
            tc.tile_pool(name="ps", bufs=2, space="PSUM") as psp,
        ):
            warm = bigp.tile([1, 1], dt.float32)
            nc.vector.memset(warm[:], 1.0)
            warm2 = bigp.tile([1, 1], dt.float32)
            nc.scalar.activation(warm2[:], warm[:], AF.Copy)
            sb = {}
            for nm in ("xqT", "xkT", "xvT", "wqT", "wkT", "wvT"):
                ncols = ins[nm].shape[1]
                sb[nm] = bigp.tile([128, KC, ncols], dt.float8e4, name=f"{nm}_sb")
            for pair in (("wqT", "xqT"), ("wkT", "xkT"), ("wvT", "xvT")):
                for nm in pair:
                    nc.sync.dma_start(
                        sb[nm][:],
                        ins[nm][:, :].rearrange("(k p) c -> p k c", p=128),
                    )

            def copy_out(dst, src, scale):
                e = copy_plan[copy_i[0] % len(copy_plan)]
                copy_i[0] += 1
                if e == "act":
                    nc.scalar.activation(dst, src, AF.Copy, scale=scale)
                elif e == "dve":
                    nc.vector.tensor_scalar(
                        out=dst, in0=src, scalar1=scale, scalar2=None, op0=ALU.mult
                    )
                else:
                    nc.gpsimd.tensor_scalar(
                        out=dst, in0=src, scalar1=scale, scalar2=None, op0=ALU.mult
                    )

            def proj(x_nm, w_nm, out_dram, transposed_out, scale):
                xt = sb[x_nm]
                wt = sb[w_nm]
                if transposed_out:
                    lt, rt = wt, xt   # out[d_out, rows]
                else:
                    lt, rt = xt, wt   # out[rows, d_out]
                n_m = lt.shape[2] // 128
                n_n = rt.shape[2] // 512
                MG = 2
                for mg in range(0, n_m, MG):
                    ms = range(mg, min(mg + MG, n_m))
                    pss = {}
                    for m in ms:
                        for n in range(n_n):
                            pss[m, n] = psp.tile(
                                [128, 512], dt.float32, name="ps", tag=f"ps{m % MG}_{n}"
                            )
                    for k2 in range(KC // 2):
                        for m in ms:
                            for n in range(n_n):
                                nc.tensor.matmul(
                                    pss[m, n][:],
                                    lhsT=lt[:, 2 * k2 : 2 * k2 + 2, m * 128 : (m + 1) * 128],
                                    rhs=rt[:, 2 * k2 : 2 * k2 + 2, n * 512 : (n + 1) * 512],
                                    start=(k2 == 0),
                                    stop=(k2 == KC // 2 - 1),
                                    perf_mode=MPM.DoubleRow,
                                )
                    osb = outp.tile(
                        [128, MG, rt.shape[2]], dt.float8e4, name=f"{x_nm}_osb", tag="osb"
                    )
                    for m in ms:
                        for n in range(n_n):
                            copy_out(osb[:, m - mg, n * 512 : (n + 1) * 512], pss[m, n][:], scale)
                    nc.scalar.dma_start(
                        out_dram[mg * 128 : (mg + MG) * 128, :].rearrange(
                            "(g p) c -> p g c", p=128
                        ),
                        osb[:],
                    )

            for _ in range(reps):
                proj("xqT", "wqT", qT_o, True, 4.0 / 64.0)
                proj("xkT", "wkT", kT_o, True, 2.0 / 64.0)
                proj("xvT", "wvT", v_o, False, 1.0 / 64.0)

    nc.compile()
    return nc


# --------------------------------------------------------------------------
# Phase 2: attention (head-parallel, 2 heads/core).
#   inputs (per core):
#     qT  [128, B*SQ] fp8  (rows = 2 heads x 64 dims, = 4*q^T)
#     kT  [128, TNV] fp8   (compacted, = 2*k^T)
#     va  [TNV, HPC*(HD+1)] fp8 (v*mask | mask column per head)
#     va16 same as va in bf16 (for the Schraudolph bf16 AV matmuls)
#     eb  [HPC, TNV, SQ] fp8 (64 * bias^T per head, compacted rows)
#     i2  [128, 256] fp8   (DoubleRow identity: [:, :128]=I, [:, 128:]=0)
#   outputs: ctx_o [128, B*SQ] fp8 = 32*ctx/norm in [p, t, d] layout
# --------------------------------------------------------------------------
def build_phase2(nvts=(8, 8, 8, 8), reps=1, sp_bufs=6, cp_bufs=2, wm_bufs=4):
    nc = bacc.Bacc("TRN2", debug=False, num_devices=NCORES)
    QC = 512
    NQC = SQ // QC
    snvt = [0]
    for t in nvts:
        snvt.append(snvt[-1] + t)
    TNT = snvt[-1]
    TNV = TNT * 128
    NTMAX = max(nvts)

    qT = nc.dram_tensor("qT", [128, B * SQ], dt.float8e4, kind="ExternalInput").ap()
    kT = nc.dram_tensor("kT", [128, TNV], dt.float8e4, kind="ExternalInput").ap()
    va = nc.dram_tensor("va", [TNV, HPC * (HD + 1)], dt.float8e4, kind="ExternalInput").ap()
    va16 = nc.dram_tensor("va16", [TNV, HPC * (HD + 1)], dt.bfloat16, kind="ExternalInput").ap()
    eb = nc.dram_tensor("eb", [HPC, TNV, SQ], dt.float8e4, kind="ExternalInput").ap()
    i2 = nc.dram_tensor("i2", [128, 256], dt.float8e4, kind="ExternalInput").ap()
    ctx_o = nc.dram_tensor("ctx_o", [128, B * SQ], dt.float8e4, kind="ExternalOutput").ap()

    with tile.TileContext(nc) as tc:
        with (
            tc.tile_pool(name="big", bufs=1) as bigp,
            tc.tile_pool(name="ebp", bufs=5) as ebp,
            tc.tile_pool(name="wp", bufs=wm_bufs) as wp,
            tc.tile_pool(name="ip", bufs=wm_bufs) as ip,
            tc.tile_pool(name="np_", bufs=6) as normp,
            tc.tile_pool(name="Sp", bufs=sp_bufs, space="PSUM") as Sp,
            tc.tile_pool(name="cp", bufs=cp_bufs, space="PSUM") as cp,
        ):
            # hd-split layouts for DoubleRow: [32 partitions, 2 slots, head, cols]
            qT_sb = bigp.tile([32, 2, HPC, B * SQ], dt.float8e4)
            kT_sb = bigp.tile([32, 2, HPC, TNV], dt.float8e4)
            va_sb = bigp.tile([128, TNT, HPC * (HD + 1)], dt.float8e4)
            va16_sb = bigp.tile([128, TNT, HPC * (HD + 1)], dt.bfloat16)
            i2_sb = bigp.tile([128, 2, 128], dt.float8e4)
            warm = bigp.tile([1, 1], dt.float32)
            nc.vector.memset(warm[:], 0.0)
            warm2 = bigp.tile([1, 1], dt.float32)
            nc.scalar.activation(warm2[:], warm[:], AF.Exp)

            def load_qk(b):
                for h in range(HPC):
                    nc.sync.dma_start(
                        qT_sb[:, :, h, b * SQ : (b + 1) * SQ],
                        qT[64 * h : 64 * h + 64, b * SQ : (b + 1) * SQ].rearrange(
                            "(s p) c -> p s c", p=32
                        ),
                    )
                    cs, ce = snvt[b] * 128, snvt[b + 1] * 128
                    nc.sync.dma_start(
                        kT_sb[:, :, h, cs:ce],
                        kT[64 * h : 64 * h + 64, cs:ce].rearrange("(s p) c -> p s c", p=32),
                    )

            def load_va(b):
                cs, ce = snvt[b] * 128, snvt[b + 1] * 128
                nc.sync.dma_start(
                    va_sb[:, snvt[b] : snvt[b + 1], :],
                    va[cs:ce, :].rearrange("(t p) d -> p t d", p=128),
                )
                nc.sync.dma_start(
                    va16_sb[:, snvt[b] : snvt[b + 1], :],
                    va16[cs:ce, :].rearrange("(t p) d -> p t d", p=128),
                )

            def load_b(b):
                load_qk(b)
                load_va(b)

            load_qk(0)
            nc.sync.dma_start(i2_sb[:], i2[:])

            iters = [(qc, b) for qc in range(NQC) for b in range(B)] * reps

            def load_slab(qc, b, split=False, kj_range=None):
                NT = nvts[b]
                eb_sb = ebp.tile(
                    [128, NTMAX + 1, HPC, QC], dt.float8e4, name="eb_sb", tag="eb"
                )
                src_r = eb[:, snvt[b] * 128 : snvt[b + 1] * 128, :].rearrange(
                    "h (t p) q -> h p t q", p=128
                )[:, :, :, qc * QC : (qc + 1) * QC]

                def emit(kjs):
                    for kj in kjs:
                        for h in range(HPC):
                            nc.sync.dma_start(eb_sb[:, kj, h, :], src_r[h, :, kj, :])

                if split:
                    emit(range(NT) if kj_range is None else kj_range)
                else:
                    for h in range(HPC):
                        nc.sync.dma_start(eb_sb[:, 0:NT, h, :], src_r[h])
                # pad tile (read by the DR inject's zero slot on the last key
                # tile) must be initialized for the race detector
                if kj_range is None or list(kj_range)[-1] == NT - 1:
                    nc.gpsimd.memset(eb_sb[:, NT, :, :], 0.0)
                return eb_sb, emit

            slabs = {}
            # first two key tiles of iteration 0 land before the va bulk loads
            eb0, emit0 = load_slab(*iters[0], split=True, kj_range=range(2))
            load_va(0)
            emit0(range(2, nvts[iters[0][1]]))
            nc.gpsimd.memset(eb0[:, nvts[iters[0][1]], :, :], 0.0)
            slabs[0] = eb0
            for b in range(1, B):
                load_b(b)
                slabs[b], _ = load_slab(*iters[b], split=(b == 1))

            def emit_norm_piece(state):
                # mask column is 1/32, so 1/normcol = 32/sum(w): the x32 ctx
                # scaling is free. Pieces alternate DVE / Act to balance load.
                ctx, col0, holder = state
                if holder[0] is None:
                    holder[0] = normp.tile(
                        [128, QC // 128, HPC * HD], dt.float8e4, name="ctxn", tag="ctxn"
                    )
                ctxn = holder[0]
                t = holder[1]
                holder[1] += 1
                ti, tt = t // 2, t % 2
                for h in range(HPC):
                    rec = normp.tile([128, 1], dt.float32, name="rec", tag="rec")
                    nc.vector.reciprocal(
                        rec[:], ctx[ti][:, tt, h * (HD + 1) + HD : h * (HD + 1) + HD + 1]
                    )
                    if t != 3:
                        nc.vector.tensor_scalar(
                            out=ctxn[:, t, h * HD : (h + 1) * HD],
                            in0=ctx[ti][:, tt, h * (HD + 1) : h * (HD + 1) + HD],
                            scalar1=rec[:],
                            scalar2=None,
                            op0=ALU.mult,
                        )
                    else:
                        nc.scalar.activation(
                            ctxn[:, t, h * HD : (h + 1) * HD],
                            ctx[ti][:, tt, h * (HD + 1) : h * (HD + 1) + HD],
                            AF.Copy,
                            scale=rec[:],
                        )
                if t == QC // 128 - 1:
                    nc.scalar.dma_start(ctx_o[:, col0 : col0 + QC], ctxn[:])

            def emit_norm(state):
                while state[2][1] < QC // 128:
                    emit_norm_piece(state)

            def emit_av_pair(ctx, tbase, pj, wm2, start, stop):
                # DoubleRow fp8 AV over a kj pair
                for ti in range(QC // 256):
                    for tt in range(2):
                        for h in range(HPC):
                            t = ti * 2 + tt
                            nc.tensor.matmul(
                                ctx[ti][:, tt, h * (HD + 1) : (h + 1) * (HD + 1)],
                                lhsT=wm2[:, :, h, t * 128 : (t + 1) * 128],
                                rhs=va_sb[:, tbase + 2 * pj : tbase + 2 * pj + 2,
                                          h * (HD + 1) : (h + 1) * (HD + 1)],
                                start=start and (tt == 0) and (h == 0),
                                stop=stop and (ti == QC // 256 - 1) and (tt == 1) and (h == HPC - 1),
                                perf_mode=MPM.DoubleRow,
                                skip_group_check=True,
                            )

            def emit_av_sch(ctx, tbase, kj, i16, sl, start, stop):
                # plain bf16 AV for one Schraudolph kj tile (bitcast int16 weights)
                for ti in range(QC // 256):
                    for tt in range(2):
                        for h in range(HPC):
                            t = ti * 2 + tt
                            nc.tensor.matmul(
                                ctx[ti][:, tt, h * (HD + 1) : (h + 1) * (HD + 1)],
                                lhsT=i16[:, sl, h, t * 128 : (t + 1) * 128].bitcast(dt.bfloat16),
                                rhs=va16_sb[:, tbase + kj, h * (HD + 1) : (h + 1) * (HD + 1)],
                                start=start and (tt == 0) and (h == 0),
                                stop=stop and (ti == QC // 256 - 1) and (tt == 1) and (h == HPC - 1),
                                skip_group_check=True,
                            )

            def emit_av_one(ctx, tbase, kj, wm1, start, stop):
                # plain fp8 AV for the odd tail tile
                for ti in range(QC // 256):
                    for tt in range(2):
                        for h in range(HPC):
                            t = ti * 2 + tt
                            nc.tensor.matmul(
                                ctx[ti][:, tt, h * (HD + 1) : (h + 1) * (HD + 1)],
                                lhsT=wm1[:, 0, h, t * 128 : (t + 1) * 128],
                                rhs=va_sb[:, tbase + kj, h * (HD + 1) : (h + 1) * (HD + 1)],
                                start=start and (tt == 0) and (h == 0),
                                stop=stop and (ti == QC // 256 - 1) and (tt == 1) and (h == HPC - 1),
                                skip_group_check=True,
                            )

            tail_av = []     # AV thunks deferred from the previous kj
            tail_norm = None

            for it_i, (qc, b) in enumerate(iters):
                NT = nvts[b]
                NP = NT // 2
                eb_sb = slabs.pop(it_i)
                if it_i + 4 < len(iters):
                    slabs[it_i + 4], _ = load_slab(*iters[it_i + 4])
                ctx = [
                    cp.tile([128, 2, HPC * (HD + 1)], dt.float32, name=f"ctx{t}", tag="ctx")
                    for t in range(QC // 256)
                ]
                col0 = b * SQ + qc * QC
                tbase = snvt[b]

                def make_S(kj):
                    # per-head 1-bank S tiles -> deeper PSUM pipeline
                    Ss = []
                    kcol = tbase * 128 + kj * 128
                    for h in range(HPC):
                        S = Sp.tile([128, QC], dt.float32, name="S", tag="S")
                        nc.tensor.matmul(
                            S[:],
                            lhsT=kT_sb[:, :, h, kcol : kcol + 128],
                            rhs=qT_sb[:, :, h, col0 : col0 + QC],
                            start=True,
                            stop=False,
                            perf_mode=MPM.DoubleRow,
                            skip_group_check=True,
                        )
                        nc.tensor.matmul(
                            S[:],
                            lhsT=i2_sb[:],
                            rhs=eb_sb[:, kj : kj + 2, h, :],
                            start=False,
                            stop=True,
                            perf_mode=MPM.DoubleRow,
                            skip_group_check=True,
                        )
                        Ss.append(S)
                    return Ss

                first_av = [True]
                wm2_cur = [None]
                i16_cur = [None]
                for kj in range(NT):
                    S = make_S(kj)
                    # drain the deferred AVs / previous iteration's norm
                    if tail_av:
                        fin = (kj == 0)
                        for j, (fn, args) in enumerate(tail_av):
                            fn(*args, stop=(fin and j == len(tail_av) - 1) if fin else False)
                        tail_av = []
                    if tail_norm is not None and kj >= 1:
                        emit_norm_piece(tail_norm)
                        if tail_norm[2][1] >= QC // 128:
                            tail_norm = None
                    pj = kj // 2
                    is_odd_tail = (kj == NT - 1) and (NT % 2 == 1)
                    path = "act" if (is_odd_tail or pj % 2 == 0) else "sch"
                    if path == "act":
                        if is_odd_tail:
                            wm1 = wp.tile([128, 1, HPC, QC], dt.float8e4, name="wm1", tag="wm1")
                            for h in range(HPC):
                                nc.scalar.activation(
                                    wm1[:, 0, h], S[h][:], AF.Exp, scale=1.0 / 64.0
                                )
                            tail_av.append((emit_av_one, [ctx, tbase, kj, wm1, first_av[0]]))
                            first_av[0] = False
                        else:
                            if kj % 2 == 0:
                                wm2_cur[0] = wp.tile(
                                    [128, 2, HPC, QC], dt.float8e4, name="wm2", tag="wm2"
                                )
                            for h in range(HPC):
                                nc.scalar.activation(
                                    wm2_cur[0][:, kj % 2, h], S[h][:], AF.Exp, scale=1.0 / 64.0
                                )
                            if kj % 2 == 1:
                                tail_av.append(
                                    (emit_av_pair, [ctx, tbase, pj, wm2_cur[0], first_av[0]])
                                )
                                first_av[0] = False
                    else:
                        if kj % 2 == 0 or is_odd_tail:
                            i16_cur[0] = ip.tile(
                                [128, 2, HPC, QC], dt.int16, name="i16", tag="i16"
                            )
                        sl_ = 0 if is_odd_tail else kj % 2
                        for h in range(HPC):
                            nc.vector.tensor_scalar(
                                out=i16_cur[0][:, sl_, h], in0=S[h][:], scalar1=SCH_A / 64.0,
                                scalar2=SCH_B, op0=ALU.mult, op1=ALU.add,
                            )
                        tail_av.append(
                            (emit_av_sch, [ctx, tbase, kj, i16_cur[0], sl_, first_av[0]])
                        )
                        first_av[0] = False

                if tail_norm is not None:
                    emit_norm(tail_norm)   # short iterations: flush leftovers
                tail_norm = (ctx, col0, [None, 0])

            for j, (fn, args) in enumerate(tail_av):
                fn(*args, stop=(j == len(tail_av) - 1))
            emit_norm(tail_norm)

    nc.compile()
    return nc


# --------------------------------------------------------------------------
# Phase 3: out projection + residual + LayerNorm (row-parallel, fp8 DR GEMM).
#   inputs (per core): ctxT [D, RPC] fp8 (=32*ctx^T), woT [D, D] fp8 (=64*Wo^T),
#     resid [RPC, D] bf16 (query rows + bo), [gammab/betab [128, D] f32 if
#     not trivial_ln]
#   outputs: out_o [RPC, D] f32
# --------------------------------------------------------------------------
def build_phase3(trivial_ln=True, reps=1):
    nc = bacc.Bacc("TRN2", debug=False, num_devices=NCORES)
    KC = D // 128

    ctxT = nc.dram_tensor("ctxT", [D, RPC], dt.float8e4, kind="ExternalInput").ap()
    woT = nc.dram_tensor("woT", [D, D], dt.float8e4, kind="ExternalInput").ap()
    resid = nc.dram_tensor("resid", [RPC, D], dt.bfloat16, kind="ExternalInput").ap()
    if not trivial_ln:
        gammab = nc.dram_tensor("gammab", [128, D], dt.float32, kind="ExternalInput").ap()
        betab = nc.dram_tensor("betab", [128, D], dt.float32, kind="ExternalInput").ap()
    out_o = nc.dram_tensor("out_o", [RPC, D], dt.float32, kind="ExternalOutput").ap()
    PS_SCALE = 1.0 / (32.0 * 64.0)

    with tile.TileContext(nc) as tc:
        with (
            tc.tile_pool(name="big", bufs=1) as bigp,
            tc.tile_pool(name="rp", bufs=4) as rp,
            tc.tile_pool(name="wk", bufs=3) as wk,
            tc.tile_pool(name="ps", bufs=6, space="PSUM") as psp,
        ):
            ctx_sb = bigp.tile([128, KC, RPC], dt.float8e4)
            wo_sb = bigp.tile([128, KC, D], dt.float8e4)
            nc.sync.dma_start(
                ctx_sb[:], ctxT[:, :].rearrange("(k p) c -> p k c", p=128)
            )
            nc.sync.dma_start(
                wo_sb[:], woT[:, :].rearrange("(k p) c -> p k c", p=128)
            )
            eps_sb = bigp.tile([128, 1], dt.float32)
            nc.vector.memset(eps_sb[:], LN_EPS)
            warm = bigp.tile([1, 1], dt.float32)
            nc.vector.memset(warm[:], 1.0)
            warm2 = bigp.tile([1, 1], dt.float32)
            nc.scalar.activation(warm2[:], warm[:], AF.Sqrt)
            warm3 = bigp.tile([1, 1], dt.float32)
            nc.scalar.activation(warm3[:], warm[:], AF.Square)
            if not trivial_ln:
                gam_sb = bigp.tile([128, D], dt.float32)
                nc.sync.dma_start(gam_sb[:], gammab[:])
                bet_sb = bigp.tile([128, D], dt.float32)
                nc.sync.dma_start(bet_sb[:], betab[:])

            for m in [m for _ in range(reps) for m in range(RPC // 128)]:
                res_sb = rp.tile([128, D], dt.bfloat16, name="res_sb", tag="res")
                nc.sync.dma_start(res_sb[:], resid[m * 128 : (m + 1) * 128, :])
                ps = [psp.tile([128, 512], dt.float32, name=f"ps{n}", tag="ps") for n in range(2)]
                for n in range(2):
                    for k2 in range(KC // 2):
                        nc.tensor.matmul(
                            ps[n][:],
                            lhsT=ctx_sb[:, 2 * k2 : 2 * k2 + 2, m * 128 : (m + 1) * 128],
                            rhs=wo_sb[:, 2 * k2 : 2 * k2 + 2, n * 512 : (n + 1) * 512],
                            start=(k2 == 0),
                            stop=(k2 == KC // 2 - 1),
                            perf_mode=MPM.DoubleRow,
                        )
                x_sb = wk.tile([128, D], dt.float32, name="x_sb", tag="x")
                acc = [wk.tile([128, 1], dt.float32, name=f"acc{n}", tag=f"acc{n}") for n in range(2)]
                for n in range(2):
                    nc.vector.scalar_tensor_tensor(
                        out=x_sb[:, n * 512 : (n + 1) * 512],
                        in0=ps[n][:],
                        scalar=PS_SCALE,
                        in1=res_sb[:, n * 512 : (n + 1) * 512],
                        op0=ALU.mult,
                        op1=ALU.add,
                        accum_out=acc[n][:],
                    )
                mu = wk.tile([128, 1], dt.float32, name="mu", tag="mu")
                nc.vector.tensor_scalar(
                    out=mu[:], in0=acc[0][:], scalar1=acc[1][:], scalar2=1.0 / D,
                    op0=ALU.add, op1=ALU.mult,
                )
                sq = wk.tile([128, D], dt.bfloat16, name="sq", tag="sq")
                s2 = wk.tile([128, 1], dt.float32, name="s2", tag="s2")
                nc.scalar.activation(sq[:], x_sb[:], AF.Square, accum_out=s2[:])
                var = wk.tile([128, 1], dt.float32, name="var", tag="var")
                # var = s2/D - mu^2  (one fused op: (s2*(1/D)) - mu2)
                mu2 = wk.tile([128, 1], dt.float32, name="mu2", tag="mu2")
                nc.vector.tensor_tensor(mu2[:], mu[:], mu[:], op=ALU.mult)
                nc.vector.tensor_scalar(
                    out=var[:], in0=s2[:], scalar1=1.0 / D, scalar2=mu2[:],
                    op0=ALU.mult, op1=ALU.subtract,
                )
                std = wk.tile([128, 1], dt.float32, name="std", tag="std")
                nc.scalar.activation(std[:], var[:], AF.Sqrt, bias=eps_sb[:])
                rstd = wk.tile([128, 1], dt.float32, name="rstd", tag="rstd")
                nc.vector.reciprocal(rstd[:], std[:])
                mrs = wk.tile([128, 1], dt.float32, name="mrs", tag="mrs")
                nc.vector.tensor_tensor(mrs[:], mu[:], rstd[:], op=ALU.mult)
                out_sb = wk.tile([128, D], dt.float32, name="out_sb", tag="out_sb")
                if trivial_ln:
                    nc.vector.tensor_scalar(
                        out=out_sb[:], in0=x_sb[:], scalar1=rstd[:], scalar2=mrs[:],
                        op0=ALU.mult, op1=ALU.subtract,
                    )
                else:
                    tmp = wk.tile([128, D], dt.float32, name="tmp", tag="tmp")
                    nc.vector.tensor_scalar(
                        out=tmp[:], in0=x_sb[:], scalar1=rstd[:], scalar2=mrs[:],
                        op0=ALU.mult, op1=ALU.subtract,
                    )
                    y = wk.tile([128, D], dt.float32, name="y", tag="y")
                    nc.vector.scalar_tensor_tensor(
                        out=y[:], in0=tmp[:], scalar=0.0, in1=gam_sb[:],
                        op0=ALU.add, op1=ALU.mult,
                    )
                    nc.gpsimd.tensor_add(out_sb[:], y[:], bet_sb[:])
                nc.sync.dma_start(out_o[m * 128 : (m + 1) * 128, :], out_sb[:])

    nc.compile()
    return nc


def _get_program(key, builder, *args, **kwargs):
    if key not in _programs:
        _programs[key] = builder(*args, **kwargs)
    return _programs[key]


def _run(nc, in_maps):
    return bass_utils.run_bass_kernel_spmd(nc, in_maps, core_ids=list(range(NCORES)))


def kernel(query, key, value, attention_mask, relative_position_bias,
           Wq, bq, Wk, bk, Wv, bv, Wo, bo, ln_gamma, ln_beta,
           _collect_results=None):
    query = np.asarray(query, dtype=np.float32)
    key = np.asarray(key, dtype=np.float32)
    value = np.asarray(value, dtype=np.float32)
    attention_mask = np.asarray(attention_mask)
    relative_position_bias = np.asarray(relative_position_bias, dtype=np.float32)

    def xT8(x):
        return np.ascontiguousarray(x.reshape(-1, D).T).astype(F8)

    def wT8(W, scale):
        return (np.ascontiguousarray(np.asarray(W, np.float32).T) * scale).astype(F8)

    xqT = xT8(query)
    xkT = xT8(key)
    xvT = xT8(value)
    wqT = wT8(Wq, 64.0)
    wkT = wT8(Wk, 64.0)
    wvT = wT8(Wv, 64.0)

    # ---------------- phase 1 ----------------
    in1 = []
    for c in range(NCORES):
        sl = slice(c * RPC, (c + 1) * RPC)
        in1.append({
            "xqT": np.ascontiguousarray(xqT[:, sl]),
            "xkT": np.ascontiguousarray(xkT[:, sl]),
            "xvT": np.ascontiguousarray(xvT[:, sl]),
            "wqT": wqT, "wkT": wkT, "wvT": wvT,
        })
    r1 = _run(_get_program("p1", build_phase1), in1)

    qT_full = np.empty((D, B * SQ), dtype=F8)
    kT_full = np.empty((D, B * SK), dtype=F8)
    v_full = np.empty((B * SK, D), dtype=F8)
    for c in range(NCORES):
        sl = slice(c * RPC, (c + 1) * RPC)
        qT_full[:, sl] = r1.results[c]["qT_o"]
        kT_full[:, sl] = r1.results[c]["kT_o"]
        v_full[sl, :] = r1.results[c]["v_o"]

    # fold any nonzero projection biases in on the host (zero in practice)
    if np.any(np.asarray(bq)):
        qT_full = (qT_full.astype(np.float32)
                   + 4.0 * np.asarray(bq, np.float32)[:, None]).astype(F8)
    if np.any(np.asarray(bk)):
        kT_full = (kT_full.astype(np.float32)
                   + 2.0 * np.asarray(bk, np.float32)[:, None]).astype(F8)
    if np.any(np.asarray(bv)):
        v_full = (v_full.astype(np.float32)
                  + np.asarray(bv, np.float32)[None, :]).astype(F8)

    # ---------------- phase 2 ----------------
    mask2 = (attention_mask.reshape(B, SK) != 0)
    valid = [np.nonzero(mask2[b])[0] for b in range(B)]
    nvts = tuple(max(1, -(-len(ix) // 128)) for ix in valid)
    snvt = np.concatenate([[0], np.cumsum(nvts)]).astype(int)
    TNT = int(snvt[-1])
    idx_pad = np.zeros(TNT * 128, dtype=np.int64)
    maskc = np.zeros((TNT * 128,), dtype=bool)
    for b in range(B):
        ix = valid[b]
        o = snvt[b] * 128
        idx_pad[o : o + len(ix)] = ix
        maskc[o : o + len(ix)] = True

    col_idx = (np.repeat(np.arange(B) * SK, np.array(nvts) * 128) + idx_pad)
    kT_c = np.ascontiguousarray(kT_full[:, col_idx])
    v_rows = v_full[col_idx, :]
    va_all = np.zeros((TNT * 128, H * (HD + 1)), dtype=F8)
    inv32 = np.asarray(1.0 / 32.0, dtype=F8)[()]
    for h in range(H):
        blk = np.where(maskc[:, None], v_rows[:, h * HD : (h + 1) * HD], np.zeros((), F8))
        va_all[:, h * (HD + 1) : h * (HD + 1) + HD] = blk
        va_all[:, h * (HD + 1) + HD] = np.where(maskc, inv32, np.zeros((), F8))

    ebT8 = (np.ascontiguousarray(
        relative_position_bias[0].transpose(0, 2, 1)) * 64.0).astype(F8)
    eb_c = ebT8[:, idx_pad, :]  # [H, TNV, SQ] fp8

    i2_host = np.zeros((128, 256), dtype=F8)
    i2_host[:, 0:128] = np.eye(128, dtype=np.float32).astype(F8)

    in2 = []
    for c in range(NCORES):
        rs = slice(c * 128, (c + 1) * 128)
        in2.append({
            "qT": np.ascontiguousarray(qT_full[rs, :]),
            "kT": np.ascontiguousarray(kT_c[rs, :]),
            "va": np.ascontiguousarray(
                va_all[:, c * HPC * (HD + 1) : (c + 1) * HPC * (HD + 1)]
            ),
            "va16": np.ascontiguousarray(
                va_all[:, c * HPC * (HD + 1) : (c + 1) * HPC * (HD + 1)]
            ).astype(BF16),
            "eb": np.ascontiguousarray(eb_c[c * HPC : (c + 1) * HPC]),
            "i2": i2_host,
        })
    r2 = _run(_get_program(("p2",) + nvts, build_phase2, nvts), in2)

    # ctx_o[c] is [128 q-part, t, 128 d] for d-block c -> assemble ctxT [D, B*SQ]
    ctxT_full = np.empty((D, B * SQ), dtype=F8)
    for c in range(NCORES):
        blk = r2.results[c]["ctx_o"].reshape(128, B * SQ // 128, 128)
        ctxT_full[c * 128 : (c + 1) * 128, :] = (
            blk.transpose(2, 1, 0).reshape(128, B * SQ)
        )

    # ---------------- phase 3 ----------------
    woT8 = wT8(Wo, 64.0)
    q2d = query.reshape(-1, D)
    resid_h = (q2d + np.asarray(bo, np.float32)[None, :]).astype(BF16)
    trivial = (not np.any(np.asarray(ln_beta))) and np.all(
        np.asarray(ln_gamma, np.float32) == 1.0
    )
    in3 = []
    for c in range(NCORES):
        sl = slice(c * RPC, (c + 1) * RPC)
        d = {
            "ctxT": np.ascontiguousarray(ctxT_full[:, sl]),
            "woT": woT8,
            "resid": np.ascontiguousarray(resid_h[sl, :]),
        }
        if not trivial:
            d["gammab"] = np.ascontiguousarray(
                np.broadcast_to(np.asarray(ln_gamma, np.float32)[None, :], (128, D))
            )
            d["betab"] = np.ascontiguousarray(
                np.broadcast_to(np.asarray(ln_beta, np.float32)[None, :], (128, D))
            )
        in3.append(d)
    r3 = _run(_get_program(("p3", trivial), build_phase3, trivial), in3)

    out = np.empty((B * SQ, D), dtype=np.float32)
    for c in range(NCORES):
        out[c * RPC : (c + 1) * RPC, :] = r3.results[c]["out_o"]

    if _collect_results is not None:
        _collect_results.extend([r1, r2, r3])
    return out.reshape(B, SQ, D)


# revision 23
# speedup vs baseline: 1.0186x; 1.0062x over previous
"""MultiHeadCrossAttention Trainium2 kernel (8 NeuronCores, SPMD).

Problem: B=4, SQ=SK=2048, D=1024, H=16 (HD=64), f32 in/out.

Distribution (3 SPMD launches):
  Phase 1 (row-parallel): QKV projections in fp8 e4m3 with DoubleRow matmuls
    (2 contraction rows per partition -> 0.5 cyc/row). Weights host-prescaled
    by 64; outputs written as scaled fp8 (q*4, k*2, v*1).
  Phase 2 (head-parallel): attention, 2 heads/core. Keys mask-compacted on
    host. Scores S = (4q).(2k) accumulate in PSUM via fp8-DR matmuls over the
    hd=64 contraction split as [32 partitions x 2 slots]; 64*bias (fp8) is
    injected into the same PSUM via a DoubleRow identity matmul, so
    exp(score+bias) needs no elementwise multiply. exp runs split across
    engines: ScalarE true exp (scale=1/64) -> fp8 weights (DR AV matmul), and
    Schraudolph bit-trick exp on DVE/Pool (tensor_scalar -> int16, bitcast to
    bf16 -> plain AV matmul). Mask + normalizer ride as an extra fp8 value
    column; normalization multiplies by 32/norm -> fp8 ctx output.
  Phase 3 (row-parallel): out projection (fp8 DR, ctxT*32 @ woT*64, /2048
    folded into the residual add), one-pass mean/var LayerNorm.
"""

import sys

sys.path.insert(0, "/opt/trn_rl_repo")

import numpy as np
import ml_dtypes

import concourse.bass as bass
import concourse.tile as tile
from concourse import bacc, mybir
from concourse import bass_utils

BF16 = ml_dtypes.bfloat16

B, SQ, SK, D, H = 4, 2048, 2048, 1024, 16
HD = D // H  # 64
NCORES = 8
HPC = H // NCORES          # heads per core = 2
RPC = B * SQ // NCORES     # rows per core (phases 1/3) = 1024
LN_EPS = 1e-5

dt = mybir.dt
AF = mybir.ActivationFunctionType
ALU = mybir.AluOpType
MPM = mybir.MatmulPerfMode

F8 = np.dtype(mybir.dt.np(dt.float8e4))
F32 = np.float32

# Schraudolph fast-exp in bf16 bit space: bf16_bits(exp(x)) ~= x*128*log2e + B
SCH_A = 128.0 * 1.4426950408889634
SCH_B = 127.0 * 128.0 - 0.0436 * 128.0

_programs = {}


# --------------------------------------------------------------------------
# Phase 1: QKV projection (row-parallel, fp8 DoubleRow, no bias on device —
# host folds biases into the outputs if nonzero).
#   inputs (per core): xqT/xkT/xvT [D, RPC] fp8 (input^T), wqT/wkT/wvT
#                      [D, D] fp8 (W^T * 64)
#   outputs: qT_o/kT_o [D, RPC] fp8 (4*q^T, 2*k^T), v_o [RPC, D] fp8 (v)
# --------------------------------------------------------------------------
def build_phase1(reps=1):
    nc = bacc.Bacc("TRN2", debug=False, num_devices=NCORES)
    KC = D // 128  # 8 chunks of 128 = 4 double-chunks

    ins = {}
    for nm in ("xqT", "xkT", "xvT"):
        ins[nm] = nc.dram_tensor(nm, [D, RPC], dt.float8e4, kind="ExternalInput").ap()
    for nm in ("wqT", "wkT", "wvT"):
        ins[nm] = nc.dram_tensor(nm, [D, D], dt.float8e4, kind="ExternalInput").ap()
    qT_o = nc.dram_tensor("qT_o", [D, RPC], dt.float8e4, kind="ExternalOutput").ap()
    kT_o = nc.dram_tensor("kT_o", [D, RPC], dt.float8e4, kind="ExternalOutput").ap()
    v_o = nc.dram_tensor("v_o", [RPC, D], dt.float8e4, kind="ExternalOutput").ap()

    # greedy engine assignment for the 48 PSUM->SBUF scaled copies
    # (GPSIMD/Pool cannot touch PSUM on TRN2, so only Act/DVE)
    eng_cost = {"act": 570.0, "dve": 660.0}
    eng_load = {"act": 0.0, "dve": 0.0}
    copy_plan = []
    for _ in range(3 * (D // 128) * 2):
        e = min(eng_load, key=lambda k: eng_load[k] + eng_cost[k])
        copy_plan.append(e)
        eng_load[e] += eng_cost[e]
    copy_i = [0]

    with tile.TileContext(nc) as tc:
        with (
            tc.tile_pool(name="big", bufs=1) as bigp,
            tc.tile_pool(name="outp", bufs=3) as outp,
            tc.tile_pool(name="ps", bufs=2, space="PSUM") as psp,
        ):
            warm = bigp.tile([1, 1], dt.float32)
            nc.vector.memset(warm[:], 1.0)
            warm2 = bigp.tile([1, 1], dt.float32)
            nc.scalar.activation(warm2[:], warm[:], AF.Copy)
            sb = {}
            for nm in ("xqT", "xkT", "xvT", "wqT", "wkT", "wvT"):
                ncols = ins[nm].shape[1]
                sb[nm] = bigp.tile([128, KC, ncols], dt.float8e4, name=f"{nm}_sb")
            for pair in (("wqT", "xqT"), ("wkT", "xkT"), ("wvT", "xvT")):
                for nm in pair:
                    nc.sync.dma_start(
                        sb[nm][:],
                        ins[nm][:, :].rearrange("(k p) c -> p k c", p=128),
                    )

            def copy_out(dst, src, scale):
                e = copy_plan[copy_i[0] % len(copy_plan)]
                copy_i[0] += 1
                if e == "act":
                    nc.scalar.activation(dst, src, AF.Copy, scale=scale)
                elif e == "dve":
                    nc.vector.tensor_scalar(
                        out=dst, in0=src, scalar1=scale, scalar2=None, op0=ALU.mult
                    )
                else:
                    nc.gpsimd.tensor_scalar(
                        out=dst, in0=src, scalar1=scale, scalar2=None, op0=ALU.mult
                    )

            def proj(x_nm, w_nm, out_dram, transposed_out, scale):
                xt = sb[x_nm]
                wt = sb[w_nm]
                if transposed_out:
                    lt, rt = wt, xt   # out[d_out, rows]
                else:
                    lt, rt = xt, wt   # out[rows, d_out]
                n_m = lt.shape[2] // 128
                n_n = rt.shape[2] // 512
                MG = 2
                for mg in range(0, n_m, MG):
                    ms = range(mg, min(mg + MG, n_m))
                    pss = {}
                    for m in ms:
                        for n in range(n_n):
                            pss[m, n] = psp.tile(
                                [128, 512], dt.float32, name="ps", tag=f"ps{m % MG}_{n}"
                            )
                    for k2 in range(KC // 2):
                        for m in ms:
                            for n in range(n_n):
                                nc.tensor.matmul(
                                    pss[m, n][:],
                                    lhsT=lt[:, 2 * k2 : 2 * k2 + 2, m * 128 : (m + 1) * 128],
                                    rhs=rt[:, 2 * k2 : 2 * k2 + 2, n * 512 : (n + 1) * 512],
                                    start=(k2 == 0),
                                    stop=(k2 == KC // 2 - 1),
                                    perf_mode=MPM.DoubleRow,
                                )
                    osb = outp.tile(
                        [128, MG, rt.shape[2]], dt.float8e4, name=f"{x_nm}_osb", tag="osb"
                    )
                    for m in ms:
                        for n in range(n_n):
                            copy_out(osb[:, m - mg, n * 512 : (n + 1) * 512], pss[m, n][:], scale)
                    nc.sync.dma_start(
                        out_dram[mg * 128 : (mg + MG) * 128, :].rearrange(
                            "(g p) c -> p g c", p=128
                        ),
                        osb[:],
                    )

            for _ in range(reps):
                proj("xqT", "wqT", qT_o, True, 4.0 / 64.0)
                proj("xkT", "wkT", kT_o, True, 2.0 / 64.0)
                proj("xvT", "wvT", v_o, False, 1.0 / 64.0)

    nc.compile()
    return nc


# --------------------------------------------------------------------------
# Phase 2: attention (head-parallel, 2 heads/core).
#   inputs (per core):
#     qT  [128, B*SQ] fp8  (rows = 2 heads x 64 dims, = 4*q^T)
#     kT  [128, TNV] fp8   (compacted, = 2*k^T)
#     va  [TNV, HPC*(HD+1)] fp8 (v*mask | mask column per head)
#     va16 same as va in bf16 (for the Schraudolph bf16 AV matmuls)
#     eb  [HPC, TNV, SQ] fp8 (64 * bias^T per head, compacted rows)
#     i2  [128, 256] fp8   (DoubleRow identity: [:, :128]=I, [:, 128:]=0)
#   outputs: ctx_o [128, B*SQ] fp8 = 32*ctx/norm in [p, t, d] layout
# --------------------------------------------------------------------------
def build_phase2(nvts=(8, 8, 8, 8), reps=1, sp_bufs=6, cp_bufs=2, wm_bufs=4):
    nc = bacc.Bacc("TRN2", debug=False, num_devices=NCORES)
    QC = 512
    NQC = SQ // QC
    snvt = [0]
    for t in nvts:
        snvt.append(snvt[-1] + t)
    TNT = snvt[-1]
    TNV = TNT * 128
    NTMAX = max(nvts)

    qT = nc.dram_tensor("qT", [128, B * SQ], dt.float8e4, kind="ExternalInput").ap()
    kT = nc.dram_tensor("kT", [128, TNV], dt.float8e4, kind="ExternalInput").ap()
    va = nc.dram_tensor("va", [TNV, HPC * (HD + 1)], dt.float8e4, kind="ExternalInput").ap()
    va16 = nc.dram_tensor("va16", [TNV, HPC * (HD + 1)], dt.bfloat16, kind="ExternalInput").ap()
    eb = nc.dram_tensor("eb", [HPC, TNV, SQ], dt.float8e4, kind="ExternalInput").ap()
    i2 = nc.dram_tensor("i2", [128, 256], dt.float8e4, kind="ExternalInput").ap()
    ctx_o = nc.dram_tensor("ctx_o", [128, B * SQ], dt.float8e4, kind="ExternalOutput").ap()

    with tile.TileContext(nc) as tc:
        with (
            tc.tile_pool(name="big", bufs=1) as bigp,
            tc.tile_pool(name="ebp", bufs=5) as ebp,
            tc.tile_pool(name="wp", bufs=wm_bufs) as wp,
            tc.tile_pool(name="ip", bufs=wm_bufs) as ip,
            tc.tile_pool(name="np_", bufs=6) as normp,
            tc.tile_pool(name="Sp", bufs=sp_bufs, space="PSUM") as Sp,
            tc.tile_pool(name="cp", bufs=cp_bufs, space="PSUM") as cp,
        ):
            # hd-split layouts for DoubleRow: [32 partitions, 2 slots, head, cols]
            qT_sb = bigp.tile([32, 2, HPC, B * SQ], dt.float8e4)
            kT_sb = bigp.tile([32, 2, HPC, TNV], dt.float8e4)
            va_sb = bigp.tile([128, TNT, HPC * (HD + 1)], dt.float8e4)
            va16_sb = bigp.tile([128, TNT, HPC * (HD + 1)], dt.bfloat16)
            i2_sb = bigp.tile([128, 2, 128], dt.float8e4)
            warm = bigp.tile([1, 1], dt.float32)
            nc.vector.memset(warm[:], 0.0)
            warm2 = bigp.tile([1, 1], dt.float32)
            nc.scalar.activation(warm2[:], warm[:], AF.Exp)

            def load_qk(b):
                for h in range(HPC):
                    nc.sync.dma_start(
                        qT_sb[:, :, h, b * SQ : (b + 1) * SQ],
                        qT[64 * h : 64 * h + 64, b * SQ : (b + 1) * SQ].rearrange(
                            "(s p) c -> p s c", p=32
                        ),
                    )
                    cs, ce = snvt[b] * 128, snvt[b + 1] * 128
                    nc.sync.dma_start(
                        kT_sb[:, :, h, cs:ce],
                        kT[64 * h : 64 * h + 64, cs:ce].rearrange("(s p) c -> p s c", p=32),
                    )

            def load_va(b):
                cs, ce = snvt[b] * 128, snvt[b + 1] * 128
                nc.sync.dma_start(
                    va_sb[:, snvt[b] : snvt[b + 1], :],
                    va[cs:ce, :].rearrange("(t p) d -> p t d", p=128),
                )
                nc.sync.dma_start(
                    va16_sb[:, snvt[b] : snvt[b + 1], :],
                    va16[cs:ce, :].rearrange("(t p) d -> p t d", p=128),
                )

            def load_b(b):
                load_qk(b)
                load_va(b)

            load_qk(0)
            nc.sync.dma_start(i2_sb[:], i2[:])

            iters = [(qc, b) for qc in range(NQC) for b in range(B)] * reps

            def load_slab(qc, b, split=False, kj_range=None):
                NT = nvts[b]
                eb_sb = ebp.tile(
                    [128, NTMAX + 1, HPC, QC], dt.float8e4, name="eb_sb", tag="eb"
                )
                src_r = eb[:, snvt[b] * 128 : snvt[b + 1] * 128, :].rearrange(
                    "h (t p) q -> h p t q", p=128
                )[:, :, :, qc * QC : (qc + 1) * QC]

                def emit(kjs):
                    for kj in kjs:
                        for h in range(HPC):
                            nc.sync.dma_start(eb_sb[:, kj, h, :], src_r[h, :, kj, :])

                if split:
                    emit(range(NT) if kj_range is None else kj_range)
                else:
                    for h in range(HPC):
                        nc.sync.dma_start(eb_sb[:, 0:NT, h, :], src_r[h])
                # pad tile (read by the DR inject's zero slot on the last key
                # tile) must be initialized for the race detector
                if kj_range is None or list(kj_range)[-1] == NT - 1:
                    nc.gpsimd.memset(eb_sb[:, NT, :, :], 0.0)
                return eb_sb, emit

            slabs = {}
            # first two key tiles of iteration 0 land before the va bulk loads
            eb0, emit0 = load_slab(*iters[0], split=True, kj_range=range(2))
            load_va(0)
            emit0(range(2, nvts[iters[0][1]]))
            nc.gpsimd.memset(eb0[:, nvts[iters[0][1]], :, :], 0.0)
            slabs[0] = eb0
            for b in range(1, B):
                load_b(b)
                slabs[b], _ = load_slab(*iters[b], split=(b == 1))

            def emit_norm_piece(state):
                # mask column is 1/32, so 1/normcol = 32/sum(w): the x32 ctx
                # scaling is free. Pieces alternate DVE / Act to balance load.
                ctx, col0, holder = state
                if holder[0] is None:
                    holder[0] = normp.tile(
                        [128, QC // 128, HPC * HD], dt.float8e4, name="ctxn", tag="ctxn"
                    )
                ctxn = holder[0]
                t = holder[1]
                holder[1] += 1
                ti, tt = t // 2, t % 2
                for h in range(HPC):
                    rec = normp.tile([128, 1], dt.float32, name="rec", tag="rec")
                    nc.vector.reciprocal(
                        rec[:], ctx[ti][:, tt, h * (HD + 1) + HD : h * (HD + 1) + HD + 1]
                    )
                    if t != 3:
                        nc.vector.tensor_scalar(
                            out=ctxn[:, t, h * HD : (h + 1) * HD],
                            in0=ctx[ti][:, tt, h * (HD + 1) : h * (HD + 1) + HD],
                            scalar1=rec[:],
                            scalar2=None,
                            op0=ALU.mult,
                        )
                    else:
                        nc.scalar.activation(
                            ctxn[:, t, h * HD : (h + 1) * HD],
                            ctx[ti][:, tt, h * (HD + 1) : h * (HD + 1) + HD],
                            AF.Copy,
                            scale=rec[:],
                        )
                if t == QC // 128 - 1:
                    nc.sync.dma_start(ctx_o[:, col0 : col0 + QC], ctxn[:])

            def emit_norm(state):
                while state[2][1] < QC // 128:
                    emit_norm_piece(state)

            def emit_av_pair(ctx, tbase, pj, wm2, start, stop):
                # DoubleRow fp8 AV over a kj pair
                for ti in range(QC // 256):
                    for tt in range(2):
                        for h in range(HPC):
                            t = ti * 2 + tt
                            nc.tensor.matmul(
                                ctx[ti][:, tt, h * (HD + 1) : (h + 1) * (HD + 1)],
                                lhsT=wm2[:, :, h, t * 128 : (t + 1) * 128],
                                rhs=va_sb[:, tbase + 2 * pj : tbase + 2 * pj + 2,
                                          h * (HD + 1) : (h + 1) * (HD + 1)],
                                start=start and (tt == 0) and (h == 0),
                                stop=stop and (ti == QC // 256 - 1) and (tt == 1) and (h == HPC - 1),
                                perf_mode=MPM.DoubleRow,
                                skip_group_check=True,
                            )

            def emit_av_sch(ctx, tbase, kj, i16, sl, start, stop):
                # plain bf16 AV for one Schraudolph kj tile (bitcast int16 weights)
                for ti in range(QC // 256):
                    for tt in range(2):
                        for h in range(HPC):
                            t = ti * 2 + tt
                            nc.tensor.matmul(
                                ctx[ti][:, tt, h * (HD + 1) : (h + 1) * (HD + 1)],
                                lhsT=i16[:, sl, h, t * 128 : (t + 1) * 128].bitcast(dt.bfloat16),
                                rhs=va16_sb[:, tbase + kj, h * (HD + 1) : (h + 1) * (HD + 1)],
                                start=start and (tt == 0) and (h == 0),
                                stop=stop and (ti == QC // 256 - 1) and (tt == 1) and (h == HPC - 1),
                                skip_group_check=True,
                            )

            def emit_av_one(ctx, tbase, kj, wm1, start, stop):
                # plain fp8 AV for the odd tail tile
                for ti in range(QC // 256):
                    for tt in range(2):
                        for h in range(HPC):
                            t = ti * 2 + tt
                            nc.tensor.matmul(
                                ctx[ti][:, tt, h * (HD + 1) : (h + 1) * (HD + 1)],
                                lhsT=wm1[:, 0, h, t * 128 : (t + 1) * 128],
                                rhs=va_sb[:, tbase + kj, h * (HD + 1) : (h + 1) * (HD + 1)],
                                start=start and (tt == 0) and (h == 0),
                                stop=stop and (ti == QC // 256 - 1) and (tt == 1) and (h == HPC - 1),
                                skip_group_check=True,
                            )

            tail_av = []     # AV thunks deferred from the previous kj
            tail_norm = None

            for it_i, (qc, b) in enumerate(iters):
                NT = nvts[b]
                NP = NT // 2
                eb_sb = slabs.pop(it_i)
                if it_i + 4 < len(iters):
                    slabs[it_i + 4], _ = load_slab(*iters[it_i + 4])
                ctx = [
                    cp.tile([128, 2, HPC * (HD + 1)], dt.float32, name=f"ctx{t}", tag="ctx")
                    for t in range(QC // 256)
                ]
                col0 = b * SQ + qc * QC
                tbase = snvt[b]

                def make_S(kj):
                    # per-head 1-bank S tiles -> deeper PSUM pipeline
                    Ss = []
                    kcol = tbase * 128 + kj * 128
                    for h in range(HPC):
                        S = Sp.tile([128, QC], dt.float32, name="S", tag="S")
                        nc.tensor.matmul(
                            S[:],
                            lhsT=kT_sb[:, :, h, kcol : kcol + 128],
                            rhs=qT_sb[:, :, h, col0 : col0 + QC],
                            start=True,
                            stop=False,
                            perf_mode=MPM.DoubleRow,
                            skip_group_check=True,
                        )
                        nc.tensor.matmul(
                            S[:],
                            lhsT=i2_sb[:],
                            rhs=eb_sb[:, kj : kj + 2, h, :],
                            start=False,
                            stop=True,
                            perf_mode=MPM.DoubleRow,
                            skip_group_check=True,
                        )
                        Ss.append(S)
                    return Ss

                first_av = [True]
                wm2_cur = [None]
                i16_cur = [None]
                for kj in range(NT):
                    S = make_S(kj)
                    # drain the deferred AVs / previous iteration's norm
                    if tail_av:
                        fin = (kj == 0)
                        for j, (fn, args) in enumerate(tail_av):
                            fn(*args, stop=(fin and j == len(tail_av) - 1) if fin else False)
                        tail_av = []
                    if tail_norm is not None and kj >= 1:
                        emit_norm_piece(tail_norm)
                        if tail_norm[2][1] >= QC // 128:
                            tail_norm = None
                    pj = kj // 2
                    is_odd_tail = (kj == NT - 1) and (NT % 2 == 1)
                    path = "act" if (is_odd_tail or pj % 2 == 0) else "sch"
                    if path == "act":
                        if is_odd_tail:
                            wm1 = wp.tile([128, 1, HPC, QC], dt.float8e4, name="wm1", tag="wm1")
                            for h in range(HPC):
                                nc.scalar.activation(
                                    wm1[:, 0, h], S[h][:], AF.Exp, scale=1.0 / 64.0
                                )
                            tail_av.append((emit_av_one, [ctx, tbase, kj, wm1, first_av[0]]))
                            first_av[0] = False
                        else:
                            if kj % 2 == 0:
                                wm2_cur[0] = wp.tile(
                                    [128, 2, HPC, QC], dt.float8e4, name="wm2", tag="wm2"
                                )
                            for h in range(HPC):
                                nc.scalar.activation(
                                    wm2_cur[0][:, kj % 2, h], S[h][:], AF.Exp, scale=1.0 / 64.0
                                )
                            if kj % 2 == 1:
                                tail_av.append(
                                    (emit_av_pair, [ctx, tbase, pj, wm2_cur[0], first_av[0]])
                                )
                                first_av[0] = False
                    else:
                        if kj % 2 == 0 or is_odd_tail:
                            i16_cur[0] = ip.tile(
                                [128, 2, HPC, QC], dt.int16, name="i16", tag="i16"
                            )
                        sl_ = 0 if is_odd_tail else kj % 2
                        for h in range(HPC):
                            nc.vector.tensor_scalar(
                                out=i16_cur[0][:, sl_, h], in0=S[h][:], scalar1=SCH_A / 64.0,
                                scalar2=SCH_B, op0=ALU.mult, op1=ALU.add,
                            )
                        tail_av.append(
                            (emit_av_sch, [ctx, tbase, kj, i16_cur[0], sl_, first_av[0]])
                        )
                        first_av[0] = False

                if tail_norm is not None:
                    emit_norm(tail_norm)   # short iterations: flush leftovers
                tail_norm = (ctx, col0, [None, 0])

            for j, (fn, args) in enumerate(tail_av):
                fn(*args, stop=(j == len(tail_av) - 1))
            emit_norm(tail_norm)

    nc.compile()
    return nc


# --------------------------------------------------------------------------
# Phase 3: out projection + residual + LayerNorm (row-parallel, fp8 DR GEMM).
#   inputs (per core): ctxT [D, RPC] fp8 (=32*ctx^T), woT [D, D] fp8 (=64*Wo^T),
#     resid [RPC, D] bf16 (query rows + bo), [gammab/betab [128, D] f32 if
#     not trivial_ln]
#   outputs: out_o [RPC, D] f32
# --------------------------------------------------------------------------
def build_phase3(trivial_ln=True, reps=1):
    nc = bacc.Bacc("TRN2", debug=False, num_devices=NCORES)
    KC = D // 128

    ctxT = nc.dram_tensor("ctxT", [D, RPC], dt.float8e4, kind="ExternalInput").ap()
    woT = nc.dram_tensor("woT", [D, D], dt.float8e4, kind="ExternalInput").ap()
    resid = nc.dram_tensor("resid", [RPC, D], dt.bfloat16, kind="ExternalInput").ap()
    if not trivial_ln:
        gammab = nc.dram_tensor("gammab", [128, D], dt.float32, kind="ExternalInput").ap()
        betab = nc.dram_tensor("betab", [128, D], dt.float32, kind="ExternalInput").ap()
    out_o = nc.dram_tensor("out_o", [RPC, D], dt.float32, kind="ExternalOutput").ap()
    PS_SCALE = 1.0 / (32.0 * 64.0)

    with tile.TileContext(nc) as tc:
        with (
            tc.tile_pool(name="big", bufs=1) as bigp,
            tc.tile_pool(name="rp", bufs=4) as rp,
            tc.tile_pool(name="wk", bufs=3) as wk,
            tc.tile_pool(name="ps", bufs=6, space="PSUM") as psp,
        ):
            ctx_sb = bigp.tile([128, KC, RPC], dt.float8e4)
            wo_sb = bigp.tile([128, KC, D], dt.float8e4)
            nc.sync.dma_start(
                ctx_sb[:], ctxT[:, :].rearrange("(k p) c -> p k c", p=128)
            )
            nc.sync.dma_start(
                wo_sb[:], woT[:, :].rearrange("(k p) c -> p k c", p=128)
            )
            eps_sb = bigp.tile([128, 1], dt.float32)
            nc.vector.memset(eps_sb[:], LN_EPS)
            warm = bigp.tile([1, 1], dt.float32)
            nc.vector.memset(warm[:], 1.0)
            warm2 = bigp.tile([1, 1], dt.float32)
            nc.scalar.activation(warm2[:], warm[:], AF.Sqrt)
            warm3 = bigp.tile([1, 1], dt.float32)
            nc.scalar.activation(warm3[:], warm[:], AF.Square)
            if not trivial_ln:
                gam_sb = bigp.tile([128, D], dt.float32)
                nc.sync.dma_start(gam_sb[:], gammab[:])
                bet_sb = bigp.tile([128, D], dt.float32)
                nc.sync.dma_start(bet_sb[:], betab[:])

            for m in [m for _ in range(reps) for m in range(RPC // 128)]:
                res_sb = rp.tile([128, D], dt.bfloat16, name="res_sb", tag="res")
                nc.sync.dma_start(res_sb[:], resid[m * 128 : (m + 1) * 128, :])
                ps = [psp.tile([128, 512], dt.float32, name=f"ps{n}", tag="ps") for n in range(2)]
                for n in range(2):
                    for k2 in range(KC // 2):
                        nc.tensor.matmul(
                            ps[n][:],
                            lhsT=ctx_sb[:, 2 * k2 : 2 * k2 + 2, m * 128 : (m + 1) * 128],
                            rhs=wo_sb[:, 2 * k2 : 2 * k2 + 2, n * 512 : (n + 1) * 512],
                            start=(k2 == 0),
                            stop=(k2 == KC // 2 - 1),
                            perf_mode=MPM.DoubleRow,
                        )
                x_sb = wk.tile([128, D], dt.float32, name="x_sb", tag="x")
                acc = [wk.tile([128, 1], dt.float32, name=f"acc{n}", tag=f"acc{n}") for n in range(2)]
                for n in range(2):
                    nc.vector.scalar_tensor_tensor(
                        out=x_sb[:, n * 512 : (n + 1) * 512],
                        in0=ps[n][:],
                        scalar=PS_SCALE,
                        in1=res_sb[:, n * 512 : (n + 1) * 512],
                        op0=ALU.mult,
                        op1=ALU.add,
                        accum_out=acc[n][:],
                    )
                mu = wk.tile([128, 1], dt.float32, name="mu", tag="mu")
                nc.vector.tensor_scalar(
                    out=mu[:], in0=acc[0][:], scalar1=acc[1][:], scalar2=1.0 / D,
                    op0=ALU.add, op1=ALU.mult,
                )
                sq = wk.tile([128, D], dt.bfloat16, name="sq", tag="sq")
                s2 = wk.tile([128, 1], dt.float32, name="s2", tag="s2")
                nc.scalar.activation(sq[:], x_sb[:], AF.Square, accum_out=s2[:])
                var = wk.tile([128, 1], dt.float32, name="var", tag="var")
                # var = s2/D - mu^2  (one fused op: (s2*(1/D)) - mu2)
                mu2 = wk.tile([128, 1], dt.float32, name="mu2", tag="mu2")
                nc.vector.tensor_tensor(mu2[:], mu[:], mu[:], op=ALU.mult)
                nc.vector.tensor_scalar(
                    out=var[:], in0=s2[:], scalar1=1.0 / D, scalar2=mu2[:],
                    op0=ALU.mult, op1=ALU.subtract,
                )
                std = wk.tile([128, 1], dt.float32, name="std", tag="std")
                nc.scalar.activation(std[:], var[:], AF.Sqrt, bias=eps_sb[:])
                rstd = wk.tile([128, 1], dt.float32, name="rstd", tag="rstd")
                nc.vector.reciprocal(rstd[:], std[:])
                mrs = wk.tile([128, 1], dt.float32, name="mrs", tag="mrs")
                nc.vector.tensor_tensor(mrs[:], mu[:], rstd[:], op=ALU.mult)
                out_sb = wk.tile([128, D], dt.float32, name="out_sb", tag="out_sb")
                if trivial_ln:
                    nc.vector.tensor_scalar(
                        out=out_sb[:], in0=x_sb[:], scalar1=rstd[:], scalar2=mrs[:],
                        op0=ALU.mult, op1=ALU.subtract,
                    )
                else:
                    tmp = wk.tile([128, D], dt.float32, name="tmp", tag="tmp")
                    nc.vector.tensor_scalar(
                        out=tmp[:], in0=x_sb[:], scalar1=rstd[:], scalar2=mrs[:],
                        op0=ALU.mult, op1=ALU.subtract,
                    )
                    y = wk.tile([128, D], dt.float32, name="y", tag="y")
                    nc.vector.scalar_tensor_tensor(
                        out=y[:], in0=tmp[:], scalar=0.0, in1=gam_sb[:],
                        op0=ALU.add, op1=ALU.mult,
                    )
                    nc.gpsimd.tensor_add(out_sb[:], y[:], bet_sb[:])
                nc.sync.dma_start(out_o[m * 128 : (m + 1) * 128, :], out_sb[:])

    nc.compile()
    return nc


def _get_program(key, builder, *args, **kwargs):
    if key not in _programs:
        _programs[key] = builder(*args, **kwargs)
    return _programs[key]


def _run(nc, in_maps):
    return bass_utils.run_bass_kernel_spmd(nc, in_maps, core_ids=list(range(NCORES)))


def kernel(query, key, value, attention_mask, relative_position_bias,
           Wq, bq, Wk, bk, Wv, bv, Wo, bo, ln_gamma, ln_beta,
           _collect_results=None):
    query = np.asarray(query, dtype=np.float32)
    key = np.asarray(key, dtype=np.float32)
    value = np.asarray(value, dtype=np.float32)
    attention_mask = np.asarray(attention_mask)
    relative_position_bias = np.asarray(relative_position_bias, dtype=np.float32)

    def xT8(x):
        return np.ascontiguousarray(x.reshape(-1, D).T).astype(F8)

    def wT8(W, scale):
        return (np.ascontiguousarray(np.asarray(W, np.float32).T) * scale).astype(F8)

    xqT = xT8(query)
    xkT = xT8(key)
    xvT = xT8(value)
    wqT = wT8(Wq, 64.0)
    wkT = wT8(Wk, 64.0)
    wvT = wT8(Wv, 64.0)

    # ---------------- phase 1 ----------------
    in1 = []
    for c in range(NCORES):
        sl = slice(c * RPC, (c + 1) * RPC)
        in1.append({
            "xqT": np.ascontiguousarray(xqT[:, sl]),
            "xkT": np.ascontiguousarray(xkT[:, sl]),
            "xvT": np.ascontiguousarray(xvT[:, sl]),
            "wqT": wqT, "wkT": wkT, "wvT": wvT,
        })
    r1 = _run(_get_program("p1", build_phase1), in1)

    qT_full = np.empty((D, B * SQ), dtype=F8)
    kT_full = np.empty((D, B * SK), dtype=F8)
    v_full = np.empty((B * SK, D), dtype=F8)
    for c in range(NCORES):
        sl = slice(c * RPC, (c + 1) * RPC)
        qT_full[:, sl] = r1.results[c]["qT_o"]
        kT_full[:, sl] = r1.results[c]["kT_o"]
        v_full[sl, :] = r1.results[c]["v_o"]

    # fold any nonzero projection biases in on the host (zero in practice)
    if np.any(np.asarray(bq)):
        qT_full = (qT_full.astype(np.float32)
                   + 4.0 * np.asarray(bq, np.float32)[:, None]).astype(F8)
    if np.any(np.asarray(bk)):
        kT_full = (kT_full.astype(np.float32)
                   + 2.0 * np.asarray(bk, np.float32)[:, None]).astype(F8)
    if np.any(np.asarray(bv)):
        v_full = (v_full.astype(np.float32)
                  + np.asarray(bv, np.float32)[None, :]).astype(F8)

    # ---------------- phase 2 ----------------
    mask2 = (attention_mask.reshape(B, SK) != 0)
    valid = [np.nonzero(mask2[b])[0] for b in range(B)]
    nvts = tuple(max(1, -(-len(ix) // 128)) for ix in valid)
    snvt = np.concatenate([[0], np.cumsum(nvts)]).astype(int)
    TNT = int(snvt[-1])
    idx_pad = np.zeros(TNT * 128, dtype=np.int64)
    maskc = np.zeros((TNT * 128,), dtype=bool)
    for b in range(B):
        ix = valid[b]
        o = snvt[b] * 128
        idx_pad[o : o + len(ix)] = ix
        maskc[o : o + len(ix)] = True

    col_idx = (np.repeat(np.arange(B) * SK, np.array(nvts) * 128) + idx_pad)
    kT_c = np.ascontiguousarray(kT_full[:, col_idx])
    v_rows = v_full[col_idx, :]
    va_all = np.zeros((TNT * 128, H * (HD + 1)), dtype=F8)
    inv32 = np.asarray(1.0 / 32.0, dtype=F8)[()]
    for h in range(H):
        blk = np.where(maskc[:, None], v_rows[:, h * HD : (h + 1) * HD], np.zeros((), F8))
        va_all[:, h * (HD + 1) : h * (HD + 1) + HD] = blk
        va_all[:, h * (HD + 1) + HD] = np.where(maskc, inv32, np.zeros((), F8))

    ebT8 = (np.ascontiguousarray(
        relative_position_bias[0].transpose(0, 2, 1)) * 64.0).astype(F8)
    eb_c = ebT8[:, idx_pad, :]  # [H, TNV, SQ] fp8

    i2_host = np.zeros((128, 256), dtype=F8)
    i2_host[:, 0:128] = np.eye(128, dtype=np.float32).astype(F8)

    in2 = []
    for c in range(NCORES):
        rs = slice(c * 128, (c + 1) * 128)
        in2.append({
            "qT": np.ascontiguousarray(qT_full[rs, :]),
            "kT": np.ascontiguousarray(kT_c[rs, :]),
            "va": np.ascontiguousarray(
                va_all[:, c * HPC * (HD + 1) : (c + 1) * HPC * (HD + 1)]
            ),
            "va16": np.ascontiguousarray(
                va_all[:, c * HPC * (HD + 1) : (c + 1) * HPC * (HD + 1)]
            ).astype(BF16),
            "eb": np.ascontiguousarray(eb_c[c * HPC : (c + 1) * HPC]),
            "i2": i2_host,
        })
    r2 = _run(_get_program(("p2",) + nvts, build_phase2, nvts), in2)

    # ctx_o[c] is [128 q-part, t, 128 d] for d-block c -> assemble ctxT [D, B*SQ]
    ctxT_full = np.empty((D, B * SQ), dtype=F8)
    for c in range(NCORES):
        blk = r2.results[c]["ctx_o"].reshape(128, B * SQ // 128, 128)
        ctxT_full[c * 128 : (c + 1) * 128, :] = (
            blk.transpose(2, 1, 0).reshape(128, B * SQ)
        )

    # ---------------- phase 3 ----------------
    woT8 = wT8(Wo, 64.0)
    q2d = query.reshape(-1, D)
    resid_h = (q2d + np.asarray(bo, np.float32)[None, :]).astype(BF16)
    trivial = (not np.any(np.asarray(ln_beta))) and np.all(
        np.asarray(ln_gamma, np.float32) == 1.0
    )
    in3 = []
    for c in range(NCORES):
        sl = slice(c * RPC, (c + 1) * RPC)
        d = {
            "ctxT": np.ascontiguousarray(ctxT_full[:, sl]),
            "woT": woT8,
            "resid": np.ascontiguousarray(resid_h[sl, :]),
        }
        if not trivial:
            d["gammab"] = np.ascontiguousarray(
                np.broadcast_to(np.asarray(ln_gamma, np.float32)[None, :], (128, D))
            )
            d["betab"] = np.ascontiguousarray(
                np.broadcast_to(np.asarray(ln_beta, np.float32)[None, :], (128, D))
            )
        in3.append(d)
    r3 = _run(_get_program(("p3", trivial), build_phase3, trivial), in3)

    out = np.empty((B * SQ, D), dtype=np.float32)
    for c in range(NCORES):
        out[c * RPC : (c + 1) * RPC, :] = r3.results[c]["out_o"]

    if _collect_results is not None:
        _collect_results.extend([r1, r2, r3])
    return out.reshape(B, SQ, D)


# revision 26
# speedup vs baseline: 1.0470x; 1.0278x over previous
"""MultiHeadCrossAttention Trainium2 kernel (8 NeuronCores, SPMD).

Problem: B=4, SQ=SK=2048, D=1024, H=16 (HD=64), f32 in/out.

Distribution (3 SPMD launches):
  Phase 1 (row-parallel): QKV projections in fp8 e4m3 with DoubleRow matmuls
    (2 contraction rows per partition -> 0.5 cyc/row). Weights host-prescaled
    by 64; outputs written as scaled fp8 (q*4, k*2, v*1).
  Phase 2 (head-parallel): attention, 2 heads/core. Keys mask-compacted on
    host. Scores S = (4q).(2k) accumulate in PSUM via fp8-DR matmuls over the
    hd=64 contraction split as [32 partitions x 2 slots]; 64*bias (fp8) is
    injected into the same PSUM via a DoubleRow identity matmul, so
    exp(score+bias) needs no elementwise multiply. exp runs split across
    engines: ScalarE true exp (scale=1/64) -> fp8 weights (DR AV matmul), and
    Schraudolph bit-trick exp on DVE/Pool (tensor_scalar -> int16, bitcast to
    bf16 -> plain AV matmul). Mask + normalizer ride as an extra fp8 value
    column; normalization multiplies by 32/norm -> fp8 ctx output.
  Phase 3 (row-parallel): out projection (fp8 DR, ctxT*32 @ woT*64, /2048
    folded into the residual add), one-pass mean/var LayerNorm.
"""

import sys

sys.path.insert(0, "/opt/trn_rl_repo")

import numpy as np
import ml_dtypes

import concourse.bass as bass
import concourse.tile as tile
from concourse import bacc, mybir
from concourse import bass_utils

BF16 = ml_dtypes.bfloat16

B, SQ, SK, D, H = 4, 2048, 2048, 1024, 16
HD = D // H  # 64
NCORES = 8
HPC = H // NCORES          # heads per core = 2
RPC = B * SQ // NCORES     # rows per core (phases 1/3) = 1024
LN_EPS = 1e-5

dt = mybir.dt
AF = mybir.ActivationFunctionType
ALU = mybir.AluOpType
MPM = mybir.MatmulPerfMode

F8 = np.dtype(mybir.dt.np(dt.float8e4))
F32 = np.float32

# Schraudolph fast-exp in bf16 bit space: bf16_bits(exp(x)) ~= x*128*log2e + B
SCH_A = 128.0 * 1.4426950408889634
SCH_B = 127.0 * 128.0 - 0.0436 * 128.0

_programs = {}


# --------------------------------------------------------------------------
# Phase 1: QKV projection (row-parallel, fp8 DoubleRow, no bias on device —
# host folds biases into the outputs if nonzero).
#   inputs (per core): xqT/xkT/xvT [D, RPC] fp8 (input^T), wqT/wkT/wvT
#                      [D, D] fp8 (W^T * 64)
#   outputs: qT_o/kT_o [D, RPC] fp8 (4*q^T, 2*k^T), v_o [RPC, D] fp8 (v)
# --------------------------------------------------------------------------
def build_phase1(reps=1):
    nc = bacc.Bacc("TRN2", debug=False, num_devices=NCORES)
    KC = D // 128  # 8 chunks of 128 = 4 double-chunks

    ins = {}
    for nm in ("xqT", "xkT", "xvT"):
        ins[nm] = nc.dram_tensor(nm, [D, RPC], dt.float8e4, kind="ExternalInput").ap()
    for nm in ("wqT", "wkT", "wvT"):
        ins[nm] = nc.dram_tensor(nm, [D, D], dt.float8e4, kind="ExternalInput").ap()
    qT_o = nc.dram_tensor("qT_o", [D, RPC], dt.float8e4, kind="ExternalOutput").ap()
    kT_o = nc.dram_tensor("kT_o", [D, RPC], dt.float8e4, kind="ExternalOutput").ap()
    v_o = nc.dram_tensor("v_o", [RPC, D], dt.float8e4, kind="ExternalOutput").ap()

    # greedy engine assignment for the 48 PSUM->SBUF scaled copies
    # (GPSIMD/Pool cannot touch PSUM on TRN2, so only Act/DVE)
    eng_cost = {"act": 570.0, "dve": 660.0}
    eng_load = {"act": 0.0, "dve": 0.0}
    copy_plan = []
    for _ in range(3 * (D // 128) * 2):
        e = min(eng_load, key=lambda k: eng_load[k] + eng_cost[k])
        copy_plan.append(e)
        eng_load[e] += eng_cost[e]
    copy_i = [0]

    with tile.TileContext(nc) as tc:
        with (
            tc.tile_pool(name="big", bufs=1) as bigp,
            tc.tile_pool(name="outp", bufs=3) as outp,
            tc.tile_pool(name="ps", bufs=2, space="PSUM") as psp,
        ):
            warm = bigp.tile([1, 1], dt.float32)
            nc.vector.memset(warm[:], 1.0)
            warm2 = bigp.tile([1, 1], dt.float32)
            nc.scalar.activation(warm2[:], warm[:], AF.Copy)
            sb = {}
            for nm in ("xqT", "xkT", "xvT", "wqT", "wkT", "wvT"):
                ncols = ins[nm].shape[1]
                sb[nm] = bigp.tile([128, KC, ncols], dt.float8e4, name=f"{nm}_sb")
            for pair in (("wqT", "xqT"), ("wkT", "xkT"), ("wvT", "xvT")):
                for nm in pair:
                    nc.sync.dma_start(
                        sb[nm][:],
                        ins[nm][:, :].rearrange("(k p) c -> p k c", p=128),
                    )

            def copy_out(dst, src, scale):
                e = copy_plan[copy_i[0] % len(copy_plan)]
                copy_i[0] += 1
                if e == "act":
                    nc.scalar.activation(dst, src, AF.Copy, scale=scale)
                elif e == "dve":
                    nc.vector.tensor_scalar(
                        out=dst, in0=src, scalar1=scale, scalar2=None, op0=ALU.mult
                    )
                else:
                    nc.gpsimd.tensor_scalar(
                        out=dst, in0=src, scalar1=scale, scalar2=None, op0=ALU.mult
                    )

            def proj(x_nm, w_nm, out_dram, transposed_out, scale):
                xt = sb[x_nm]
                wt = sb[w_nm]
                if transposed_out:
                    lt, rt = wt, xt   # out[d_out, rows]
                else:
                    lt, rt = xt, wt   # out[rows, d_out]
                n_m = lt.shape[2] // 128
                n_n = rt.shape[2] // 512
                MG = 2
                for mg in range(0, n_m, MG):
                    ms = range(mg, min(mg + MG, n_m))
                    pss = {}
                    for m in ms:
                        for n in range(n_n):
                            pss[m, n] = psp.tile(
                                [128, 512], dt.float32, name="ps", tag=f"ps{m % MG}_{n}"
                            )
                    for k2 in range(KC // 2):
                        for m in ms:
                            for n in range(n_n):
                                nc.tensor.matmul(
                                    pss[m, n][:],
                                    lhsT=lt[:, 2 * k2 : 2 * k2 + 2, m * 128 : (m + 1) * 128],
                                    rhs=rt[:, 2 * k2 : 2 * k2 + 2, n * 512 : (n + 1) * 512],
                                    start=(k2 == 0),
                                    stop=(k2 == KC // 2 - 1),
                                    perf_mode=MPM.DoubleRow,
                                )
                    osb = outp.tile(
                        [128, MG, rt.shape[2]], dt.float8e4, name=f"{x_nm}_osb", tag="osb"
                    )
                    for m in ms:
                        for n in range(n_n):
                            copy_out(osb[:, m - mg, n * 512 : (n + 1) * 512], pss[m, n][:], scale)
                    nc.sync.dma_start(
                        out_dram[mg * 128 : (mg + MG) * 128, :].rearrange(
                            "(g p) c -> p g c", p=128
                        ),
                        osb[:],
                    )

            for _ in range(reps):
                proj("xqT", "wqT", qT_o, True, 4.0 / 64.0)
                proj("xkT", "wkT", kT_o, True, 2.0 / 64.0)
                proj("xvT", "wvT", v_o, False, 1.0 / 64.0)

    nc.compile()
    return nc


# --------------------------------------------------------------------------
# Phase 2: attention (head-parallel, 2 heads/core).
#   inputs (per core):
#     qT  [128, B*SQ] fp8  (rows = 2 heads x 64 dims, = 4*q^T)
#     kT  [128, TNV] fp8   (compacted, = 2*k^T)
#     va  [TNV, HPC*(HD+1)] fp8 (v*mask | mask column per head)
#     va16 same as va in bf16 (for the Schraudolph bf16 AV matmuls)
#     eb  [HPC, TNV, SQ] fp8 (64 * bias^T per head, compacted rows)
#     i2  [128, 256] fp8   (DoubleRow identity: [:, :128]=I, [:, 128:]=0)
#   outputs: ctx_o [128, B*SQ] fp8 = 32*ctx/norm in [p, t, d] layout
# --------------------------------------------------------------------------
def build_phase2(nvts=(8, 8, 8, 8), reps=1, sp_bufs=6, cp_bufs=2, wm_bufs=4):
    nc = bacc.Bacc("TRN2", debug=False, num_devices=NCORES)
    QC = 512
    NQC = SQ // QC
    snvt = [0]
    for t in nvts:
        snvt.append(snvt[-1] + t)
    TNT = snvt[-1]
    TNV = TNT * 128
    NTMAX = max(nvts)

    qT = nc.dram_tensor("qT", [128, B * SQ], dt.float8e4, kind="ExternalInput").ap()
    kT = nc.dram_tensor("kT", [128, TNV], dt.float8e4, kind="ExternalInput").ap()
    va = nc.dram_tensor("va", [TNV, HPC * (HD + 1)], dt.float8e4, kind="ExternalInput").ap()
    va16 = nc.dram_tensor("va16", [TNV, HPC * (HD + 1)], dt.bfloat16, kind="ExternalInput").ap()
    eb = nc.dram_tensor("eb", [HPC, TNV, SQ], dt.float8e4, kind="ExternalInput").ap()
    i2 = nc.dram_tensor("i2", [128, 256], dt.float8e4, kind="ExternalInput").ap()
    ctx_o = nc.dram_tensor("ctx_o", [128, B * SQ], dt.float8e4, kind="ExternalOutput").ap()

    with tile.TileContext(nc) as tc:
        with (
            tc.tile_pool(name="big", bufs=1) as bigp,
            tc.tile_pool(name="ebp", bufs=5) as ebp,
            tc.tile_pool(name="wp", bufs=wm_bufs) as wp,
            tc.tile_pool(name="ip", bufs=wm_bufs) as ip,
            tc.tile_pool(name="np_", bufs=6) as normp,
            tc.tile_pool(name="Sp", bufs=sp_bufs, space="PSUM") as Sp,
            tc.tile_pool(name="cp", bufs=cp_bufs, space="PSUM") as cp,
        ):
            # hd-split layouts for DoubleRow: [32 partitions, 2 slots, head, cols]
            qT_sb = bigp.tile([32, 2, HPC, B * SQ], dt.float8e4)
            kT_sb = bigp.tile([32, 2, HPC, TNV], dt.float8e4)
            va_sb = bigp.tile([128, TNT, HPC * (HD + 1)], dt.float8e4)
            va16_sb = bigp.tile([128, TNT, HPC * (HD + 1)], dt.bfloat16)
            i2_sb = bigp.tile([128, 2, 128], dt.float8e4)
            warm = bigp.tile([1, 1], dt.float32)
            nc.vector.memset(warm[:], 0.0)
            warm2 = bigp.tile([1, 1], dt.float32)
            nc.scalar.activation(warm2[:], warm[:], AF.Exp)

            def load_qk_h(b, h):
                nc.sync.dma_start(
                    qT_sb[:, :, h, b * SQ : (b + 1) * SQ],
                    qT[64 * h : 64 * h + 64, b * SQ : (b + 1) * SQ].rearrange(
                        "(s p) c -> p s c", p=32
                    ),
                )
                cs, ce = snvt[b] * 128, snvt[b + 1] * 128
                nc.sync.dma_start(
                    kT_sb[:, :, h, cs:ce],
                    kT[64 * h : 64 * h + 64, cs:ce].rearrange("(s p) c -> p s c", p=32),
                )

            def load_qk(b):
                for h in range(HPC):
                    load_qk_h(b, h)

            def load_va(b):
                cs, ce = snvt[b] * 128, snvt[b + 1] * 128
                nc.sync.dma_start(
                    va_sb[:, snvt[b] : snvt[b + 1], :],
                    va[cs:ce, :].rearrange("(t p) d -> p t d", p=128),
                )
                nc.sync.dma_start(
                    va16_sb[:, snvt[b] : snvt[b + 1], :],
                    va16[cs:ce, :].rearrange("(t p) d -> p t d", p=128),
                )

            def load_b(b):
                load_qk(b)
                load_va(b)

            load_qk(0)
            nc.sync.dma_start(i2_sb[:], i2[:])

            iters = [(qc, b) for qc in range(NQC) for b in range(B)] * reps

            def load_slab(qc, b, split=False, kj_range=None):
                NT = nvts[b]
                eb_sb = ebp.tile(
                    [128, NTMAX + 1, HPC, QC], dt.float8e4, name="eb_sb", tag="eb"
                )
                src_r = eb[:, snvt[b] * 128 : snvt[b + 1] * 128, :].rearrange(
                    "h (t p) q -> h p t q", p=128
                )[:, :, :, qc * QC : (qc + 1) * QC]

                def emit(kjs):
                    for kj in kjs:
                        for h in range(HPC):
                            nc.sync.dma_start(eb_sb[:, kj, h, :], src_r[h, :, kj, :])

                if split:
                    emit(range(NT) if kj_range is None else kj_range)
                else:
                    for h in range(HPC):
                        nc.sync.dma_start(eb_sb[:, 0:NT, h, :], src_r[h])
                # pad tile (read by the DR inject's zero slot on the last key
                # tile) must be initialized for the race detector
                if kj_range is None or list(kj_range)[-1] == NT - 1:
                    nc.gpsimd.memset(eb_sb[:, NT, :, :], 0.0)
                return eb_sb, emit

            slabs = {}
            # first two key tiles of iteration 0 land before the va bulk loads
            eb0, emit0 = load_slab(*iters[0], split=True, kj_range=range(2))
            load_va(0)
            emit0(range(2, nvts[iters[0][1]]))
            nc.gpsimd.memset(eb0[:, nvts[iters[0][1]], :, :], 0.0)
            slabs[0] = eb0
            for b in range(1, B):
                load_b(b)
                slabs[b], _ = load_slab(*iters[b], split=(b == 1))

            def emit_norm_piece(state):
                # mask column is 1/32, so 1/normcol = 32/sum(w): the x32 ctx
                # scaling is free. Pieces alternate DVE / Act to balance load.
                ctx, col0, holder = state
                if holder[0] is None:
                    holder[0] = normp.tile(
                        [128, QC // 128, HPC * HD], dt.float8e4, name="ctxn", tag="ctxn"
                    )
                ctxn = holder[0]
                t = holder[1]
                holder[1] += 1
                ti, tt = t // 2, t % 2
                if tt == 0:
                    # one strided reciprocal covers all 4 norm scalars of this
                    # ctx tile (2 tt x 2 heads) instead of 4 tiny ops
                    rec4 = normp.tile([128, 2, HPC], dt.float32, name="rec4", tag=f"rec{ti}")
                    nc.vector.reciprocal(
                        rec4[:], ctx[ti][:, :, HD :: HD + 1]
                    )
                    holder[2 + ti] = rec4
                rec4 = holder[2 + ti]
                for h in range(HPC):
                    if t != 3:
                        nc.vector.tensor_scalar(
                            out=ctxn[:, t, h * HD : (h + 1) * HD],
                            in0=ctx[ti][:, tt, h * (HD + 1) : h * (HD + 1) + HD],
                            scalar1=rec4[:, tt, h : h + 1],
                            scalar2=None,
                            op0=ALU.mult,
                        )
                    else:
                        nc.scalar.activation(
                            ctxn[:, t, h * HD : (h + 1) * HD],
                            ctx[ti][:, tt, h * (HD + 1) : h * (HD + 1) + HD],
                            AF.Copy,
                            scale=rec4[:, tt, h : h + 1],
                        )
                if t == QC // 128 - 1:
                    nc.sync.dma_start(ctx_o[:, col0 : col0 + QC], ctxn[:])

            def emit_norm(state):
                while state[2][1] < QC // 128:
                    emit_norm_piece(state)

            def emit_av_pair(ctx, tbase, pj, wm2, start, stop):
                # DoubleRow fp8 AV over a kj pair
                for ti in range(QC // 256):
                    for tt in range(2):
                        for h in range(HPC):
                            t = ti * 2 + tt
                            nc.tensor.matmul(
                                ctx[ti][:, tt, h * (HD + 1) : (h + 1) * (HD + 1)],
                                lhsT=wm2[:, :, h, t * 128 : (t + 1) * 128],
                                rhs=va_sb[:, tbase + 2 * pj : tbase + 2 * pj + 2,
                                          h * (HD + 1) : (h + 1) * (HD + 1)],
                                start=start and (tt == 0) and (h == 0),
                                stop=stop and (ti == QC // 256 - 1) and (tt == 1) and (h == HPC - 1),
                                perf_mode=MPM.DoubleRow,
                                skip_group_check=True,
                            )

            def emit_av_sch(ctx, tbase, kj, i16, sl, start, stop):
                # plain bf16 AV for one Schraudolph kj tile (bitcast int16 weights)
                for ti in range(QC // 256):
                    for tt in range(2):
                        for h in range(HPC):
                            t = ti * 2 + tt
                            nc.tensor.matmul(
                                ctx[ti][:, tt, h * (HD + 1) : (h + 1) * (HD + 1)],
                                lhsT=i16[:, sl, h, t * 128 : (t + 1) * 128].bitcast(dt.bfloat16),
                                rhs=va16_sb[:, tbase + kj, h * (HD + 1) : (h + 1) * (HD + 1)],
                                start=start and (tt == 0) and (h == 0),
                                stop=stop and (ti == QC // 256 - 1) and (tt == 1) and (h == HPC - 1),
                                skip_group_check=True,
                            )

            def emit_av_one(ctx, tbase, kj, wm1, start, stop):
                # plain fp8 AV for the odd tail tile
                for ti in range(QC // 256):
                    for tt in range(2):
                        for h in range(HPC):
                            t = ti * 2 + tt
                            nc.tensor.matmul(
                                ctx[ti][:, tt, h * (HD + 1) : (h + 1) * (HD + 1)],
                                lhsT=wm1[:, 0, h, t * 128 : (t + 1) * 128],
                                rhs=va_sb[:, tbase + kj, h * (HD + 1) : (h + 1) * (HD + 1)],
                                start=start and (tt == 0) and (h == 0),
                                stop=stop and (ti == QC // 256 - 1) and (tt == 1) and (h == HPC - 1),
                                skip_group_check=True,
                            )

            tail_av = []     # AV thunks deferred from the previous kj
            tail_norm = None

            for it_i, (qc, b) in enumerate(iters):
                NT = nvts[b]
                NP = NT // 2
                eb_sb = slabs.pop(it_i)
                if it_i + 4 < len(iters):
                    slabs[it_i + 4], _ = load_slab(*iters[it_i + 4])
                ctx = [
                    cp.tile([128, 2, HPC * (HD + 1)], dt.float32, name=f"ctx{t}", tag="ctx")
                    for t in range(QC // 256)
                ]
                col0 = b * SQ + qc * QC
                tbase = snvt[b]

                def make_S(kj):
                    # per-head 1-bank S tiles -> deeper PSUM pipeline
                    Ss = []
                    kcol = tbase * 128 + kj * 128
                    for h in range(HPC):
                        S = Sp.tile([128, QC], dt.float32, name="S", tag="S")
                        nc.tensor.matmul(
                            S[:],
                            lhsT=kT_sb[:, :, h, kcol : kcol + 128],
                            rhs=qT_sb[:, :, h, col0 : col0 + QC],
                            start=True,
                            stop=False,
                            perf_mode=MPM.DoubleRow,
                            skip_group_check=True,
                        )
                        nc.tensor.matmul(
                            S[:],
                            lhsT=i2_sb[:],
                            rhs=eb_sb[:, kj : kj + 2, h, :],
                            start=False,
                            stop=True,
                            perf_mode=MPM.DoubleRow,
                            skip_group_check=True,
                        )
                        Ss.append(S)
                    return Ss

                first_av = [True]
                wm2_cur = [None]
                i16_cur = [None]
                for kj in range(NT):
                    S = make_S(kj)
                    # drain the deferred AVs / previous iteration's norm
                    if tail_av:
                        fin = (kj == 0)
                        for j, (fn, args) in enumerate(tail_av):
                            fn(*args, stop=(fin and j == len(tail_av) - 1) if fin else False)
                        tail_av = []
                    if tail_norm is not None and kj >= 1:
                        emit_norm_piece(tail_norm)
                        if tail_norm[2][1] >= QC // 128:
                            tail_norm = None
                    pj = kj // 2
                    is_odd_tail = (kj == NT - 1) and (NT % 2 == 1)
                    path = "act" if (is_odd_tail or pj % 2 == 0) else "sch"
                    if path == "act":
                        if is_odd_tail:
                            wm1 = wp.tile([128, 1, HPC, QC], dt.float8e4, name="wm1", tag="wm1")
                            for h in range(HPC):
                                nc.scalar.activation(
                                    wm1[:, 0, h], S[h][:], AF.Exp, scale=1.0 / 64.0
                                )
                            tail_av.append((emit_av_one, [ctx, tbase, kj, wm1, first_av[0]]))
                            first_av[0] = False
                        else:
                            if kj % 2 == 0:
                                wm2_cur[0] = wp.tile(
                                    [128, 2, HPC, QC], dt.float8e4, name="wm2", tag="wm2"
                                )
                            for h in range(HPC):
                                nc.scalar.activation(
                                    wm2_cur[0][:, kj % 2, h], S[h][:], AF.Exp, scale=1.0 / 64.0
                                )
                            if kj % 2 == 1:
                                tail_av.append(
                                    (emit_av_pair, [ctx, tbase, pj, wm2_cur[0], first_av[0]])
                                )
                                first_av[0] = False
                    else:
                        if kj % 2 == 0 or is_odd_tail:
                            i16_cur[0] = ip.tile(
                                [128, 2, HPC, QC], dt.int16, name="i16", tag="i16"
                            )
                        sl_ = 0 if is_odd_tail else kj % 2
                        for h in range(HPC):
                            nc.vector.tensor_scalar(
                                out=i16_cur[0][:, sl_, h], in0=S[h][:], scalar1=SCH_A / 64.0,
                                scalar2=SCH_B, op0=ALU.mult, op1=ALU.add,
                            )
                        tail_av.append(
                            (emit_av_sch, [ctx, tbase, kj, i16_cur[0], sl_, first_av[0]])
                        )
                        first_av[0] = False

                if tail_norm is not None:
                    emit_norm(tail_norm)   # short iterations: flush leftovers
                tail_norm = (ctx, col0, [None, 0, None, None])

            for j, (fn, args) in enumerate(tail_av):
                fn(*args, stop=(j == len(tail_av) - 1))
            emit_norm(tail_norm)

    nc.compile()
    return nc


# --------------------------------------------------------------------------
# Phase 3: out projection + residual + LayerNorm (row-parallel, fp8 DR GEMM).
#   inputs (per core): ctxT [D, RPC] fp8 (=32*ctx^T), woT [D, D] fp8 (=64*Wo^T),
#     resid [RPC, D] bf16 (query rows + bo), [gammab/betab [128, D] f32 if
#     not trivial_ln]
#   outputs: out_o [RPC, D] f32
# --------------------------------------------------------------------------
def build_phase3(trivial_ln=True, reps=1):
    nc = bacc.Bacc("TRN2", debug=False, num_devices=NCORES)
    KC = D // 128

    ctxT = nc.dram_tensor("ctxT", [D, RPC], dt.float8e4, kind="ExternalInput").ap()
    woT = nc.dram_tensor("woT", [D, D], dt.float8e4, kind="ExternalInput").ap()
    resid = nc.dram_tensor("resid", [RPC, D], dt.bfloat16, kind="ExternalInput").ap()
    if not trivial_ln:
        gammab = nc.dram_tensor("gammab", [128, D], dt.float32, kind="ExternalInput").ap()
        betab = nc.dram_tensor("betab", [128, D], dt.float32, kind="ExternalInput").ap()
    out_o = nc.dram_tensor("out_o", [RPC, D], dt.float32, kind="ExternalOutput").ap()
    PS_SCALE = 1.0 / (32.0 * 64.0)

    with tile.TileContext(nc) as tc:
        with (
            tc.tile_pool(name="big", bufs=1) as bigp,
            tc.tile_pool(name="rp", bufs=4) as rp,
            tc.tile_pool(name="wk", bufs=3) as wk,
            tc.tile_pool(name="ps", bufs=6, space="PSUM") as psp,
        ):
            ctx_sb = bigp.tile([128, KC, RPC], dt.float8e4)
            wo_sb = bigp.tile([128, KC, D], dt.float8e4)
            nc.sync.dma_start(
                ctx_sb[:], ctxT[:, :].rearrange("(k p) c -> p k c", p=128)
            )
            nc.sync.dma_start(
                wo_sb[:], woT[:, :].rearrange("(k p) c -> p k c", p=128)
            )
            eps_sb = bigp.tile([128, 1], dt.float32)
            nc.vector.memset(eps_sb[:], LN_EPS)
            warm = bigp.tile([1, 1], dt.float32)
            nc.vector.memset(warm[:], 1.0)
            warm2 = bigp.tile([1, 1], dt.float32)
            nc.scalar.activation(warm2[:], warm[:], AF.Sqrt)
            warm3 = bigp.tile([1, 1], dt.float32)
            nc.scalar.activation(warm3[:], warm[:], AF.Square)
            if not trivial_ln:
                gam_sb = bigp.tile([128, D], dt.float32)
                nc.sync.dma_start(gam_sb[:], gammab[:])
                bet_sb = bigp.tile([128, D], dt.float32)
                nc.sync.dma_start(bet_sb[:], betab[:])

            for m in [m for _ in range(reps) for m in range(RPC // 128)]:
                res_sb = rp.tile([128, D], dt.bfloat16, name="res_sb", tag="res")
                nc.sync.dma_start(res_sb[:], resid[m * 128 : (m + 1) * 128, :])
                ps = [psp.tile([128, 512], dt.float32, name=f"ps{n}", tag="ps") for n in range(2)]
                for n in range(2):
                    for k2 in range(KC // 2):
                        nc.tensor.matmul(
                            ps[n][:],
                            lhsT=ctx_sb[:, 2 * k2 : 2 * k2 + 2, m * 128 : (m + 1) * 128],
                            rhs=wo_sb[:, 2 * k2 : 2 * k2 + 2, n * 512 : (n + 1) * 512],
                            start=(k2 == 0),
                            stop=(k2 == KC // 2 - 1),
                            perf_mode=MPM.DoubleRow,
                        )
                x_sb = wk.tile([128, D], dt.float32, name="x_sb", tag="x")
                acc = [wk.tile([128, 1], dt.float32, name=f"acc{n}", tag=f"acc{n}") for n in range(2)]
                for n in range(2):
                    nc.vector.scalar_tensor_tensor(
                        out=x_sb[:, n * 512 : (n + 1) * 512],
                        in0=ps[n][:],
                        scalar=PS_SCALE,
                        in1=res_sb[:, n * 512 : (n + 1) * 512],
                        op0=ALU.mult,
                        op1=ALU.add,
                        accum_out=acc[n][:],
                    )
                mu = wk.tile([128, 1], dt.float32, name="mu", tag="mu")
                nc.vector.tensor_scalar(
                    out=mu[:], in0=acc[0][:], scalar1=acc[1][:], scalar2=1.0 / D,
                    op0=ALU.add, op1=ALU.mult,
                )
                sq = wk.tile([128, D], dt.bfloat16, name="sq", tag="sq")
                s2 = wk.tile([128, 1], dt.float32, name="s2", tag="s2")
                nc.scalar.activation(sq[:], x_sb[:], AF.Square, accum_out=s2[:])
                var = wk.tile([128, 1], dt.float32, name="var", tag="var")
                # var = s2/D - mu^2  (one fused op: (s2*(1/D)) - mu2)
                mu2 = wk.tile([128, 1], dt.float32, name="mu2", tag="mu2")
                nc.vector.tensor_tensor(mu2[:], mu[:], mu[:], op=ALU.mult)
                nc.vector.tensor_scalar(
                    out=var[:], in0=s2[:], scalar1=1.0 / D, scalar2=mu2[:],
                    op0=ALU.mult, op1=ALU.subtract,
                )
                std = wk.tile([128, 1], dt.float32, name="std", tag="std")
                nc.scalar.activation(std[:], var[:], AF.Sqrt, bias=eps_sb[:])
                rstd = wk.tile([128, 1], dt.float32, name="rstd", tag="rstd")
                nc.vector.reciprocal(rstd[:], std[:])
                mrs = wk.tile([128, 1], dt.float32, name="mrs", tag="mrs")
                nc.vector.tensor_tensor(mrs[:], mu[:], rstd[:], op=ALU.mult)
                out_sb = wk.tile([128, D], dt.float32, name="out_sb", tag="out_sb")
                if trivial_ln:
                    nc.vector.tensor_scalar(
                        out=out_sb[:], in0=x_sb[:], scalar1=rstd[:], scalar2=mrs[:],
                        op0=ALU.mult, op1=ALU.subtract,
                    )
                else:
                    tmp = wk.tile([128, D], dt.float32, name="tmp", tag="tmp")
                    nc.vector.tensor_scalar(
                        out=tmp[:], in0=x_sb[:], scalar1=rstd[:], scalar2=mrs[:],
                        op0=ALU.mult, op1=ALU.subtract,
                    )
                    y = wk.tile([128, D], dt.float32, name="y", tag="y")
                    nc.vector.scalar_tensor_tensor(
                        out=y[:], in0=tmp[:], scalar=0.0, in1=gam_sb[:],
                        op0=ALU.add, op1=ALU.mult,
                    )
                    nc.gpsimd.tensor_add(out_sb[:], y[:], bet_sb[:])
                nc.sync.dma_start(out_o[m * 128 : (m + 1) * 128, :], out_sb[:])

    nc.compile()
    return nc


def _get_program(key, builder, *args, **kwargs):
    if key not in _programs:
        _programs[key] = builder(*args, **kwargs)
    return _programs[key]


def _run(nc, in_maps):
    return bass_utils.run_bass_kernel_spmd(nc, in_maps, core_ids=list(range(NCORES)))


def kernel(query, key, value, attention_mask, relative_position_bias,
           Wq, bq, Wk, bk, Wv, bv, Wo, bo, ln_gamma, ln_beta,
           _collect_results=None):
    query = np.asarray(query, dtype=np.float32)
    key = np.asarray(key, dtype=np.float32)
    value = np.asarray(value, dtype=np.float32)
    attention_mask = np.asarray(attention_mask)
    relative_position_bias = np.asarray(relative_position_bias, dtype=np.float32)

    def xT8(x):
        return np.ascontiguousarray(x.reshape(-1, D).T).astype(F8)

    def wT8(W, scale):
        return (np.ascontiguousarray(np.asarray(W, np.float32).T) * scale).astype(F8)

    xqT = xT8(query)
    xkT = xT8(key)
    xvT = xT8(value)
    wqT = wT8(Wq, 64.0)
    wkT = wT8(Wk, 64.0)
    wvT = wT8(Wv, 64.0)

    # ---------------- phase 1 ----------------
    in1 = []
    for c in range(NCORES):
        sl = slice(c * RPC, (c + 1) * RPC)
        in1.append({
            "xqT": np.ascontiguousarray(xqT[:, sl]),
            "xkT": np.ascontiguousarray(xkT[:, sl]),
            "xvT": np.ascontiguousarray(xvT[:, sl]),
            "wqT": wqT, "wkT": wkT, "wvT": wvT,
        })
    r1 = _run(_get_program("p1", build_phase1), in1)

    qT_full = np.empty((D, B * SQ), dtype=F8)
    kT_full = np.empty((D, B * SK), dtype=F8)
    v_full = np.empty((B * SK, D), dtype=F8)
    for c in range(NCORES):
        sl = slice(c * RPC, (c + 1) * RPC)
        qT_full[:, sl] = r1.results[c]["qT_o"]
        kT_full[:, sl] = r1.results[c]["kT_o"]
        v_full[sl, :] = r1.results[c]["v_o"]

    # fold any nonzero projection biases in on the host (zero in practice)
    if np.any(np.asarray(bq)):
        qT_full = (qT_full.astype(np.float32)
                   + 4.0 * np.asarray(bq, np.float32)[:, None]).astype(F8)
    if np.any(np.asarray(bk)):
        kT_full = (kT_full.astype(np.float32)
                   + 2.0 * np.asarray(bk, np.float32)[:, None]).astype(F8)
    if np.any(np.asarray(bv)):
        v_full = (v_full.astype(np.float32)
                  + np.asarray(bv, np.float32)[None, :]).astype(F8)

    # ---------------- phase 2 ----------------
    mask2 = (attention_mask.reshape(B, SK) != 0)
    valid = [np.nonzero(mask2[b])[0] for b in range(B)]
    nvts = tuple(max(1, -(-len(ix) // 128)) for ix in valid)
    snvt = np.concatenate([[0], np.cumsum(nvts)]).astype(int)
    TNT = int(snvt[-1])
    idx_pad = np.zeros(TNT * 128, dtype=np.int64)
    maskc = np.zeros((TNT * 128,), dtype=bool)
    for b in range(B):
        ix = valid[b]
        o = snvt[b] * 128
        idx_pad[o : o + len(ix)] = ix
        maskc[o : o + len(ix)] = True

    col_idx = (np.repeat(np.arange(B) * SK, np.array(nvts) * 128) + idx_pad)
    kT_c = np.ascontiguousarray(kT_full[:, col_idx])
    v_rows = v_full[col_idx, :]
    va_all = np.zeros((TNT * 128, H * (HD + 1)), dtype=F8)
    inv32 = np.asarray(1.0 / 32.0, dtype=F8)[()]
    for h in range(H):
        blk = np.where(maskc[:, None], v_rows[:, h * HD : (h + 1) * HD], np.zeros((), F8))
        va_all[:, h * (HD + 1) : h * (HD + 1) + HD] = blk
        va_all[:, h * (HD + 1) + HD] = np.where(maskc, inv32, np.zeros((), F8))

    ebT8 = (np.ascontiguousarray(
        relative_position_bias[0].transpose(0, 2, 1)) * 64.0).astype(F8)
    eb_c = ebT8[:, idx_pad, :]  # [H, TNV, SQ] fp8

    i2_host = np.zeros((128, 256), dtype=F8)
    i2_host[:, 0:128] = np.eye(128, dtype=np.float32).astype(F8)

    in2 = []
    for c in range(NCORES):
        rs = slice(c * 128, (c + 1) * 128)
        in2.append({
            "qT": np.ascontiguousarray(qT_full[rs, :]),
            "kT": np.ascontiguousarray(kT_c[rs, :]),
            "va": np.ascontiguousarray(
                va_all[:, c * HPC * (HD + 1) : (c + 1) * HPC * (HD + 1)]
            ),
            "va16": np.ascontiguousarray(
                va_all[:, c * HPC * (HD + 1) : (c + 1) * HPC * (HD + 1)]
            ).astype(BF16),
            "eb": np.ascontiguousarray(eb_c[c * HPC : (c + 1) * HPC]),
            "i2": i2_host,
        })
    r2 = _run(_get_program(("p2",) + nvts, build_phase2, nvts), in2)

    # ctx_o[c] is [128 q-part, t, 128 d] for d-block c -> assemble ctxT [D, B*SQ]
    ctxT_full = np.empty((D, B * SQ), dtype=F8)
    for c in range(NCORES):
        blk = r2.results[c]["ctx_o"].reshape(128, B * SQ // 128, 128)
        ctxT_full[c * 128 : (c + 1) * 128, :] = (
            blk.transpose(2, 1, 0).reshape(128, B * SQ)
        )

    # ---------------- phase 3 ----------------
    woT8 = wT8(Wo, 64.0)
    q2d = query.reshape(-1, D)
    resid_h = (q2d + np.asarray(bo, np.float32)[None, :]).astype(BF16)
    trivial = (not np.any(np.asarray(ln_beta))) and np.all(
        np.asarray(ln_gamma, np.float32) == 1.0
    )
    in3 = []
    for c in range(NCORES):
        sl = slice(c * RPC, (c + 1) * RPC)
        d = {
            "ctxT": np.ascontiguousarray(ctxT_full[:, sl]),
            "woT": woT8,
            "resid": np.ascontiguousarray(resid_h[sl, :]),
        }
        if not trivial:
            d["gammab"] = np.ascontiguousarray(
                np.broadcast_to(np.asarray(ln_gamma, np.float32)[None, :], (128, D))
            )
            d["betab"] = np.ascontiguousarray(
                np.broadcast_to(np.asarray(ln_beta, np.float32)[None, :], (128, D))
            )
        in3.append(d)
    r3 = _run(_get_program(("p3", trivial), build_phase3, trivial), in3)

    out = np.empty((B * SQ, D), dtype=np.float32)
    for c in range(NCORES):
        out[c * RPC : (c + 1) * RPC, :] = r3.results[c]["out_o"]

    if _collect_results is not None:
        _collect_results.extend([r1, r2, r3])
    return out.reshape(B, SQ, D)


# revision 30
# speedup vs baseline: 1.0555x; 1.0082x over previous
"""MultiHeadCrossAttention Trainium2 kernel (8 NeuronCores, SPMD).

Problem: B=4, SQ=SK=2048, D=1024, H=16 (HD=64), f32 in/out.

Distribution (3 SPMD launches):
  Phase 1 (row-parallel): QKV projections in fp8 e4m3 with DoubleRow matmuls
    (2 contraction rows per partition -> 0.5 cyc/row). Weights host-prescaled
    by 64; outputs written as scaled fp8 (q*4, k*2, v*1).
  Phase 2 (head-parallel): attention, 2 heads/core. Keys mask-compacted on
    host. Scores S = (4q).(2k) accumulate in PSUM via fp8-DR matmuls over the
    hd=64 contraction split as [32 partitions x 2 slots]; 64*bias (fp8) is
    injected into the same PSUM via a DoubleRow identity matmul, so
    exp(score+bias) needs no elementwise multiply. exp runs split across
    engines: ScalarE true exp (scale=1/64) -> fp8 weights (DR AV matmul), and
    Schraudolph bit-trick exp on DVE/Pool (tensor_scalar -> int16, bitcast to
    bf16 -> plain AV matmul). Mask + normalizer ride as an extra fp8 value
    column; normalization multiplies by 32/norm -> fp8 ctx output.
  Phase 3 (row-parallel): out projection (fp8 DR, ctxT*32 @ woT*64, /2048
    folded into the residual add), one-pass mean/var LayerNorm.
"""

import sys

sys.path.insert(0, "/opt/trn_rl_repo")

import numpy as np
import ml_dtypes

import concourse.bass as bass
import concourse.tile as tile
from concourse import bacc, mybir
from concourse import bass_utils

BF16 = ml_dtypes.bfloat16

B, SQ, SK, D, H = 4, 2048, 2048, 1024, 16
HD = D // H  # 64
NCORES = 8
HPC = H // NCORES          # heads per core = 2
RPC = B * SQ // NCORES     # rows per core (phases 1/3) = 1024
LN_EPS = 1e-5

dt = mybir.dt
AF = mybir.ActivationFunctionType
ALU = mybir.AluOpType
MPM = mybir.MatmulPerfMode

F8 = np.dtype(mybir.dt.np(dt.float8e4))
F32 = np.float32

# Schraudolph fast-exp in bf16 bit space: bf16_bits(exp(x)) ~= x*128*log2e + B
SCH_A = 128.0 * 1.4426950408889634
SCH_B = 127.0 * 128.0 - 0.0436 * 128.0

_programs = {}


# --------------------------------------------------------------------------
# Phase 1: QKV projection (row-parallel, fp8 DoubleRow, no bias on device —
# host folds biases into the outputs if nonzero).
#   inputs (per core): xqT/xkT/xvT [D, RPC] fp8 (input^T), wqT/wkT/wvT
#                      [D, D] fp8 (W^T * 64)
#   outputs: qT_o/kT_o [D, RPC] fp8 (4*q^T, 2*k^T), v_o [RPC, D] fp8 (v)
# --------------------------------------------------------------------------
def build_phase1(reps=1):
    nc = bacc.Bacc("TRN2", debug=False, num_devices=NCORES)
    KC = D // 128  # 8 chunks of 128 = 4 double-chunks

    ins = {}
    for nm in ("xqT", "xkT", "xvT"):
        ins[nm] = nc.dram_tensor(nm, [D, RPC], dt.float8e4, kind="ExternalInput").ap()
    for nm in ("wqT", "wkT", "wvT"):
        ins[nm] = nc.dram_tensor(nm, [D, D], dt.float8e4, kind="ExternalInput").ap()
    qT_o = nc.dram_tensor("qT_o", [D, RPC], dt.float8e4, kind="ExternalOutput").ap()
    kT_o = nc.dram_tensor("kT_o", [D, RPC], dt.float8e4, kind="ExternalOutput").ap()
    v_o = nc.dram_tensor("v_o", [RPC, D], dt.float8e4, kind="ExternalOutput").ap()

    # greedy engine assignment for the 48 PSUM->SBUF scaled copies
    # (GPSIMD/Pool cannot touch PSUM on TRN2, so only Act/DVE)
    eng_cost = {"act": 570.0, "dve": 660.0}
    eng_load = {"act": 0.0, "dve": 0.0}
    copy_plan = []
    for _ in range(3 * (D // 128) * 2):
        e = min(eng_load, key=lambda k: eng_load[k] + eng_cost[k])
        copy_plan.append(e)
        eng_load[e] += eng_cost[e]
    copy_i = [0]

    with tile.TileContext(nc) as tc:
        with (
            tc.tile_pool(name="big", bufs=1) as bigp,
            tc.tile_pool(name="outp", bufs=3) as outp,
            tc.tile_pool(name="ps", bufs=2, space="PSUM") as psp,
        ):
            warm = bigp.tile([1, 1], dt.float32)
            nc.vector.memset(warm[:], 1.0)
            warm2 = bigp.tile([1, 1], dt.float32)
            nc.scalar.activation(warm2[:], warm[:], AF.Copy)
            sb = {}
            for nm in ("xqT", "xkT", "xvT", "wqT", "wkT", "wvT"):
                ncols = ins[nm].shape[1]
                sb[nm] = bigp.tile([128, KC, ncols], dt.float8e4, name=f"{nm}_sb")
            for pair in (("wqT", "xqT"), ("wkT", "xkT"), ("wvT", "xvT")):
                for nm in pair:
                    nc.sync.dma_start(
                        sb[nm][:],
                        ins[nm][:, :].rearrange("(k p) c -> p k c", p=128),
                    )

            def copy_out(dst, src, scale):
                e = copy_plan[copy_i[0] % len(copy_plan)]
                copy_i[0] += 1
                if e == "act":
                    nc.scalar.activation(dst, src, AF.Copy, scale=scale)
                elif e == "dve":
                    nc.vector.tensor_scalar(
                        out=dst, in0=src, scalar1=scale, scalar2=None, op0=ALU.mult
                    )
                else:
                    nc.gpsimd.tensor_scalar(
                        out=dst, in0=src, scalar1=scale, scalar2=None, op0=ALU.mult
                    )

            def proj(x_nm, w_nm, out_dram, transposed_out, scale):
                xt = sb[x_nm]
                wt = sb[w_nm]
                if transposed_out:
                    lt, rt = wt, xt   # out[d_out, rows]
                else:
                    lt, rt = xt, wt   # out[rows, d_out]
                n_m = lt.shape[2] // 128
                n_n = rt.shape[2] // 512
                MG = 2
                for mg in range(0, n_m, MG):
                    ms = range(mg, min(mg + MG, n_m))
                    pss = {}
                    for m in ms:
                        for n in range(n_n):
                            pss[m, n] = psp.tile(
                                [128, 512], dt.float32, name="ps", tag=f"ps{m % MG}_{n}"
                            )
                    for k2 in range(KC // 2):
                        for m in ms:
                            for n in range(n_n):
                                nc.tensor.matmul(
                                    pss[m, n][:],
                                    lhsT=lt[:, 2 * k2 : 2 * k2 + 2, m * 128 : (m + 1) * 128],
                                    rhs=rt[:, 2 * k2 : 2 * k2 + 2, n * 512 : (n + 1) * 512],
                                    start=(k2 == 0),
                                    stop=(k2 == KC // 2 - 1),
                                    perf_mode=MPM.DoubleRow,
                                )
                    osb = outp.tile(
                        [128, MG, rt.shape[2]], dt.float8e4, name=f"{x_nm}_osb", tag="osb"
                    )
                    for m in ms:
                        for n in range(n_n):
                            copy_out(osb[:, m - mg, n * 512 : (n + 1) * 512], pss[m, n][:], scale)
                    nc.sync.dma_start(
                        out_dram[mg * 128 : (mg + MG) * 128, :].rearrange(
                            "(g p) c -> p g c", p=128
                        ),
                        osb[:],
                    )

            for _ in range(reps):
                proj("xqT", "wqT", qT_o, True, 4.0 / 64.0)
                proj("xkT", "wkT", kT_o, True, 2.0 / 64.0)
                proj("xvT", "wvT", v_o, False, 1.0 / 64.0)

    nc.compile()
    return nc


# --------------------------------------------------------------------------
# Phase 2: attention (head-parallel, 2 heads/core).
#   inputs (per core):
#     qT  [128, B*SQ] fp8  (rows = 2 heads x 64 dims, = 4*q^T)
#     kT  [128, TNV] fp8   (compacted, = 2*k^T)
#     va  [TNV, HPC*(HD+1)] fp8 (v*mask | mask column per head)
#     va16 same as va in bf16 (for the Schraudolph bf16 AV matmuls)
#     eb  [HPC, TNV, SQ] fp8 (64 * bias^T per head, compacted rows)
#     i2  [128, 256] fp8   (DoubleRow identity: [:, :128]=I, [:, 128:]=0)
#   outputs: ctx_o [128, B*SQ] fp8 = 32*ctx/norm in [p, t, d] layout
# --------------------------------------------------------------------------
def build_phase2(nvts=(8, 8, 8, 8), reps=1, sp_bufs=6, cp_bufs=2, wm_bufs=4):
    nc = bacc.Bacc("TRN2", debug=False, num_devices=NCORES)
    QC = 512
    NQC = SQ // QC
    snvt = [0]
    for t in nvts:
        snvt.append(snvt[-1] + t)
    TNT = snvt[-1]
    TNV = TNT * 128
    NTMAX = max(nvts)

    qT = nc.dram_tensor("qT", [128, B * SQ], dt.float8e4, kind="ExternalInput").ap()
    kT = nc.dram_tensor("kT", [128, TNV], dt.float8e4, kind="ExternalInput").ap()
    va = nc.dram_tensor("va", [TNV, HPC * (HD + 1)], dt.float8e4, kind="ExternalInput").ap()
    va16 = nc.dram_tensor("va16", [TNV, HPC * (HD + 1)], dt.bfloat16, kind="ExternalInput").ap()
    eb = nc.dram_tensor("eb", [HPC, TNV, SQ], dt.float8e4, kind="ExternalInput").ap()
    i2 = nc.dram_tensor("i2", [128, 256], dt.float8e4, kind="ExternalInput").ap()
    ctx_o = nc.dram_tensor("ctx_o", [128, B * SQ], dt.float8e4, kind="ExternalOutput").ap()

    with tile.TileContext(nc) as tc:
        with (
            tc.tile_pool(name="big", bufs=1) as bigp,
            tc.tile_pool(name="ebp", bufs=5) as ebp,
            tc.tile_pool(name="wp", bufs=wm_bufs) as wp,
            tc.tile_pool(name="ip", bufs=wm_bufs) as ip,
            tc.tile_pool(name="np_", bufs=6) as normp,
            tc.tile_pool(name="Sp", bufs=sp_bufs, space="PSUM") as Sp,
            tc.tile_pool(name="cp", bufs=cp_bufs, space="PSUM") as cp,
        ):
            # hd-split layouts for DoubleRow: [32 partitions, 2 slots, head, cols]
            qT_sb = bigp.tile([32, 2, HPC, B * SQ], dt.float8e4)
            kT_sb = bigp.tile([32, 2, HPC, TNV], dt.float8e4)
            va_sb = bigp.tile([128, TNT, HPC * (HD + 1)], dt.float8e4)
            va16_sb = bigp.tile([128, TNT, HPC * (HD + 1)], dt.bfloat16)
            i2_sb = bigp.tile([128, 2, 128], dt.float8e4)
            warm = bigp.tile([1, 1], dt.float32)
            nc.vector.memset(warm[:], 0.0)
            warm2 = bigp.tile([1, 1], dt.float32)
            nc.scalar.activation(warm2[:], warm[:], AF.Exp)

            def load_qk_h(b, h):
                nc.sync.dma_start(
                    qT_sb[:, :, h, b * SQ : (b + 1) * SQ],
                    qT[64 * h : 64 * h + 64, b * SQ : (b + 1) * SQ].rearrange(
                        "(s p) c -> p s c", p=32
                    ),
                )
                cs, ce = snvt[b] * 128, snvt[b + 1] * 128
                nc.sync.dma_start(
                    kT_sb[:, :, h, cs:ce],
                    kT[64 * h : 64 * h + 64, cs:ce].rearrange("(s p) c -> p s c", p=32),
                )

            def load_qk(b):
                for h in range(HPC):
                    load_qk_h(b, h)

            def load_va(b):
                cs, ce = snvt[b] * 128, snvt[b + 1] * 128
                nc.sync.dma_start(
                    va_sb[:, snvt[b] : snvt[b + 1], :],
                    va[cs:ce, :].rearrange("(t p) d -> p t d", p=128),
                )
                nc.sync.dma_start(
                    va16_sb[:, snvt[b] : snvt[b + 1], :],
                    va16[cs:ce, :].rearrange("(t p) d -> p t d", p=128),
                )

            def load_b(b):
                load_qk(b)
                load_va(b)

            load_qk(0)
            nc.sync.dma_start(i2_sb[:], i2[:])

            iters = [(qc, b) for qc in range(NQC) for b in range(B)] * reps

            def load_slab(qc, b, split=False, kj_range=None):
                NT = nvts[b]
                eb_sb = ebp.tile(
                    [128, NTMAX + 1, HPC, QC], dt.float8e4, name="eb_sb", tag="eb"
                )
                src_r = eb[:, snvt[b] * 128 : snvt[b + 1] * 128, :].rearrange(
                    "h (t p) q -> h p t q", p=128
                )[:, :, :, qc * QC : (qc + 1) * QC]

                def emit(kjs):
                    for kj in kjs:
                        for h in range(HPC):
                            nc.sync.dma_start(eb_sb[:, kj, h, :], src_r[h, :, kj, :])

                if split:
                    emit(range(NT) if kj_range is None else kj_range)
                else:
                    for h in range(HPC):
                        nc.sync.dma_start(eb_sb[:, 0:NT, h, :], src_r[h])
                # pad tile (read by the DR inject's zero slot on the last key
                # tile) must be initialized for the race detector
                if kj_range is None or list(kj_range)[-1] == NT - 1:
                    nc.gpsimd.memset(eb_sb[:, NT, :, :], 0.0)
                return eb_sb, emit

            slabs = {}
            # first two key tiles of iteration 0 land before the va bulk loads
            eb0, emit0 = load_slab(*iters[0], split=True, kj_range=range(2))
            load_va(0)
            emit0(range(2, nvts[iters[0][1]]))
            nc.gpsimd.memset(eb0[:, nvts[iters[0][1]], :, :], 0.0)
            slabs[0] = eb0
            for b in range(1, B):
                load_b(b)
                slabs[b], _ = load_slab(*iters[b], split=(b == 1))

            def emit_norm_piece(state):
                # mask column is 1/32, so 1/normcol = 32/sum(w): the x32 ctx
                # scaling is free. Pieces alternate DVE / Act to balance load.
                ctx, col0, holder = state
                if holder[0] is None:
                    holder[0] = normp.tile(
                        [128, QC // 128, HPC * HD], dt.float8e4, name="ctxn", tag="ctxn"
                    )
                ctxn = holder[0]
                t = holder[1]
                holder[1] += 1
                ti, tt = t // 2, t % 2
                if tt == 0:
                    # one strided reciprocal covers all 4 norm scalars of this
                    # ctx tile (2 tt x 2 heads) instead of 4 tiny ops
                    rec4 = normp.tile([128, 2, HPC], dt.float32, name="rec4", tag=f"rec{ti}")
                    nc.vector.reciprocal(
                        rec4[:], ctx[ti][:, :, HD :: HD + 1]
                    )
                    holder[2 + ti] = rec4
                rec4 = holder[2 + ti]
                for h in range(HPC):
                    if t < 2 or (t == 2 and h == 0):
                        nc.vector.tensor_scalar(
                            out=ctxn[:, t, h * HD : (h + 1) * HD],
                            in0=ctx[ti][:, tt, h * (HD + 1) : h * (HD + 1) + HD],
                            scalar1=rec4[:, tt, h : h + 1],
                            scalar2=None,
                            op0=ALU.mult,
                        )
                    else:
                        nc.scalar.activation(
                            ctxn[:, t, h * HD : (h + 1) * HD],
                            ctx[ti][:, tt, h * (HD + 1) : h * (HD + 1) + HD],
                            AF.Copy,
                            scale=rec4[:, tt, h : h + 1],
                        )
                if t == QC // 128 - 1:
                    nc.sync.dma_start(ctx_o[:, col0 : col0 + QC], ctxn[:])

            def emit_norm(state):
                while state[2][1] < QC // 128:
                    emit_norm_piece(state)

            def emit_av_pair(ctx, tbase, pj, wm2, start, stop):
                # DoubleRow fp8 AV over a kj pair
                for ti in range(QC // 256):
                    for tt in range(2):
                        for h in range(HPC):
                            t = ti * 2 + tt
                            nc.tensor.matmul(
                                ctx[ti][:, tt, h * (HD + 1) : (h + 1) * (HD + 1)],
                                lhsT=wm2[:, :, h, t * 128 : (t + 1) * 128],
                                rhs=va_sb[:, tbase + 2 * pj : tbase + 2 * pj + 2,
                                          h * (HD + 1) : (h + 1) * (HD + 1)],
                                start=start and (tt == 0) and (h == 0),
                                stop=stop and (ti == QC // 256 - 1) and (tt == 1) and (h == HPC - 1),
                                perf_mode=MPM.DoubleRow,
                                skip_group_check=True,
                            )

            def emit_av_sch(ctx, tbase, kj, i16, sl, start, stop):
                # plain bf16 AV for one Schraudolph kj tile (bitcast int16 weights)
                for ti in range(QC // 256):
                    for tt in range(2):
                        for h in range(HPC):
                            t = ti * 2 + tt
                            nc.tensor.matmul(
                                ctx[ti][:, tt, h * (HD + 1) : (h + 1) * (HD + 1)],
                                lhsT=i16[:, sl, h, t * 128 : (t + 1) * 128].bitcast(dt.bfloat16),
                                rhs=va16_sb[:, tbase + kj, h * (HD + 1) : (h + 1) * (HD + 1)],
                                start=start and (tt == 0) and (h == 0),
                                stop=stop and (ti == QC // 256 - 1) and (tt == 1) and (h == HPC - 1),
                                skip_group_check=True,
                            )

            def emit_av_one(ctx, tbase, kj, wm1, start, stop):
                # plain fp8 AV for the odd tail tile
                for ti in range(QC // 256):
                    for tt in range(2):
                        for h in range(HPC):
                            t = ti * 2 + tt
                            nc.tensor.matmul(
                                ctx[ti][:, tt, h * (HD + 1) : (h + 1) * (HD + 1)],
                                lhsT=wm1[:, 0, h, t * 128 : (t + 1) * 128],
                                rhs=va_sb[:, tbase + kj, h * (HD + 1) : (h + 1) * (HD + 1)],
                                start=start and (tt == 0) and (h == 0),
                                stop=stop and (ti == QC // 256 - 1) and (tt == 1) and (h == HPC - 1),
                                skip_group_check=True,
                            )

            tail_av = []     # AV thunks deferred from the previous kj
            tail_norm = None

            for it_i, (qc, b) in enumerate(iters):
                NT = nvts[b]
                NP = NT // 2
                eb_sb = slabs.pop(it_i)
                if it_i + 4 < len(iters):
                    slabs[it_i + 4], _ = load_slab(*iters[it_i + 4])
                ctx = [
                    cp.tile([128, 2, HPC * (HD + 1)], dt.float32, name=f"ctx{t}", tag="ctx")
                    for t in range(QC // 256)
                ]
                col0 = b * SQ + qc * QC
                tbase = snvt[b]

                def make_S(kj):
                    # per-head 1-bank S tiles -> deeper PSUM pipeline
                    Ss = []
                    kcol = tbase * 128 + kj * 128
                    for h in range(HPC):
                        S = Sp.tile([128, QC], dt.float32, name="S", tag="S")
                        nc.tensor.matmul(
                            S[:],
                            lhsT=kT_sb[:, :, h, kcol : kcol + 128],
                            rhs=qT_sb[:, :, h, col0 : col0 + QC],
                            start=True,
                            stop=False,
                            perf_mode=MPM.DoubleRow,
                            skip_group_check=True,
                        )
                        nc.tensor.matmul(
                            S[:],
                            lhsT=i2_sb[:],
                            rhs=eb_sb[:, kj : kj + 2, h, :],
                            start=False,
                            stop=True,
                            perf_mode=MPM.DoubleRow,
                            skip_group_check=True,
                        )
                        Ss.append(S)
                    return Ss

                first_av = [True]
                wm2_cur = [None]
                i16_cur = [None]
                for kj in range(NT):
                    S = make_S(kj)
                    # drain the deferred AVs / previous iteration's norm
                    if tail_av:
                        fin = (kj == 0)
                        for j, (fn, args) in enumerate(tail_av):
                            fn(*args, stop=(fin and j == len(tail_av) - 1) if fin else False)
                        tail_av = []
                    if tail_norm is not None and kj >= 1:
                        emit_norm_piece(tail_norm)
                        if tail_norm[2][1] >= QC // 128:
                            tail_norm = None
                    pj = kj // 2
                    is_odd_tail = (kj == NT - 1) and (NT % 2 == 1)
                    path = "act" if (is_odd_tail or pj % 2 == 0) else "sch"
                    if path == "act":
                        if is_odd_tail:
                            wm1 = wp.tile([128, 1, HPC, QC], dt.float8e4, name="wm1", tag="wm1")
                            for h in range(HPC):
                                nc.scalar.activation(
                                    wm1[:, 0, h], S[h][:], AF.Exp, scale=1.0 / 64.0
                                )
                            tail_av.append((emit_av_one, [ctx, tbase, kj, wm1, first_av[0]]))
                            first_av[0] = False
                        else:
                            if kj % 2 == 0:
                                wm2_cur[0] = wp.tile(
                                    [128, 2, HPC, QC], dt.float8e4, name="wm2", tag="wm2"
                                )
                            for h in range(HPC):
                                nc.scalar.activation(
                                    wm2_cur[0][:, kj % 2, h], S[h][:], AF.Exp, scale=1.0 / 64.0
                                )
                            if kj % 2 == 1:
                                tail_av.append(
                                    (emit_av_pair, [ctx, tbase, pj, wm2_cur[0], first_av[0]])
                                )
                                first_av[0] = False
                    else:
                        if kj % 2 == 0 or is_odd_tail:
                            i16_cur[0] = ip.tile(
                                [128, 2, HPC, QC], dt.int16, name="i16", tag="i16"
                            )
                        sl_ = 0 if is_odd_tail else kj % 2
                        for h in range(HPC):
                            nc.vector.tensor_scalar(
                                out=i16_cur[0][:, sl_, h], in0=S[h][:], scalar1=SCH_A / 64.0,
                                scalar2=SCH_B, op0=ALU.mult, op1=ALU.add,
                            )
                        tail_av.append(
                            (emit_av_sch, [ctx, tbase, kj, i16_cur[0], sl_, first_av[0]])
                        )
                        first_av[0] = False

                if tail_norm is not None:
                    emit_norm(tail_norm)   # short iterations: flush leftovers
                tail_norm = (ctx, col0, [None, 0, None, None])

            for j, (fn, args) in enumerate(tail_av):
                fn(*args, stop=(j == len(tail_av) - 1))
            emit_norm(tail_norm)

    nc.compile()
    return nc


# --------------------------------------------------------------------------
# Phase 3: out projection + residual + LayerNorm (row-parallel, fp8 DR GEMM).
#   inputs (per core): ctxT [D, RPC] fp8 (=32*ctx^T), woT [D, D] fp8 (=64*Wo^T),
#     resid [RPC, D] bf16 (query rows + bo), [gammab/betab [128, D] f32 if
#     not trivial_ln]
#   outputs: out_o [RPC, D] f32
# --------------------------------------------------------------------------
def build_phase3(trivial_ln=True, reps=1):
    nc = bacc.Bacc("TRN2", debug=False, num_devices=NCORES)
    KC = D // 128

    ctxT = nc.dram_tensor("ctxT", [D, RPC], dt.float8e4, kind="ExternalInput").ap()
    woT = nc.dram_tensor("woT", [D, D], dt.float8e4, kind="ExternalInput").ap()
    resid = nc.dram_tensor("resid", [RPC, D], dt.bfloat16, kind="ExternalInput").ap()
    if not trivial_ln:
        gammab = nc.dram_tensor("gammab", [128, D], dt.float32, kind="ExternalInput").ap()
        betab = nc.dram_tensor("betab", [128, D], dt.float32, kind="ExternalInput").ap()
    out_o = nc.dram_tensor("out_o", [RPC, D], dt.float32, kind="ExternalOutput").ap()
    PS_SCALE = 1.0 / (32.0 * 64.0)

    with tile.TileContext(nc) as tc:
        with (
            tc.tile_pool(name="big", bufs=1) as bigp,
            tc.tile_pool(name="rp", bufs=4) as rp,
            tc.tile_pool(name="wk", bufs=3) as wk,
            tc.tile_pool(name="ps", bufs=6, space="PSUM") as psp,
        ):
            ctx_sb = bigp.tile([128, KC, RPC], dt.float8e4)
            wo_sb = bigp.tile([128, KC, D], dt.float8e4)
            nc.sync.dma_start(
                ctx_sb[:], ctxT[:, :].rearrange("(k p) c -> p k c", p=128)
            )
            nc.sync.dma_start(
                wo_sb[:], woT[:, :].rearrange("(k p) c -> p k c", p=128)
            )
            eps_sb = bigp.tile([128, 1], dt.float32)
            nc.vector.memset(eps_sb[:], LN_EPS)
            warm = bigp.tile([1, 1], dt.float32)
            nc.vector.memset(warm[:], 1.0)
            warm2 = bigp.tile([1, 1], dt.float32)
            nc.scalar.activation(warm2[:], warm[:], AF.Sqrt)
            warm3 = bigp.tile([1, 1], dt.float32)
            nc.scalar.activation(warm3[:], warm[:], AF.Square)
            if not trivial_ln:
                gam_sb = bigp.tile([128, D], dt.float32)
                nc.sync.dma_start(gam_sb[:], gammab[:])
                bet_sb = bigp.tile([128, D], dt.float32)
                nc.sync.dma_start(bet_sb[:], betab[:])

            for m in [m for _ in range(reps) for m in range(RPC // 128)]:
                res_sb = rp.tile([128, D], dt.bfloat16, name="res_sb", tag="res")
                nc.sync.dma_start(res_sb[:], resid[m * 128 : (m + 1) * 128, :])
                ps = [psp.tile([128, 512], dt.float32, name=f"ps{n}", tag="ps") for n in range(2)]
                for n in range(2):
                    for k2 in range(KC // 2):
                        nc.tensor.matmul(
                            ps[n][:],
                            lhsT=ctx_sb[:, 2 * k2 : 2 * k2 + 2, m * 128 : (m + 1) * 128],
                            rhs=wo_sb[:, 2 * k2 : 2 * k2 + 2, n * 512 : (n + 1) * 512],
                            start=(k2 == 0),
                            stop=(k2 == KC // 2 - 1),
                            perf_mode=MPM.DoubleRow,
                        )
                x_sb = wk.tile([128, D], dt.float32, name="x_sb", tag="x")
                acc = [wk.tile([128, 1], dt.float32, name=f"acc{n}", tag=f"acc{n}") for n in range(2)]
                for n in range(2):
                    nc.vector.scalar_tensor_tensor(
                        out=x_sb[:, n * 512 : (n + 1) * 512],
                        in0=ps[n][:],
                        scalar=PS_SCALE,
                        in1=res_sb[:, n * 512 : (n + 1) * 512],
                        op0=ALU.mult,
                        op1=ALU.add,
                        accum_out=acc[n][:],
                    )
                mu = wk.tile([128, 1], dt.float32, name="mu", tag="mu")
                nc.vector.tensor_scalar(
                    out=mu[:], in0=acc[0][:], scalar1=acc[1][:], scalar2=1.0 / D,
                    op0=ALU.add, op1=ALU.mult,
                )
                sq = wk.tile([128, D], dt.bfloat16, name="sq", tag="sq")
                s2 = wk.tile([128, 1], dt.float32, name="s2", tag="s2")
                nc.scalar.activation(sq[:], x_sb[:], AF.Square, accum_out=s2[:])
                var = wk.tile([128, 1], dt.float32, name="var", tag="var")
                # var = s2/D - mu^2  (one fused op: (s2*(1/D)) - mu2)
                mu2 = wk.tile([128, 1], dt.float32, name="mu2", tag="mu2")
                nc.vector.tensor_tensor(mu2[:], mu[:], mu[:], op=ALU.mult)
                nc.vector.tensor_scalar(
                    out=var[:], in0=s2[:], scalar1=1.0 / D, scalar2=mu2[:],
                    op0=ALU.mult, op1=ALU.subtract,
                )
                std = wk.tile([128, 1], dt.float32, name="std", tag="std")
                nc.scalar.activation(std[:], var[:], AF.Sqrt, bias=eps_sb[:])
                rstd = wk.tile([128, 1], dt.float32, name="rstd", tag="rstd")
                nc.vector.reciprocal(rstd[:], std[:])
                mrs = wk.tile([128, 1], dt.float32, name="mrs", tag="mrs")
                nc.vector.tensor_tensor(mrs[:], mu[:], rstd[:], op=ALU.mult)
                out_sb = wk.tile([128, D], dt.float32, name="out_sb", tag="out_sb")
                if trivial_ln:
                    nc.vector.tensor_scalar(
                        out=out_sb[:], in0=x_sb[:], scalar1=rstd[:], scalar2=mrs[:],
                        op0=ALU.mult, op1=ALU.subtract,
                    )
                else:
                    tmp = wk.tile([128, D], dt.float32, name="tmp", tag="tmp")
                    nc.vector.tensor_scalar(
                        out=tmp[:], in0=x_sb[:], scalar1=rstd[:], scalar2=mrs[:],
                        op0=ALU.mult, op1=ALU.subtract,
                    )
                    y = wk.tile([128, D], dt.float32, name="y", tag="y")
                    nc.vector.scalar_tensor_tensor(
                        out=y[:], in0=tmp[:], scalar=0.0, in1=gam_sb[:],
                        op0=ALU.add, op1=ALU.mult,
                    )
                    nc.gpsimd.tensor_add(out_sb[:], y[:], bet_sb[:])
                nc.sync.dma_start(out_o[m * 128 : (m + 1) * 128, :], out_sb[:])

    nc.compile()
    return nc


def _get_program(key, builder, *args, **kwargs):
    if key not in _programs:
        _programs[key] = builder(*args, **kwargs)
    return _programs[key]


def _run(nc, in_maps):
    return bass_utils.run_bass_kernel_spmd(nc, in_maps, core_ids=list(range(NCORES)))


def kernel(query, key, value, attention_mask, relative_position_bias,
           Wq, bq, Wk, bk, Wv, bv, Wo, bo, ln_gamma, ln_beta,
           _collect_results=None):
    query = np.asarray(query, dtype=np.float32)
    key = np.asarray(key, dtype=np.float32)
    value = np.asarray(value, dtype=np.float32)
    attention_mask = np.asarray(attention_mask)
    relative_position_bias = np.asarray(relative_position_bias, dtype=np.float32)

    def xT8(x):
        return np.ascontiguousarray(x.reshape(-1, D).T).astype(F8)

    def wT8(W, scale):
        return (np.ascontiguousarray(np.asarray(W, np.float32).T) * scale).astype(F8)

    xqT = xT8(query)
    xkT = xT8(key)
    xvT = xT8(value)
    wqT = wT8(Wq, 64.0)
    wkT = wT8(Wk, 64.0)
    wvT = wT8(Wv, 64.0)

    # ---------------- phase 1 ----------------
    in1 = []
    for c in range(NCORES):
        sl = slice(c * RPC, (c + 1) * RPC)
        in1.append({
            "xqT": np.ascontiguousarray(xqT[:, sl]),
            "xkT": np.ascontiguousarray(xkT[:, sl]),
            "xvT": np.ascontiguousarray(xvT[:, sl]),
            "wqT": wqT, "wkT": wkT, "wvT": wvT,
        })
    r1 = _run(_get_program("p1", build_phase1), in1)

    qT_full = np.empty((D, B * SQ), dtype=F8)
    kT_full = np.empty((D, B * SK), dtype=F8)
    v_full = np.empty((B * SK, D), dtype=F8)
    for c in range(NCORES):
        sl = slice(c * RPC, (c + 1) * RPC)
        qT_full[:, sl] = r1.results[c]["qT_o"]
        kT_full[:, sl] = r1.results[c]["kT_o"]
        v_full[sl, :] = r1.results[c]["v_o"]

    # fold any nonzero projection biases in on the host (zero in practice)
    if np.any(np.asarray(bq)):
        qT_full = (qT_full.astype(np.float32)
                   + 4.0 * np.asarray(bq, np.float32)[:, None]).astype(F8)
    if np.any(np.asarray(bk)):
        kT_full = (kT_full.astype(np.float32)
                   + 2.0 * np.asarray(bk, np.float32)[:, None]).astype(F8)
    if np.any(np.asarray(bv)):
        v_full = (v_full.astype(np.float32)
                  + np.asarray(bv, np.float32)[None, :]).astype(F8)

    # ---------------- phase 2 ----------------
    mask2 = (attention_mask.reshape(B, SK) != 0)
    valid = [np.nonzero(mask2[b])[0] for b in range(B)]
    nvts = tuple(max(1, -(-len(ix) // 128)) for ix in valid)
    snvt = np.concatenate([[0], np.cumsum(nvts)]).astype(int)
    TNT = int(snvt[-1])
    idx_pad = np.zeros(TNT * 128, dtype=np.int64)
    maskc = np.zeros((TNT * 128,), dtype=bool)
    for b in range(B):
        ix = valid[b]
        o = snvt[b] * 128
        idx_pad[o : o + len(ix)] = ix
        maskc[o : o + len(ix)] = True

    col_idx = (np.repeat(np.arange(B) * SK, np.array(nvts) * 128) + idx_pad)
    kT_c = np.ascontiguousarray(kT_full[:, col_idx])
    v_rows = v_full[col_idx, :]
    va_all = np.zeros((TNT * 128, H * (HD + 1)), dtype=F8)
    inv32 = np.asarray(1.0 / 32.0, dtype=F8)[()]
    for h in range(H):
        blk = np.where(maskc[:, None], v_rows[:, h * HD : (h + 1) * HD], np.zeros((), F8))
        va_all[:, h * (HD + 1) : h * (HD + 1) + HD] = blk
        va_all[:, h * (HD + 1) + HD] = np.where(maskc, inv32, np.zeros((), F8))

    ebT8 = (np.ascontiguousarray(
        relative_position_bias[0].transpose(0, 2, 1)) * 64.0).astype(F8)
    eb_c = ebT8[:, idx_pad, :]  # [H, TNV, SQ] fp8

    i2_host = np.zeros((128, 256), dtype=F8)
    i2_host[:, 0:128] = np.eye(128, dtype=np.float32).astype(F8)

    in2 = []
    for c in range(NCORES):
        rs = slice(c * 128, (c + 1) * 128)
        in2.append({
            "qT": np.ascontiguousarray(qT_full[rs, :]),
            "kT": np.ascontiguousarray(kT_c[rs, :]),
            "va": np.ascontiguousarray(
                va_all[:, c * HPC * (HD + 1) : (c + 1) * HPC * (HD + 1)]
            ),
            "va16": np.ascontiguousarray(
                va_all[:, c * HPC * (HD + 1) : (c + 1) * HPC * (HD + 1)]
            ).astype(BF16),
            "eb": np.ascontiguousarray(eb_c[c * HPC : (c + 1) * HPC]),
            "i2": i2_host,
        })
    r2 = _run(_get_program(("p2",) + nvts, build_phase2, nvts), in2)

    # ctx_o[c] is [128 q-part, t, 128 d] for d-block c -> assemble ctxT [D, B*SQ]
    ctxT_full = np.empty((D, B * SQ), dtype=F8)
    for c in range(NCORES):
        blk = r2.results[c]["ctx_o"].reshape(128, B * SQ // 128, 128)
        ctxT_full[c * 128 : (c + 1) * 128, :] = (
            blk.transpose(2, 1, 0).reshape(128, B * SQ)
        )

    # ---------------- phase 3 ----------------
    woT8 = wT8(Wo, 64.0)
    q2d = query.reshape(-1, D)
    resid_h = (q2d + np.asarray(bo, np.float32)[None, :]).astype(BF16)
    trivial = (not np.any(np.asarray(ln_beta))) and np.all(
        np.asarray(ln_gamma, np.float32) == 1.0
    )
    in3 = []
    for c in range(NCORES):
        sl = slice(c * RPC, (c + 1) * RPC)
        d = {
            "ctxT": np.ascontiguousarray(ctxT_full[:, sl]),
            "woT": woT8,
            "resid": np.ascontiguousarray(resid_h[sl, :]),
        }
        if not trivial:
            d["gammab"] = np.ascontiguousarray(
                np.broadcast_to(np.asarray(ln_gamma, np.float32)[None, :], (128, D))
            )
            d["betab"] = np.ascontiguousarray(
                np.broadcast_to(np.asarray(ln_beta, np.float32)[None, :], (128, D))
            )
        in3.append(d)
    r3 = _run(_get_program(("p3", trivial), build_phase3, trivial), in3)

    out = np.empty((B * SQ, D), dtype=np.float32)
    for c in range(NCORES):
        out[c * RPC : (c + 1) * RPC, :] = r3.results[c]["out_o"]

    if _collect_results is not None:
        _collect_results.extend([r1, r2, r3])
    return out.reshape(B, SQ, D)


# revision 37
# speedup vs baseline: 1.0847x; 1.0276x over previous
"""MultiHeadCrossAttention Trainium2 kernel (8 NeuronCores, SPMD).

Problem: B=4, SQ=SK=2048, D=1024, H=16 (HD=64), f32 in/out.

Distribution (3 SPMD launches):
  Phase 1 (row-parallel): QKV projections in fp8 e4m3 with DoubleRow matmuls
    (2 contraction rows per partition -> 0.5 cyc/row). Weights host-prescaled
    by 64; outputs written as scaled fp8 (q*4, k*2, v*1).
  Phase 2 (head-parallel): attention, 2 heads/core. Keys mask-compacted on
    host. Scores S = (4q).(2k) accumulate in PSUM via fp8-DR matmuls over the
    hd=64 contraction split as [32 partitions x 2 slots]; 64*bias (fp8) is
    injected into the same PSUM via a DoubleRow identity matmul, so
    exp(score+bias) needs no elementwise multiply. exp runs split across
    engines: ScalarE true exp (scale=1/64) -> fp8 weights (DR AV matmul), and
    Schraudolph bit-trick exp on DVE/Pool (tensor_scalar -> int16, bitcast to
    bf16 -> plain AV matmul). Mask + normalizer ride as an extra fp8 value
    column; normalization multiplies by 32/norm -> fp8 ctx output.
  Phase 3 (row-parallel): out projection (fp8 DR, ctxT*32 @ woT*64, /2048
    folded into the residual add), one-pass mean/var LayerNorm.
"""

import sys

sys.path.insert(0, "/opt/trn_rl_repo")

import numpy as np
import ml_dtypes

import concourse.bass as bass
import concourse.tile as tile
from concourse import bacc, mybir
from concourse import bass_utils

BF16 = ml_dtypes.bfloat16

B, SQ, SK, D, H = 4, 2048, 2048, 1024, 16
HD = D // H  # 64
NCORES = 8
HPC = H // NCORES          # heads per core = 2
RPC = B * SQ // NCORES     # rows per core (phases 1/3) = 1024
LN_EPS = 1e-5

dt = mybir.dt
AF = mybir.ActivationFunctionType
ALU = mybir.AluOpType
MPM = mybir.MatmulPerfMode

F8 = np.dtype(mybir.dt.np(dt.float8e4))
F32 = np.float32

# Schraudolph fast-exp in bf16 bit space: bf16_bits(exp(x)) ~= x*128*log2e + B
SCH_A = 128.0 * 1.4426950408889634
SCH_B = 127.0 * 128.0 - 0.0436 * 128.0

_programs = {}


# --------------------------------------------------------------------------
# Phase 1: QKV projection (row-parallel, fp8 DoubleRow, no bias on device —
# host folds biases into the outputs if nonzero).
#   inputs (per core): xqT/xkT/xvT [D, RPC] fp8 (input^T), wqT/wkT/wvT
#                      [D, D] fp8 (W^T * 64)
#   outputs: qT_o/kT_o [D, RPC] fp8 (4*q^T, 2*k^T), v_o [RPC, D] fp8 (v)
# --------------------------------------------------------------------------
def build_phase1(reps=1):
    nc = bacc.Bacc("TRN2", debug=False, num_devices=NCORES)
    KC = D // 128  # 8 chunks of 128 = 4 double-chunks

    ins = {}
    for nm in ("xqT", "xkT", "xvT"):
        ins[nm] = nc.dram_tensor(nm, [D, RPC], dt.float8e4, kind="ExternalInput").ap()
    for nm in ("wqT", "wkT", "wvT"):
        ins[nm] = nc.dram_tensor(nm, [D, D], dt.float8e4, kind="ExternalInput").ap()
    qT_o = nc.dram_tensor("qT_o", [D, RPC], dt.float8e4, kind="ExternalOutput").ap()
    kT_o = nc.dram_tensor("kT_o", [D, RPC], dt.float8e4, kind="ExternalOutput").ap()
    v_o = nc.dram_tensor("v_o", [RPC, D], dt.float8e4, kind="ExternalOutput").ap()

    # greedy engine assignment for the 48 PSUM->SBUF scaled copies
    # (GPSIMD/Pool cannot touch PSUM on TRN2, so only Act/DVE)
    eng_cost = {"act": 570.0, "dve": 660.0}
    eng_load = {"act": 0.0, "dve": 0.0}
    copy_plan = []
    for _ in range(3 * (D // 128) * 2):
        e = min(eng_load, key=lambda k: eng_load[k] + eng_cost[k])
        copy_plan.append(e)
        eng_load[e] += eng_cost[e]
    copy_i = [0]

    with tile.TileContext(nc) as tc:
        with (
            tc.tile_pool(name="big", bufs=1) as bigp,
            tc.tile_pool(name="outp", bufs=3) as outp,
            tc.tile_pool(name="ps", bufs=2, space="PSUM") as psp,
        ):
            warm = bigp.tile([1, 1], dt.float32)
            nc.vector.memset(warm[:], 1.0)
            warm2 = bigp.tile([1, 1], dt.float32)
            nc.scalar.activation(warm2[:], warm[:], AF.Copy)
            sb = {}
            for nm in ("xqT", "xkT", "xvT", "wqT", "wkT", "wvT"):
                ncols = ins[nm].shape[1]
                sb[nm] = bigp.tile([128, KC, ncols], dt.float8e4, name=f"{nm}_sb")
            for pair in (("wqT", "xqT"), ("wkT", "xkT"), ("wvT", "xvT")):
                for nm in pair:
                    nc.sync.dma_start(
                        sb[nm][:],
                        ins[nm][:, :].rearrange("(k p) c -> p k c", p=128),
                    )

            def copy_out(dst, src, scale):
                e = copy_plan[copy_i[0] % len(copy_plan)]
                copy_i[0] += 1
                if e == "act":
                    nc.scalar.activation(dst, src, AF.Copy, scale=scale)
                elif e == "dve":
                    nc.vector.tensor_scalar(
                        out=dst, in0=src, scalar1=scale, scalar2=None, op0=ALU.mult
                    )
                else:
                    nc.gpsimd.tensor_scalar(
                        out=dst, in0=src, scalar1=scale, scalar2=None, op0=ALU.mult
                    )

            def proj(x_nm, w_nm, out_dram, transposed_out, scale):
                xt = sb[x_nm]
                wt = sb[w_nm]
                if transposed_out:
                    lt, rt = wt, xt   # out[d_out, rows]
                else:
                    lt, rt = xt, wt   # out[rows, d_out]
                n_m = lt.shape[2] // 128
                n_n = rt.shape[2] // 512
                MG = 2
                for mg in range(0, n_m, MG):
                    ms = range(mg, min(mg + MG, n_m))
                    pss = {}
                    for m in ms:
                        for n in range(n_n):
                            pss[m, n] = psp.tile(
                                [128, 512], dt.float32, name="ps", tag=f"ps{m % MG}_{n}"
                            )
                    for k2 in range(KC // 2):
                        for m in ms:
                            for n in range(n_n):
                                nc.tensor.matmul(
                                    pss[m, n][:],
                                    lhsT=lt[:, 2 * k2 : 2 * k2 + 2, m * 128 : (m + 1) * 128],
                                    rhs=rt[:, 2 * k2 : 2 * k2 + 2, n * 512 : (n + 1) * 512],
                                    start=(k2 == 0),
                                    stop=(k2 == KC // 2 - 1),
                                    perf_mode=MPM.DoubleRow,
                                )
                    osb = outp.tile(
                        [128, MG, rt.shape[2]], dt.float8e4, name=f"{x_nm}_osb", tag="osb"
                    )
                    for m in ms:
                        for n in range(n_n):
                            copy_out(osb[:, m - mg, n * 512 : (n + 1) * 512], pss[m, n][:], scale)
                    nc.sync.dma_start(
                        out_dram[mg * 128 : (mg + MG) * 128, :].rearrange(
                            "(g p) c -> p g c", p=128
                        ),
                        osb[:],
                    )

            for _ in range(reps):
                proj("xqT", "wqT", qT_o, True, 4.0 / 64.0)
                proj("xkT", "wkT", kT_o, True, 2.0 / 64.0)
                proj("xvT", "wvT", v_o, False, 1.0 / 64.0)

    nc.compile()
    return nc


# --------------------------------------------------------------------------
# Phase 2: attention (head-parallel, 2 heads/core).
#   inputs (per core):
#     qT  [128, B*SQ] fp8  (rows = 2 heads x 64 dims, = 4*q^T)
#     kT  [128, TNV] fp8   (compacted, = 2*k^T)
#     va  [TNV, HPC*(HD+1)] fp8 (v*mask | mask column per head)
#     va16 same as va in bf16 (for the Schraudolph bf16 AV matmuls)
#     eb  [HPC, TNV, SQ] fp8 (64 * bias^T per head, compacted rows)
#     i2  [128, 256] fp8   (DoubleRow identity: [:, :128]=I, [:, 128:]=0)
#   outputs: ctx_o [128, B*SQ] fp8 = 32*ctx/norm in [p, t, d] layout
# --------------------------------------------------------------------------
def build_phase2(nvts=(8, 8, 8, 8), reps=1, sp_bufs=6, cp_bufs=2, wm_bufs=4):
    nc = bacc.Bacc("TRN2", debug=False, num_devices=NCORES)
    QC = 512
    NQC = SQ // QC
    snvt = [0]
    for t in nvts:
        snvt.append(snvt[-1] + t)
    TNT = snvt[-1]
    TNV = TNT * 128
    NTMAX = max(nvts)

    qT = nc.dram_tensor("qT", [128, B * SQ], dt.float8e4, kind="ExternalInput").ap()
    kT = nc.dram_tensor("kT", [128, TNV], dt.float8e4, kind="ExternalInput").ap()
    va = nc.dram_tensor("va", [TNV, HPC * (HD + 1)], dt.float8e4, kind="ExternalInput").ap()
    va16 = nc.dram_tensor("va16", [TNV, HPC * (HD + 1)], dt.bfloat16, kind="ExternalInput").ap()
    eb = nc.dram_tensor("eb", [HPC, TNV, SQ], dt.float8e4, kind="ExternalInput").ap()
    i2 = nc.dram_tensor("i2", [128, 256], dt.float8e4, kind="ExternalInput").ap()
    ctx_o = nc.dram_tensor("ctx_o", [128, B * SQ], dt.float8e4, kind="ExternalOutput").ap()

    with tile.TileContext(nc) as tc:
        with (
            tc.tile_pool(name="big", bufs=1) as bigp,
            tc.tile_pool(name="ebp", bufs=5) as ebp,
            tc.tile_pool(name="wp", bufs=wm_bufs) as wp,
            tc.tile_pool(name="ip", bufs=wm_bufs) as ip,
            tc.tile_pool(name="np_", bufs=6) as normp,
            tc.tile_pool(name="Sp", bufs=sp_bufs, space="PSUM") as Sp,
            tc.tile_pool(name="cp", bufs=cp_bufs, space="PSUM") as cp,
        ):
            # hd-split layouts for DoubleRow: [32 partitions, 2 slots, head, cols]
            qT_sb = bigp.tile([32, 2, HPC, B * SQ], dt.float8e4)
            kT_sb = bigp.tile([32, 2, HPC, TNV], dt.float8e4)
            va_sb = bigp.tile([128, TNT, HPC * (HD + 1)], dt.float8e4)
            va16_sb = bigp.tile([128, TNT, HPC * (HD + 1)], dt.bfloat16)
            i2_sb = bigp.tile([128, 2, 128], dt.float8e4)
            warm = bigp.tile([1, 1], dt.float32)
            nc.vector.memset(warm[:], 0.0)
            warm2 = bigp.tile([1, 1], dt.float32)
            nc.scalar.activation(warm2[:], warm[:], AF.Exp)

            def load_qk_h(b, h):
                nc.sync.dma_start(
                    qT_sb[:, :, h, b * SQ : (b + 1) * SQ],
                    qT[64 * h : 64 * h + 64, b * SQ : (b + 1) * SQ].rearrange(
                        "(s p) c -> p s c", p=32
                    ),
                )
                cs, ce = snvt[b] * 128, snvt[b + 1] * 128
                nc.sync.dma_start(
                    kT_sb[:, :, h, cs:ce],
                    kT[64 * h : 64 * h + 64, cs:ce].rearrange("(s p) c -> p s c", p=32),
                )

            def load_qk(b):
                for h in range(HPC):
                    load_qk_h(b, h)

            def load_va(b):
                cs, ce = snvt[b] * 128, snvt[b + 1] * 128
                nc.sync.dma_start(
                    va_sb[:, snvt[b] : snvt[b + 1], :],
                    va[cs:ce, :].rearrange("(t p) d -> p t d", p=128),
                )
                nc.sync.dma_start(
                    va16_sb[:, snvt[b] : snvt[b + 1], :],
                    va16[cs:ce, :].rearrange("(t p) d -> p t d", p=128),
                )

            def load_b(b):
                load_qk(b)
                load_va(b)

            nc.scalar.dma_start(i2_sb[:], i2[:])
            load_qk(0)

            iters = [(qc, b) for qc in range(NQC) for b in range(B)] * reps

            def load_slab(qc, b, split=False, kj_range=None, eng=None):
                NT = nvts[b]
                eng = eng or nc.gpsimd
                eb_sb = ebp.tile(
                    [128, NTMAX + 1, HPC, QC], dt.float8e4, name="eb_sb", tag="eb"
                )
                src_r = eb[:, snvt[b] * 128 : snvt[b + 1] * 128, :].rearrange(
                    "h (t p) q -> h p t q", p=128
                )[:, :, :, qc * QC : (qc + 1) * QC]

                def emit(kjs):
                    for kj in kjs:
                        for h in range(HPC):
                            eng.dma_start(eb_sb[:, kj, h, :], src_r[h, :, kj, :])

                if split:
                    emit(range(NT) if kj_range is None else kj_range)
                else:
                    for h in range(HPC):
                        nc.sync.dma_start(eb_sb[:, 0:NT, h, :], src_r[h])
                # pad tile (read by the DR inject's zero slot on the last key
                # tile) must be initialized for the race detector
                if kj_range is None or list(kj_range)[-1] == NT - 1:
                    nc.gpsimd.memset(eb_sb[:, NT, :, :], 0.0)
                return eb_sb, emit

            slabs = {}
            # first two key tiles of iteration 0 land before the va bulk loads
            eb0, emit0 = load_slab(*iters[0], split=True, kj_range=range(2), eng=nc.scalar)
            load_va(0)
            emit0(range(2, nvts[iters[0][1]]))
            nc.gpsimd.memset(eb0[:, nvts[iters[0][1]], :, :], 0.0)
            slabs[0] = eb0
            for b in range(1, B):
                load_b(b)
                slabs[b], _ = load_slab(*iters[b], split=(b == 1))

            def emit_norm_piece(state):
                # mask column is 1/32, so 1/normcol = 32/sum(w): the x32 ctx
                # scaling is free. Pieces alternate DVE / Act to balance load.
                ctx, col0, holder = state
                if holder[0] is None:
                    holder[0] = normp.tile(
                        [128, QC // 128, HPC * HD], dt.float8e4, name="ctxn", tag="ctxn"
                    )
                ctxn = holder[0]
                t = holder[1]
                holder[1] += 1
                ti, tt = t // 2, t % 2
                if tt == 0:
                    # one strided reciprocal covers all 4 norm scalars of this
                    # ctx tile (2 tt x 2 heads) instead of 4 tiny ops
                    rec4 = normp.tile([128, 2, HPC], dt.float32, name="rec4", tag=f"rec{ti}")
                    nc.vector.reciprocal(
                        rec4[:], ctx[ti][:, :, HD :: HD + 1]
                    )
                    holder[2 + ti] = rec4
                rec4 = holder[2 + ti]
                for h in range(HPC):
                    if t < 2 or (t == 2 and h == 0):
                        nc.vector.tensor_scalar(
                            out=ctxn[:, t, h * HD : (h + 1) * HD],
                            in0=ctx[ti][:, tt, h * (HD + 1) : h * (HD + 1) + HD],
                            scalar1=rec4[:, tt, h : h + 1],
                            scalar2=None,
                            op0=ALU.mult,
                        )
                    else:
                        nc.scalar.activation(
                            ctxn[:, t, h * HD : (h + 1) * HD],
                            ctx[ti][:, tt, h * (HD + 1) : h * (HD + 1) + HD],
                            AF.Copy,
                            scale=rec4[:, tt, h : h + 1],
                        )
                if t == QC // 128 - 1:
                    nc.sync.dma_start(ctx_o[:, col0 : col0 + QC], ctxn[:])

            def emit_norm(state):
                while state[2][1] < QC // 128:
                    emit_norm_piece(state)

            def emit_av_pair(ctx, tbase, pj, wm2, start, stop):
                # DoubleRow fp8 AV over a kj pair
                for ti in range(QC // 256):
                    for tt in range(2):
                        for h in range(HPC):
                            t = ti * 2 + tt
                            nc.tensor.matmul(
                                ctx[ti][:, tt, h * (HD + 1) : (h + 1) * (HD + 1)],
                                lhsT=wm2[:, :, h, t * 128 : (t + 1) * 128],
                                rhs=va_sb[:, tbase + 2 * pj : tbase + 2 * pj + 2,
                                          h * (HD + 1) : (h + 1) * (HD + 1)],
                                start=start and (tt == 0) and (h == 0),
                                stop=stop and (ti == QC // 256 - 1) and (tt == 1) and (h == HPC - 1),
                                perf_mode=MPM.DoubleRow,
                                skip_group_check=True,
                            )

            def emit_av_sch(ctx, tbase, kj, i16, sl, start, stop):
                # plain bf16 AV for one Schraudolph kj tile (bitcast int16 weights)
                for ti in range(QC // 256):
                    for tt in range(2):
                        for h in range(HPC):
                            t = ti * 2 + tt
                            nc.tensor.matmul(
                                ctx[ti][:, tt, h * (HD + 1) : (h + 1) * (HD + 1)],
                                lhsT=i16[:, sl, h, t * 128 : (t + 1) * 128].bitcast(dt.bfloat16),
                                rhs=va16_sb[:, tbase + kj, h * (HD + 1) : (h + 1) * (HD + 1)],
                                start=start and (tt == 0) and (h == 0),
                                stop=stop and (ti == QC // 256 - 1) and (tt == 1) and (h == HPC - 1),
                                skip_group_check=True,
                            )

            def emit_av_one(ctx, tbase, kj, wm1, start, stop):
                # plain fp8 AV for the odd tail tile
                for ti in range(QC // 256):
                    for tt in range(2):
                        for h in range(HPC):
                            t = ti * 2 + tt
                            nc.tensor.matmul(
                                ctx[ti][:, tt, h * (HD + 1) : (h + 1) * (HD + 1)],
                                lhsT=wm1[:, 0, h, t * 128 : (t + 1) * 128],
                                rhs=va_sb[:, tbase + kj, h * (HD + 1) : (h + 1) * (HD + 1)],
                                start=start and (tt == 0) and (h == 0),
                                stop=stop and (ti == QC // 256 - 1) and (tt == 1) and (h == HPC - 1),
                                skip_group_check=True,
                            )

            tail_av = []     # AV thunks deferred from the previous kj
            tail_norm = None

            for it_i, (qc, b) in enumerate(iters):
                NT = nvts[b]
                NP = NT // 2
                eb_sb = slabs.pop(it_i)
                if it_i + 4 < len(iters):
                    slabs[it_i + 4], _ = load_slab(*iters[it_i + 4])
                ctx = [
                    cp.tile([128, 2, HPC * (HD + 1)], dt.float32, name=f"ctx{t}", tag="ctx")
                    for t in range(QC // 256)
                ]
                col0 = b * SQ + qc * QC
                tbase = snvt[b]

                def make_S(kj):
                    # per-head 1-bank S tiles -> deeper PSUM pipeline
                    Ss = []
                    kcol = tbase * 128 + kj * 128
                    for h in range(HPC):
                        S = Sp.tile([128, QC], dt.float32, name="S", tag="S")
                        nc.tensor.matmul(
                            S[:],
                            lhsT=kT_sb[:, :, h, kcol : kcol + 128],
                            rhs=qT_sb[:, :, h, col0 : col0 + QC],
                            start=True,
                            stop=False,
                            perf_mode=MPM.DoubleRow,
                            skip_group_check=True,
                        )
                        nc.tensor.matmul(
                            S[:],
                            lhsT=i2_sb[:],
                            rhs=eb_sb[:, kj : kj + 2, h, :],
                            start=False,
                            stop=True,
                            perf_mode=MPM.DoubleRow,
                            skip_group_check=True,
                        )
                        Ss.append(S)
                    return Ss

                first_av = [True]
                wm2_cur = [None]
                i16_cur = [None]
                for kj in range(NT):
                    S = make_S(kj)
                    # drain the deferred AVs / previous iteration's norm
                    if tail_av:
                        fin = (kj == 0)
                        for j, (fn, args) in enumerate(tail_av):
                            fn(*args, stop=(fin and j == len(tail_av) - 1) if fin else False)
                        tail_av = []
                    if tail_norm is not None and kj >= 1:
                        emit_norm_piece(tail_norm)
                        if tail_norm[2][1] >= QC // 128:
                            tail_norm = None
                    pj = kj // 2
                    is_odd_tail = (kj == NT - 1) and (NT % 2 == 1)
                    path = "act" if (is_odd_tail or pj % 2 == 0) else "sch"
                    if path == "act":
                        if is_odd_tail:
                            wm1 = wp.tile([128, 1, HPC, QC], dt.float8e4, name="wm1", tag="wm1")
                            for h in range(HPC):
                                nc.scalar.activation(
                                    wm1[:, 0, h], S[h][:], AF.Exp, scale=1.0 / 64.0
                                )
                            tail_av.append((emit_av_one, [ctx, tbase, kj, wm1, first_av[0]]))
                            first_av[0] = False
                        else:
                            if kj % 2 == 0:
                                wm2_cur[0] = wp.tile(
                                    [128, 2, HPC, QC], dt.float8e4, name="wm2", tag="wm2"
                                )
                            for h in range(HPC):
                                nc.scalar.activation(
                                    wm2_cur[0][:, kj % 2, h], S[h][:], AF.Exp, scale=1.0 / 64.0
                                )
                            if kj % 2 == 1:
                                tail_av.append(
                                    (emit_av_pair, [ctx, tbase, pj, wm2_cur[0], first_av[0]])
                                )
                                first_av[0] = False
                    else:
                        if kj % 2 == 0 or is_odd_tail:
                            i16_cur[0] = ip.tile(
                                [128, 2, HPC, QC], dt.int16, name="i16", tag="i16"
                            )
                        sl_ = 0 if is_odd_tail else kj % 2
                        for h in range(HPC):
                            nc.vector.tensor_scalar(
                                out=i16_cur[0][:, sl_, h], in0=S[h][:], scalar1=SCH_A / 64.0,
                                scalar2=SCH_B, op0=ALU.mult, op1=ALU.add,
                            )
                        tail_av.append(
                            (emit_av_sch, [ctx, tbase, kj, i16_cur[0], sl_, first_av[0]])
                        )
                        first_av[0] = False

                if tail_norm is not None:
                    emit_norm(tail_norm)   # short iterations: flush leftovers
                tail_norm = (ctx, col0, [None, 0, None, None])

            for j, (fn, args) in enumerate(tail_av):
                fn(*args, stop=(j == len(tail_av) - 1))
            emit_norm(tail_norm)

    nc.compile()
    return nc


# --------------------------------------------------------------------------
# Phase 3: out projection + residual + LayerNorm (row-parallel, fp8 DR GEMM).
#   inputs (per core): ctxT [D, RPC] fp8 (=32*ctx^T), woT [D, D] fp8 (=64*Wo^T),
#     resid [RPC, D] bf16 (query rows + bo), [gammab/betab [128, D] f32 if
#     not trivial_ln]
#   outputs: out_o [RPC, D] f32
# --------------------------------------------------------------------------
def build_phase3(trivial_ln=True, reps=1):
    nc = bacc.Bacc("TRN2", debug=False, num_devices=NCORES)
    KC = D // 128

    ctxT = nc.dram_tensor("ctxT", [D, RPC], dt.float8e4, kind="ExternalInput").ap()
    woT = nc.dram_tensor("woT", [D, D], dt.float8e4, kind="ExternalInput").ap()
    resid = nc.dram_tensor("resid", [RPC, D], dt.bfloat16, kind="ExternalInput").ap()
    if not trivial_ln:
        gammab = nc.dram_tensor("gammab", [128, D], dt.float32, kind="ExternalInput").ap()
        betab = nc.dram_tensor("betab", [128, D], dt.float32, kind="ExternalInput").ap()
    out_o = nc.dram_tensor("out_o", [RPC, D], dt.float32, kind="ExternalOutput").ap()
    PS_SCALE = 1.0 / (32.0 * 64.0)

    with tile.TileContext(nc) as tc:
        with (
            tc.tile_pool(name="big", bufs=1) as bigp,
            tc.tile_pool(name="rp", bufs=4) as rp,
            tc.tile_pool(name="wk", bufs=3) as wk,
            tc.tile_pool(name="ps", bufs=6, space="PSUM") as psp,
        ):
            ctx_sb = bigp.tile([128, KC, RPC], dt.float8e4)
            wo_sb = bigp.tile([128, KC, D], dt.float8e4)
            nc.sync.dma_start(
                ctx_sb[:], ctxT[:, :].rearrange("(k p) c -> p k c", p=128)
            )
            nc.sync.dma_start(
                wo_sb[:], woT[:, :].rearrange("(k p) c -> p k c", p=128)
            )
            eps_sb = bigp.tile([128, 1], dt.float32)
            nc.vector.memset(eps_sb[:], LN_EPS)
            warm = bigp.tile([1, 1], dt.float32)
            nc.vector.memset(warm[:], 1.0)
            warm2 = bigp.tile([1, 1], dt.float32)
            nc.scalar.activation(warm2[:], warm[:], AF.Sqrt)
            warm3 = bigp.tile([1, 1], dt.float32)
            nc.scalar.activation(warm3[:], warm[:], AF.Square)
            if not trivial_ln:
                gam_sb = bigp.tile([128, D], dt.float32)
                nc.sync.dma_start(gam_sb[:], gammab[:])
                bet_sb = bigp.tile([128, D], dt.float32)
                nc.sync.dma_start(bet_sb[:], betab[:])

            for m in [m for _ in range(reps) for m in range(RPC // 128)]:
                res_sb = rp.tile([128, D], dt.bfloat16, name="res_sb", tag="res")
                nc.sync.dma_start(res_sb[:], resid[m * 128 : (m + 1) * 128, :])
                ps = [psp.tile([128, 512], dt.float32, name=f"ps{n}", tag="ps") for n in range(2)]
                for n in range(2):
                    for k2 in range(KC // 2):
                        nc.tensor.matmul(
                            ps[n][:],
                            lhsT=ctx_sb[:, 2 * k2 : 2 * k2 + 2, m * 128 : (m + 1) * 128],
                            rhs=wo_sb[:, 2 * k2 : 2 * k2 + 2, n * 512 : (n + 1) * 512],
                            start=(k2 == 0),
                            stop=(k2 == KC // 2 - 1),
                            perf_mode=MPM.DoubleRow,
                        )
                x_sb = wk.tile([128, D], dt.float32, name="x_sb", tag="x")
                acc = [wk.tile([128, 1], dt.float32, name=f"acc{n}", tag=f"acc{n}") for n in range(2)]
                for n in range(2):
                    nc.vector.scalar_tensor_tensor(
                        out=x_sb[:, n * 512 : (n + 1) * 512],
                        in0=ps[n][:],
                        scalar=PS_SCALE,
                        in1=res_sb[:, n * 512 : (n + 1) * 512],
                        op0=ALU.mult,
                        op1=ALU.add,
                        accum_out=acc[n][:],
                    )
                mu = wk.tile([128, 1], dt.float32, name="mu", tag="mu")
                nc.vector.tensor_scalar(
                    out=mu[:], in0=acc[0][:], scalar1=acc[1][:], scalar2=1.0 / D,
                    op0=ALU.add, op1=ALU.mult,
                )
                sq = wk.tile([128, D], dt.bfloat16, name="sq", tag="sq")
                s2 = wk.tile([128, 1], dt.float32, name="s2", tag="s2")
                nc.scalar.activation(sq[:], x_sb[:], AF.Square, accum_out=s2[:])
                var = wk.tile([128, 1], dt.float32, name="var", tag="var")
                # var = s2/D - mu^2  (one fused op: (s2*(1/D)) - mu2)
                mu2 = wk.tile([128, 1], dt.float32, name="mu2", tag="mu2")
                nc.vector.tensor_tensor(mu2[:], mu[:], mu[:], op=ALU.mult)
                nc.vector.tensor_scalar(
                    out=var[:], in0=s2[:], scalar1=1.0 / D, scalar2=mu2[:],
                    op0=ALU.mult, op1=ALU.subtract,
                )
                std = wk.tile([128, 1], dt.float32, name="std", tag="std")
                nc.scalar.activation(std[:], var[:], AF.Sqrt, bias=eps_sb[:])
                rstd = wk.tile([128, 1], dt.float32, name="rstd", tag="rstd")
                nc.vector.reciprocal(rstd[:], std[:])
                mrs = wk.tile([128, 1], dt.float32, name="mrs", tag="mrs")
                nc.vector.tensor_tensor(mrs[:], mu[:], rstd[:], op=ALU.mult)
                out_sb = wk.tile([128, D], dt.float32, name="out_sb", tag="out_sb")
                if trivial_ln:
                    nc.vector.tensor_scalar(
                        out=out_sb[:], in0=x_sb[:], scalar1=rstd[:], scalar2=mrs[:],
                        op0=ALU.mult, op1=ALU.subtract,
                    )
                else:
                    tmp = wk.tile([128, D], dt.float32, name="tmp", tag="tmp")
                    nc.vector.tensor_scalar(
                        out=tmp[:], in0=x_sb[:], scalar1=rstd[:], scalar2=mrs[:],
                        op0=ALU.mult, op1=ALU.subtract,
                    )
                    y = wk.tile([128, D], dt.float32, name="y", tag="y")
                    nc.vector.scalar_tensor_tensor(
                        out=y[:], in0=tmp[:], scalar=0.0, in1=gam_sb[:],
                        op0=ALU.add, op1=ALU.mult,
                    )
                    nc.gpsimd.tensor_add(out_sb[:], y[:], bet_sb[:])
                nc.sync.dma_start(out_o[m * 128 : (m + 1) * 128, :], out_sb[:])

    nc.compile()
    return nc


def _get_program(key, builder, *args, **kwargs):
    if key not in _programs:
        _programs[key] = builder(*args, **kwargs)
    return _programs[key]


def _run(nc, in_maps):
    return bass_utils.run_bass_kernel_spmd(nc, in_maps, core_ids=list(range(NCORES)))


def kernel(query, key, value, attention_mask, relative_position_bias,
           Wq, bq, Wk, bk, Wv, bv, Wo, bo, ln_gamma, ln_beta,
           _collect_results=None):
    query = np.asarray(query, dtype=np.float32)
    key = np.asarray(key, dtype=np.float32)
    value = np.asarray(value, dtype=np.float32)
    attention_mask = np.asarray(attention_mask)
    relative_position_bias = np.asarray(relative_position_bias, dtype=np.float32)

    def xT8(x):
        return np.ascontiguousarray(x.reshape(-1, D).T).astype(F8)

    def wT8(W, scale):
        return (np.ascontiguousarray(np.asarray(W, np.float32).T) * scale).astype(F8)

    xqT = xT8(query)
    xkT = xT8(key)
    xvT = xT8(value)
    wqT = wT8(Wq, 64.0)
    wkT = wT8(Wk, 64.0)
    wvT = wT8(Wv, 64.0)

    # ---------------- phase 1 ----------------
    in1 = []
    for c in range(NCORES):
        sl = slice(c * RPC, (c + 1) * RPC)
        in1.append({
            "xqT": np.ascontiguousarray(xqT[:, sl]),
            "xkT": np.ascontiguousarray(xkT[:, sl]),
            "xvT": np.ascontiguousarray(xvT[:, sl]),
            "wqT": wqT, "wkT": wkT, "wvT": wvT,
        })
    r1 = _run(_get_program("p1", build_phase1), in1)

    qT_full = np.empty((D, B * SQ), dtype=F8)
    kT_full = np.empty((D, B * SK), dtype=F8)
    v_full = np.empty((B * SK, D), dtype=F8)
    for c in range(NCORES):
        sl = slice(c * RPC, (c + 1) * RPC)
        qT_full[:, sl] = r1.results[c]["qT_o"]
        kT_full[:, sl] = r1.results[c]["kT_o"]
        v_full[sl, :] = r1.results[c]["v_o"]

    # fold any nonzero projection biases in on the host (zero in practice)
    if np.any(np.asarray(bq)):
        qT_full = (qT_full.astype(np.float32)
                   + 4.0 * np.asarray(bq, np.float32)[:, None]).astype(F8)
    if np.any(np.asarray(bk)):
        kT_full = (kT_full.astype(np.float32)
                   + 2.0 * np.asarray(bk, np.float32)[:, None]).astype(F8)
    if np.any(np.asarray(bv)):
        v_full = (v_full.astype(np.float32)
                  + np.asarray(bv, np.float32)[None, :]).astype(F8)

    # ---------------- phase 2 ----------------
    mask2 = (attention_mask.reshape(B, SK) != 0)
    valid = [np.nonzero(mask2[b])[0] for b in range(B)]
    nvts = tuple(max(1, -(-len(ix) // 128)) for ix in valid)
    snvt = np.concatenate([[0], np.cumsum(nvts)]).astype(int)
    TNT = int(snvt[-1])
    idx_pad = np.zeros(TNT * 128, dtype=np.int64)
    maskc = np.zeros((TNT * 128,), dtype=bool)
    for b in range(B):
        ix = valid[b]
        o = snvt[b] * 128
        idx_pad[o : o + len(ix)] = ix
        maskc[o : o + len(ix)] = True

    col_idx = (np.repeat(np.arange(B) * SK, np.array(nvts) * 128) + idx_pad)
    kT_c = np.ascontiguousarray(kT_full[:, col_idx])
    v_rows = v_full[col_idx, :]
    va_all = np.zeros((TNT * 128, H * (HD + 1)), dtype=F8)
    inv32 = np.asarray(1.0 / 32.0, dtype=F8)[()]
    for h in range(H):
        blk = np.where(maskc[:, None], v_rows[:, h * HD : (h + 1) * HD], np.zeros((), F8))
        va_all[:, h * (HD + 1) : h * (HD + 1) + HD] = blk
        va_all[:, h * (HD + 1) + HD] = np.where(maskc, inv32, np.zeros((), F8))

    ebT8 = (np.ascontiguousarray(
        relative_position_bias[0].transpose(0, 2, 1)) * 64.0).astype(F8)
    eb_c = ebT8[:, idx_pad, :]  # [H, TNV, SQ] fp8

    i2_host = np.zeros((128, 256), dtype=F8)
    i2_host[:, 0:128] = np.eye(128, dtype=np.float32).astype(F8)

    in2 = []
    for c in range(NCORES):
        rs = slice(c * 128, (c + 1) * 128)
        in2.append({
            "qT": np.ascontiguousarray(qT_full[rs, :]),
            "kT": np.ascontiguousarray(kT_c[rs, :]),
            "va": np.ascontiguousarray(
                va_all[:, c * HPC * (HD + 1) : (c + 1) * HPC * (HD + 1)]
            ),
            "va16": np.ascontiguousarray(
                va_all[:, c * HPC * (HD + 1) : (c + 1) * HPC * (HD + 1)]
            ).astype(BF16),
            "eb": np.ascontiguousarray(eb_c[c * HPC : (c + 1) * HPC]),
            "i2": i2_host,
        })
    r2 = _run(_get_program(("p2",) + nvts, build_phase2, nvts), in2)

    # ctx_o[c] is [128 q-part, t, 128 d] for d-block c -> assemble ctxT [D, B*SQ]
    ctxT_full = np.empty((D, B * SQ), dtype=F8)
    for c in range(NCORES):
        blk = r2.results[c]["ctx_o"].reshape(128, B * SQ // 128, 128)
        ctxT_full[c * 128 : (c + 1) * 128, :] = (
            blk.transpose(2, 1, 0).reshape(128, B * SQ)
        )

    # ---------------- phase 3 ----------------
    woT8 = wT8(Wo, 64.0)
    q2d = query.reshape(-1, D)
    resid_h = (q2d + np.asarray(bo, np.float32)[None, :]).astype(BF16)
    trivial = (not np.any(np.asarray(ln_beta))) and np.all(
        np.asarray(ln_gamma, np.float32) == 1.0
    )
    in3 = []
    for c in range(NCORES):
        sl = slice(c * RPC, (c + 1) * RPC)
        d = {
            "ctxT": np.ascontiguousarray(ctxT_full[:, sl]),
            "woT": woT8,
            "resid": np.ascontiguousarray(resid_h[sl, :]),
        }
        if not trivial:
            d["gammab"] = np.ascontiguousarray(
                np.broadcast_to(np.asarray(ln_gamma, np.float32)[None, :], (128, D))
            )
            d["betab"] = np.ascontiguousarray(
                np.broadcast_to(np.asarray(ln_beta, np.float32)[None, :], (128, D))
            )
        in3.append(d)
    r3 = _run(_get_program(("p3", trivial), build_phase3, trivial), in3)

    out = np.empty((B * SQ, D), dtype=np.float32)
    for c in range(NCORES):
        out[c * RPC : (c + 1) * RPC, :] = r3.results[c]["out_o"]

    if _collect_results is not None:
        _collect_results.extend([r1, r2, r3])
    return out.reshape(B, SQ, D)
